# revision 1
# baseline (speedup 1.0000x reference)
"""DWDN forward (Wiener deconvolution) Trainium2 Bass kernel.

Sharding: 8 cores = 4 batches x 2 channel-halves. Core k handles batch
b=k//2, reduced-channel half h=k%2 (32 of the 64 reduced channels).

The axon link to the cores moves ~70-110 MB/s half-duplex and every
program dispatch costs a fixed ~75 ms that serializes server-side (a
trivial program costs the same), so the design minimizes per-call
host<->device bytes (wf/clear are the rank-64 cut of the dataflow;
fp16 is the smallest dtype that holds the 2e-2 error gate) and uses
FOUR dispatches on disjoint 2-core meshes (batch b on cores 2b, 2b+1):
each batch's download starts while later batches' uploads still
stream, so the serialized fixed dispatch costs and the expand sgemms
hide inside the wire windows:
  host:   1x1 reduce conv (BLAS sgemm) -> wf [4,64,256,256], + bias,
          fp16 pack with replicated border rows -> 34 MB upload,
          streamed per-core from worker threads while later batches'
          sgemms run
  device: P0 OTF via small DFT matmuls (otf = E @ kern @ E^T), D=|otf|^2
          P2 per channel image: 3x3-median NSR stats (fp16 min/max
          network with border-band decomposition) + Wiener deconvolution
          as pad-folded DFT matmuls with Hermitian symmetry (freq rows
          k=0..147 of 294); emits clear fp16 -> 34 MB download
  host:   1x1 expand conv + bias (sgemm with ones-row augmentation),
          run per batch inside the fetch worker that lands the batch's
          second shard, so expands stream in completion order
The sharded jit executable, DFT constants, and zero output buffers stay
device-resident across calls; only wf16 + kern move per call.

All matmuls in float32r (13-bit mantissa, full PE rate). No ACT LUT
activations and no custom-DVE ops (both crash on this deployment);
division via nc.vector.reciprocal only.
"""

import os
import sys
import threading
from contextlib import ExitStack

import numpy as np

for _p in ("/opt/trn_rl_repo", "/root/.axon_site/_ro/trn_rl_repo"):
    if os.path.isdir(_p) and _p not in sys.path:
        sys.path.insert(0, _p)

import concourse.tile as tile  # noqa: E402
from concourse import bacc, mybir  # noqa: E402

F32 = mybir.dt.float32
F32R = mybir.dt.float32r
F16 = mybir.dt.float16
AT = mybir.AluOpType
AX = mybir.AxisListType

NF, KS, B, H, W = 256, 19, 4, 256, 256
N = H + 2 * KS          # 294
C = 64                  # reduced channels
CH = 32                 # channels per core
KH = N // 2 + 1         # 148 (Hermitian half)
NPIX = H * W
NTOT = N * N
HB = H + 2              # bordered image rows for the median path
LT = [128, 128, 38]     # partition-tile sizes for 294
MT = [128, 20]          # partition-tile sizes for 148


def _round_f32r(x):
    b = np.ascontiguousarray(x, dtype=np.float32).view(np.uint32).astype(np.uint64)
    b = (b + 0x200) & np.uint64(0xFFFFFC00)
    b = np.minimum(b, 0xFFFFFFFF).astype(np.uint32)
    return b.view(np.float32)


def _host_constants():
    k = np.arange(N)
    Fm = np.exp(-2j * np.pi * np.outer(k, k) / N)
    Sx = np.zeros((N, H))
    for i in range(N):
        Sx[i, min(max(i - KS, 0), H - 1)] = 1.0
    A = Fm @ Sx                                   # [294, 256]
    At = A.T                                      # [256, 294]
    B1 = np.conj(Fm)[KS:KS + H, :] / N            # [256, 294]
    wk = np.ones(KH)
    wk[1:KH - 1] = 2.0
    B4 = B1[:, :KH] * wk[None, :]                 # [256, 148]
    E = np.exp(-2j * np.pi * np.outer(k, (np.arange(KS) - KS // 2)) / N)  # [294,19]
    f = lambda v: _round_f32r(np.ascontiguousarray(v, np.float32))
    cst = {
        "AtR": f(At.real), "AtI": f(At.imag), "AtIn": f(-At.imag),
        "B1tR": f(B1.T.real), "B1tI": f(B1.T.imag), "B1tIn": f(-B1.T.imag),
        "B4R": f(B4.T.real), "B4In": f(-B4.T.imag),
        "ER": f(E.T.real), "EI": f(E.T.imag), "EIn": f(-E.T.imag),
        "onesr": f(np.ones((1, 128), np.float32)),
        "onesc": f(np.ones((128, 1), np.float32)),
    }
    u = np.ones((H, 1), np.float32)
    u[0, 0] = 20.0
    u[-1, 0] = 20.0
    cst["u16"] = u.astype(np.float16)
    return cst


_CACHE = {}


def _build():
    nc = bacc.Bacc(None, target_bir_lowering=False)
    P = {}
    P["wf16"] = nc.declare_dram_parameter("wf16", [CH, HB * W], F16, isOutput=False)
    P["kern"] = nc.declare_dram_parameter("kern", [KS, KS], F32R, isOutput=False)
    for nm, shp in [("AtR", [NF, N]), ("AtI", [NF, N]), ("AtIn", [NF, N]),
                    ("B1tR", [N, H]), ("B1tI", [N, H]), ("B1tIn", [N, H]),
                    ("B4R", [KH, H]), ("B4In", [KH, H]),
                    ("ER", [KS, N]), ("EI", [KS, N]), ("EIn", [KS, N]),
                    ("onesc", [128, 1])]:
        P[nm] = nc.declare_dram_parameter(nm, shp, F32R, isOutput=False)
    P["onesr"] = nc.declare_dram_parameter("onesr", [1, 128], F32, isOutput=False)
    P["u16"] = nc.declare_dram_parameter("u16", [H, 1], F16, isOutput=False)
    clear16_d = nc.declare_dram_parameter("clear16", [CH, NPIX], F16, isOutput=True)

    with tile.TileContext(nc) as tc:
        _emit(nc, tc, P, clear16_d)
    nc.finalize()
    return nc


def _load_consts(nc, tc, ctx, P):
    cpool = ctx.enter_context(tc.tile_pool(name="consts", bufs=1))
    CT = {}
    for nm in ("AtR", "AtI", "AtIn"):
        t = cpool.tile([128, 2 * N], F32R, tag=nm)
        for i in range(2):
            nc.sync.dma_start(t[:, i * N:(i + 1) * N], P[nm][i * 128:(i + 1) * 128, :])
        CT[nm] = t
    for nm in ("B1tR", "B1tI", "B1tIn"):
        t = cpool.tile([128, 3 * H], F32R, tag=nm)
        for i in range(3):
            rs = LT[i]
            nc.sync.dma_start(t[0:rs, i * H:i * H + H], P[nm][i * 128:i * 128 + rs, :])
        CT[nm] = t
    for nm in ("B4R", "B4In"):
        t = cpool.tile([128, 2 * H], F32R, tag=nm)
        for i in range(2):
            rs = MT[i]
            nc.sync.dma_start(t[0:rs, i * H:i * H + H], P[nm][i * 128:i * 128 + rs, :])
        CT[nm] = t
    for nm in ("ER", "EI", "EIn"):
        t = cpool.tile([KS, N], F32R, tag=nm)
        nc.sync.dma_start(t[:], P[nm][:, :])
        CT[nm] = t
    for nm, shp, dt in [("onesr", [1, 128], F32), ("onesc", [128, 1], F32R)]:
        t = cpool.tile(shp, dt, tag=nm)
        nc.sync.dma_start(t[:], P[nm][:, :])
        CT[nm] = t
    u16 = cpool.tile([128, 2], F16, tag="u16")
    for i in range(2):
        nc.sync.dma_start(u16[:, i:i + 1], P["u16"][i * 128:(i + 1) * 128, :])
    CT["u16"] = u16
    kern_sb = cpool.tile([KS, KS], F32R, tag="kern")
    nc.sync.dma_start(kern_sb[:], P["kern"][:, :])
    CT["kern"] = kern_sb
    # persistent P0 outputs
    CT["otfR"] = cpool.tile([128, 3 * KH], F32, name="otfR", tag="otfR")
    CT["otfI"] = cpool.tile([128, 3 * KH], F32, name="otfI", tag="otfI")
    CT["Dt"] = cpool.tile([128, 3 * KH], F32, name="Dtt", tag="Dtt")
    return CT


def _emit(nc, tc, P, clear16_d):
    with ExitStack() as ctx:
        CT = _load_consts(nc, tc, ctx, P)

        # ---- P0 (scoped pools) ----
        with tc.tile_pool(name="p01", bufs=3) as wp, \
             tc.tile_pool(name="p01ps", bufs=1, space="PSUM") as pp:
            _p0_otf(nc, wp, pp, CT)

        tc.strict_bb_all_engine_barrier()

        # ---- P2 ----
        with tc.tile_pool(name="sw", bufs=2) as spool, \
             tc.tile_pool(name="med", bufs=2) as mpool, \
             tc.tile_pool(name="p2ps", bufs=1, space="PSUM") as ppool:
            for ci in range(CH):
                _image(nc, spool, mpool, ppool, CT, P["wf16"], clear16_d, ci)


def _p0_otf(nc, wp, pp, CT):
    # Tt[j,k] = sum_i kern[i,j] E[k,i]: lhsT=kern [19,19], rhs=ER/EI [19,294]
    ps_a = pp.tile([KS, N], F32, tag="ps_otfa")
    ps_b = pp.tile([KS, N], F32, tag="ps_otfb")
    nc.tensor.matmul(ps_a[:], CT["kern"][:], CT["ER"][:], start=True, stop=True)
    nc.tensor.matmul(ps_b[:], CT["kern"][:], CT["EI"][:], start=True, stop=True)
    TtR = wp.tile([KS, N], F32R, tag="TtR")
    TtI = wp.tile([KS, N], F32R, tag="TtI")
    nc.vector.tensor_copy(TtR[:], ps_a[:])
    nc.vector.tensor_copy(TtI[:], ps_b[:])
    # otf_t[l,k] = sum_j E[l,j] Tt[j,k]: lhsT = E^T-as-[j,l] = ER/EI slices
    for lt in range(3):
        rs = LT[lt]
        lsl = slice(lt * 128, lt * 128 + rs)
        osl = slice(lt * KH, lt * KH + KH)
        ps_r = pp.tile([128, KH], F32, tag="ps_otfr")
        ps_i = pp.tile([128, KH], F32, tag="ps_otfi")
        nc.tensor.matmul(ps_r[0:rs, :], CT["ER"][:, lsl], TtR[:, 0:KH], start=True, stop=False)
        nc.tensor.matmul(ps_r[0:rs, :], CT["EIn"][:, lsl], TtI[:, 0:KH], start=False, stop=True)
        nc.tensor.matmul(ps_i[0:rs, :], CT["ER"][:, lsl], TtI[:, 0:KH], start=True, stop=False)
        nc.tensor.matmul(ps_i[0:rs, :], CT["EI"][:, lsl], TtR[:, 0:KH], start=False, stop=True)
        nc.vector.tensor_copy(CT["otfR"][0:rs, osl], ps_r[0:rs, :])
        nc.vector.tensor_copy(CT["otfI"][0:rs, osl], ps_i[0:rs, :])
        t2 = wp.tile([128, KH], F32, tag="d_tmp")
        nc.vector.tensor_mul(CT["Dt"][0:rs, osl], CT["otfR"][0:rs, osl], CT["otfR"][0:rs, osl])
        nc.vector.tensor_mul(t2[0:rs, :], CT["otfI"][0:rs, osl], CT["otfI"][0:rs, osl])
        nc.vector.tensor_add(CT["Dt"][0:rs, osl], CT["Dt"][0:rs, osl], t2[0:rs, :])


def _image(nc, spool, mpool, ppool, CT, wf16_p, clear16_d, ci):
    # --- loads: fp16 bordered rows; m1/cc/p1 are row-shifted views.
    # bordered row r: 0 = replicate of image row 0, r = image row r-1,
    # 257 = replicate of image row 255.
    m1 = mpool.tile([128, 2 * W], F16, tag="m1")
    cc = mpool.tile([128, 2 * W], F16, tag="cc")
    p1 = mpool.tile([128, 2 * W], F16, tag="p1s")
    for t in range(2):
        base = t * 128
        for tl, off in ((m1, 0), (cc, 1), (p1, 2)):
            nc.sync.dma_start(
                tl[:, t * W:(t + 1) * W],
                wf16_p[ci, (base + off) * W:(base + off + 128) * W]
                .rearrange("(p w) -> p w", w=W))
    r255 = mpool.tile([1, W], F16, tag="r255")
    nc.sync.dma_start(r255[:], wf16_p[ci, 256 * W:257 * W].rearrange("(a w) -> a w", a=1))
    # f32r image rows for the DFT path (== cc, the center rows)
    wfr = spool.tile([128, 2 * W], F32R, tag="wfimg")
    nc.scalar.copy(wfr[:], cc[:])

    # --- vertical clamped sort3 -> lo/me/hi ---
    lo = mpool.tile([128, 2 * W], F16, tag="lo")
    me = mpool.tile([128, 2 * W], F16, tag="me")
    hi = mpool.tile([128, 2 * W], F16, tag="hi")
    tv = mpool.tile([128, 2 * W], F16, tag="tv")
    mn = mpool.tile([128, 2 * W], F16, tag="mnv")
    mx = mpool.tile([128, 2 * W], F16, tag="mxv")
    nc.vector.tensor_tensor(mn[:], cc[:], p1[:], AT.min)
    nc.vector.tensor_tensor(mx[:], cc[:], p1[:], AT.max)
    nc.vector.tensor_tensor(lo[:], m1[:], mn[:], AT.min)
    nc.vector.tensor_tensor(hi[:], m1[:], mx[:], AT.max)
    nc.vector.tensor_tensor(tv[:], m1[:], mx[:], AT.min)
    nc.vector.tensor_tensor(me[:], tv[:], mn[:], AT.max)

    # --- horizontal stage -> Mx ---
    Ax = mpool.tile([128, 2 * W], F16, tag="Ax")
    Cx = mpool.tile([128, 2 * W], F16, tag="Cx")
    Bx = mpool.tile([128, 2 * W], F16, tag="Bx")
    Mx = mpool.tile([128, 2 * W], F16, tag="Mx")
    h1 = mpool.tile([128, 2 * W], F16, tag="h1")
    h2 = mpool.tile([128, 2 * W], F16, tag="h2")
    for t in range(2):
        o = t * W
        nc.vector.tensor_tensor(h1[:, o:o + W - 1], lo[:, o:o + W - 1], lo[:, o + 1:o + W], AT.max)
        nc.vector.tensor_tensor(Ax[:, o + 1:o + W - 1], h1[:, o:o + W - 2], lo[:, o + 2:o + W], AT.max)
        nc.vector.tensor_copy(Ax[:, o:o + 1], h1[:, o:o + 1])
        nc.vector.tensor_copy(Ax[:, o + W - 1:o + W], h1[:, o + W - 2:o + W - 1])
        nc.vector.tensor_tensor(h2[:, o:o + W - 1], hi[:, o:o + W - 1], hi[:, o + 1:o + W], AT.min)
        nc.vector.tensor_tensor(Cx[:, o + 1:o + W - 1], h2[:, o:o + W - 2], hi[:, o + 2:o + W], AT.min)
        nc.vector.tensor_copy(Cx[:, o:o + 1], h2[:, o:o + 1])
        nc.vector.tensor_copy(Cx[:, o + W - 1:o + W], h2[:, o + W - 2:o + W - 1])
        nc.vector.tensor_tensor(h1[:, o:o + W - 1], me[:, o:o + W - 1], me[:, o + 1:o + W], AT.min)
        nc.vector.tensor_tensor(h2[:, o:o + W - 1], me[:, o:o + W - 1], me[:, o + 1:o + W], AT.max)
        nc.vector.tensor_tensor(Bx[:, o + 1:o + W - 1], me[:, o:o + W - 2], h2[:, o + 1:o + W - 1], AT.min)
        nc.vector.tensor_tensor(Bx[:, o + 1:o + W - 1], Bx[:, o + 1:o + W - 1], h1[:, o + 1:o + W - 1], AT.max)
        nc.vector.tensor_copy(Bx[:, o:o + 1], me[:, o:o + 1])
        nc.vector.tensor_copy(Bx[:, o + W - 1:o + W], me[:, o + W - 1:o + W])
        nc.vector.tensor_tensor(h1[:, o:o + W], Ax[:, o:o + W], Bx[:, o:o + W], AT.min)
        nc.vector.tensor_tensor(h2[:, o:o + W], Ax[:, o:o + W], Bx[:, o:o + W], AT.max)
        nc.vector.tensor_tensor(h2[:, o:o + W], h2[:, o:o + W], Cx[:, o:o + W], AT.min)
        nc.vector.tensor_tensor(Mx[:, o:o + W], h2[:, o:o + W], h1[:, o:o + W], AT.max)

    # --- D stats ---
    Dc = mpool.tile([128, 2 * W], F16, tag="Dc")
    D2 = mpool.tile([128, 2 * W], F32, tag="D2")
    nc.vector.tensor_sub(Dc[:], Mx[:], cc[:])
    nc.vector.tensor_mul(D2[:], Dc[:], Dc[:])
    ST = spool.tile([128, 8], F32R, tag="ST")
    red = spool.tile([128, 8], F32, tag="red")
    nc.vector.reduce_sum(red[:, 0:2], Dc.rearrange("p (t w) -> p t w", t=2), axis=AX.X)
    nc.vector.reduce_sum(red[:, 2:4], D2.rearrange("p (t w) -> p t w", t=2), axis=AX.X)
    nc.vector.tensor_add(ST[:, 0:1], red[:, 0:1], red[:, 1:2])
    nc.vector.tensor_add(ST[:, 1:2], red[:, 2:3], red[:, 3:4])
    # left/right vertical-median strips (weight 19)
    sl_ = spool.tile([128, 12], F32, tag="strips")
    nc.vector.tensor_sub(sl_[:, 0:1], me[:, 0:1], cc[:, 0:1])
    nc.vector.tensor_sub(sl_[:, 1:2], me[:, W - 1:W], cc[:, W - 1:W])
    nc.vector.tensor_sub(sl_[:, 2:3], me[:, W:W + 1], cc[:, W:W + 1])
    nc.vector.tensor_sub(sl_[:, 3:4], me[:, 2 * W - 1:2 * W], cc[:, 2 * W - 1:2 * W])
    for q in range(4):
        nc.vector.tensor_mul(sl_[:, 4 + q:5 + q], sl_[:, q:q + 1], sl_[:, q:q + 1])
    nc.vector.tensor_add(sl_[:, 8:9], sl_[:, 0:1], sl_[:, 1:2])
    nc.vector.tensor_add(sl_[:, 9:10], sl_[:, 2:3], sl_[:, 3:4])
    nc.vector.tensor_add(sl_[:, 8:9], sl_[:, 8:9], sl_[:, 9:10])
    nc.vector.tensor_add(sl_[:, 10:11], sl_[:, 4:5], sl_[:, 5:6])
    nc.vector.tensor_add(sl_[:, 11:12], sl_[:, 6:7], sl_[:, 7:8])
    nc.vector.tensor_add(sl_[:, 10:11], sl_[:, 10:11], sl_[:, 11:12])
    nc.vector.tensor_scalar_mul(ST[:, 2:3], sl_[:, 8:9], 19.0)
    nc.vector.tensor_scalar_mul(ST[:, 3:4], sl_[:, 10:11], 19.0)
    # zero ST[0:1, 4:6] then accumulate the two 1-D strips (x19 inside)
    nc.vector.tensor_scalar_mul(ST[0:1, 4:6], CT["onesr"][0:1, 0:2], 0.0)
    _strip1d(nc, spool, ST, cc[0:1, 0:W])
    _strip1d(nc, spool, ST, r255[0:1, 0:W])

    nsr128 = _nsr(nc, spool, mpool, ppool, CT, ST, cc)

    # --- S1: Ut[c,k] = sum_r wf[r,c] At[r,k] ---
    ps_ut = ppool.tile([128, 2 * KH], F32, tag="ps_ut")
    ps_uti = ppool.tile([128, 2 * KH], F32, tag="ps_uti")
    for mt in range(2):
        for kt in range(2):
            lhs = wfr[:, kt * W + mt * 128: kt * W + mt * 128 + 128]
            nc.tensor.matmul(ps_ut[:, mt * KH:(mt + 1) * KH], lhs,
                             CT["AtR"][:, kt * N: kt * N + KH],
                             start=(kt == 0), stop=(kt == 1))
            nc.tensor.matmul(ps_uti[:, mt * KH:(mt + 1) * KH], lhs,
                             CT["AtI"][:, kt * N: kt * N + KH],
                             start=(kt == 0), stop=(kt == 1))
    UtR = spool.tile([128, 2 * KH], F32R, tag="UtR")
    UtI = spool.tile([128, 2 * KH], F32R, tag="UtI")
    nc.vector.tensor_copy(UtR[:], ps_ut[:])
    nc.scalar.copy(UtI[:], ps_uti[:])

    # --- S2: Gt[l,k] = sum_c At[c,l] Ut[c,k] (complex) ---
    ps_gr = ppool.tile([128, 3 * KH], F32, tag="ps_gr")
    ps_gi = ppool.tile([128, 3 * KH], F32, tag="ps_gi")
    for lt in range(3):
        rs = LT[lt]
        o = slice(lt * KH, lt * KH + KH)
        for kt in range(2):
            lA = slice(kt * N + lt * 128, kt * N + lt * 128 + rs)
            uR = UtR[:, kt * KH:(kt + 1) * KH]
            uI = UtI[:, kt * KH:(kt + 1) * KH]
            nc.tensor.matmul(ps_gr[0:rs, o], CT["AtR"][:, lA], uR, start=(kt == 0), stop=False)
            nc.tensor.matmul(ps_gi[0:rs, o], CT["AtR"][:, lA], uI, start=(kt == 0), stop=False)
        for kt in range(2):
            lA = slice(kt * N + lt * 128, kt * N + lt * 128 + rs)
            uR = UtR[:, kt * KH:(kt + 1) * KH]
            uI = UtI[:, kt * KH:(kt + 1) * KH]
            nc.tensor.matmul(ps_gr[0:rs, o], CT["AtIn"][:, lA], uI, start=False, stop=(kt == 1))
            nc.tensor.matmul(ps_gi[0:rs, o], CT["AtI"][:, lA], uR, start=False, stop=(kt == 1))

    # --- Wiener elementwise -> Zt (f32r) ---
    ZtR = spool.tile([128, 3 * KH], F32R, tag="ZtR")
    ZtI = spool.tile([128, 3 * KH], F32R, tag="ZtI")
    Rv = spool.tile([128, 3 * KH], F32, tag="Rv")
    t1 = spool.tile([128, KH], F32, tag="wt1")
    t2 = spool.tile([128, KH], F32, tag="wt2")
    for lt in range(3):
        rs = LT[lt]
        o = slice(lt * KH, lt * KH + KH)
        nc.vector.tensor_scalar_add(Rv[0:rs, o], CT["Dt"][0:rs, o], nsr128[0:rs, 0:1])
        nc.vector.reciprocal(Rv[0:rs, o], Rv[0:rs, o])
        nc.vector.tensor_mul(t1[0:rs, :], ps_gr[0:rs, o], CT["otfR"][0:rs, o])
        nc.vector.tensor_mul(t2[0:rs, :], ps_gi[0:rs, o], CT["otfI"][0:rs, o])
        nc.vector.tensor_add(t1[0:rs, :], t1[0:rs, :], t2[0:rs, :])
        nc.vector.tensor_mul(ZtR[0:rs, o], t1[0:rs, :], Rv[0:rs, o])
        nc.vector.tensor_mul(t1[0:rs, :], ps_gi[0:rs, o], CT["otfR"][0:rs, o])
        nc.vector.tensor_mul(t2[0:rs, :], ps_gr[0:rs, o], CT["otfI"][0:rs, o])
        nc.vector.tensor_sub(t1[0:rs, :], t1[0:rs, :], t2[0:rs, :])
        nc.vector.tensor_mul(ZtI[0:rs, o], t1[0:rs, :], Rv[0:rs, o])

    # --- S3: V[k,j] = sum_l Zt[l,k] B1t[l,j] (complex) ---
    ps_vr = ppool.tile([128, 2 * W], F32, tag="ps_vr")
    ps_vi = ppool.tile([128, 2 * W], F32, tag="ps_vi")
    for mt in range(2):
        ms = MT[mt]
        o = slice(mt * W, mt * W + W)
        for lt in range(3):
            rs = LT[lt]
            zR = ZtR[0:rs, lt * KH + mt * 128: lt * KH + mt * 128 + ms]
            zI = ZtI[0:rs, lt * KH + mt * 128: lt * KH + mt * 128 + ms]
            bR = CT["B1tR"][0:rs, lt * H:(lt + 1) * H]
            bI = CT["B1tI"][0:rs, lt * H:(lt + 1) * H]
            bIn = CT["B1tIn"][0:rs, lt * H:(lt + 1) * H]
            nc.tensor.matmul(ps_vr[0:ms, o], zR, bR, start=(lt == 0), stop=False)
            nc.tensor.matmul(ps_vi[0:ms, o], zR, bI, start=(lt == 0), stop=False)
            nc.tensor.matmul(ps_vr[0:ms, o], zI, bIn, start=False, stop=(lt == 2))
            nc.tensor.matmul(ps_vi[0:ms, o], zI, bR, start=False, stop=(lt == 2))
    VR = spool.tile([128, 2 * W], F32R, tag="VR")
    VI = spool.tile([128, 2 * W], F32R, tag="VI")
    nc.vector.tensor_copy(VR[:, 0:W], ps_vr[:, 0:W])
    nc.scalar.copy(VI[:, 0:W], ps_vi[:, 0:W])
    nc.vector.tensor_copy(VR[0:MT[1], W:2 * W], ps_vr[0:MT[1], W:2 * W])
    nc.scalar.copy(VI[0:MT[1], W:2 * W], ps_vi[0:MT[1], W:2 * W])

    # --- S4: img[i,j] = sum_k B4[i,k] VR[k,j] - B4i[i,k] VI[k,j] ---
    ps_o = ppool.tile([128, 2 * W], F32, tag="ps_img")
    ps_oa = ps_o[:, 0:W]
    ps_ob = ps_o[:, W:2 * W]
    for mt, ps in ((0, ps_oa), (1, ps_ob)):
        for kt in range(2):
            ks = MT[kt]
            b4r = CT["B4R"][0:ks, kt * H + mt * 128: kt * H + mt * 128 + 128]
            b4i = CT["B4In"][0:ks, kt * H + mt * 128: kt * H + mt * 128 + 128]
            nc.tensor.matmul(ps[:], b4r, VR[0:ks, kt * W:(kt + 1) * W],
                             start=(kt == 0), stop=False)
            nc.tensor.matmul(ps[:], b4i, VI[0:ks, kt * W:(kt + 1) * W],
                             start=False, stop=(kt == 1))
    img16 = spool.tile([128, 2 * W], F16, tag="img16")
    nc.vector.tensor_copy(img16[:, 0:W], ps_oa)
    nc.scalar.copy(img16[:, W:2 * W], ps_ob)
    for t in range(2):
        nc.sync.dma_start(
            clear16_d[ci, t * 128 * W:(t + 1) * 128 * W].rearrange("(p w) -> p w", w=W),
            img16[:, t * W:(t + 1) * W])


def _strip1d(nc, spool, ST, row):
    """Accumulate 19*(sum, sumsq) of (clamped-1D-med3(row) - row) into
    ST[0:1, 4:6]. row: [1, W] fp16."""
    h = spool.tile([1, 4 * W], F32, tag="strip1d")
    mnp = h[:, 0:W - 1]
    mxp = h[:, W:2 * W - 1]
    med = h[:, 2 * W:3 * W]
    nc.vector.tensor_tensor(mnp[:], row[:, 0:W - 1], row[:, 1:W], AT.min)
    nc.vector.tensor_tensor(mxp[:], row[:, 0:W - 1], row[:, 1:W], AT.max)
    nc.vector.tensor_tensor(h[:, 3 * W + 1:4 * W - 1], row[:, 0:W - 2],
                            mxp[:, 1:W - 1], AT.min)
    nc.vector.tensor_tensor(med[:, 1:W - 1], h[:, 3 * W + 1:4 * W - 1],
                            mnp[:, 1:W - 1], AT.max)
    nc.vector.tensor_copy(med[:, 0:1], row[:, 0:1])
    nc.vector.tensor_copy(med[:, W - 1:W], row[:, W - 1:W])
    d = h[:, 3 * W:4 * W]
    nc.vector.tensor_sub(d[:], med[:], row[:])
    s = spool.tile([1, 2], F32, tag="strip1d_s")
    nc.vector.reduce_sum(s[:, 0:1], d[:], axis=AX.X)
    d2 = h[:, 0:W]
    nc.vector.tensor_mul(d2[:], d[:], d[:])
    nc.vector.reduce_sum(s[:, 1:2], d2[:], axis=AX.X)
    nc.vector.tensor_scalar_mul(s[:], s[:], 19.0)
    nc.vector.tensor_add(ST[0:1, 4:6], ST[0:1, 4:6], s[:, 0:2])


def _nsr(nc, spool, mpool, ppool, CT, ST, cc):
    u16 = CT["u16"]
    wsq = mpool.tile([128, 2 * W], F16, tag="wsq")
    nc.vector.tensor_mul(wsq[:], cc[:], cc[:])
    # u-weighted rowsums: rowsum[c] for c = m*128+p; cols (2m, 2m+1) = (W, W2)
    ps_u = ppool.tile([128, 8], F32, tag="ps_small")
    for m in range(2):
        for t in range(2):
            nc.tensor.matmul(ps_u[:, 2 * m:2 * m + 1],
                             cc[:, t * W + m * 128: t * W + m * 128 + 128],
                             u16[:, t:t + 1], start=(t == 0), stop=(t == 1))
            nc.tensor.matmul(ps_u[:, 2 * m + 1:2 * m + 2],
                             wsq[:, t * W + m * 128: t * W + m * 128 + 128],
                             u16[:, t:t + 1], start=(t == 0), stop=(t == 1))
    rsum = spool.tile([128, 4], F16, tag="rsum16")
    nc.vector.tensor_copy(rsum[:], ps_u[:, 0:4])
    # column weights: m=0 cols weighted by u16[:,0] (20 at c=0), m=1 by u16[:,1]
    ps_t = ppool.tile([4, 8], F32, tag="ps_small")
    nc.tensor.matmul(ps_t[0:1, 0:2], u16[:, 0:1], rsum[:, 0:2], start=True, stop=True)
    nc.tensor.matmul(ps_t[0:1, 2:4], u16[:, 1:2], rsum[:, 2:4], start=True, stop=True)
    # D-stat cross-partition sums
    nc.tensor.matmul(ps_t[0:1, 4:8], CT["onesc"][:], ST[:, 0:4], start=True, stop=True)
    sW = spool.tile([1, 16], F32, tag="scal")
    nc.vector.tensor_copy(sW[:, 0:8], ps_t[0:1, 0:8])
    # sums: u-wf = c0+c2 ; u-wf2 = c1+c3 ; sD = c4+c6+ST[0,4] ; sD2 = c5+c7+ST[0,5]
    nc.vector.tensor_add(sW[:, 8:9], sW[:, 0:1], sW[:, 2:3])
    nc.vector.tensor_add(sW[:, 9:10], sW[:, 1:2], sW[:, 3:4])
    nc.vector.tensor_add(sW[:, 10:11], sW[:, 4:5], sW[:, 6:7])
    nc.vector.tensor_add(sW[:, 10:11], sW[:, 10:11], ST[0:1, 4:5])
    nc.vector.tensor_add(sW[:, 11:12], sW[:, 5:6], sW[:, 7:8])
    nc.vector.tensor_add(sW[:, 11:12], sW[:, 11:12], ST[0:1, 5:6])
    ninv, nm1inv = 1.0 / NTOT, 1.0 / (NTOT - 1)
    sc = spool.tile([1, 8], F32, tag="scal2")
    # var_n = (sD2 - sD^2/n)/(n-1)
    nc.vector.tensor_mul(sc[:, 0:1], sW[:, 10:11], sW[:, 10:11])
    nc.vector.tensor_scalar_mul(sc[:, 0:1], sc[:, 0:1], ninv)
    nc.vector.tensor_sub(sc[:, 0:1], sW[:, 11:12], sc[:, 0:1])
    nc.vector.tensor_scalar_mul(sc[:, 0:1], sc[:, 0:1], nm1inv)
    # var_s
    nc.vector.tensor_mul(sc[:, 1:2], sW[:, 8:9], sW[:, 8:9])
    nc.vector.tensor_scalar_mul(sc[:, 1:2], sc[:, 1:2], ninv)
    nc.vector.tensor_sub(sc[:, 1:2], sW[:, 9:10], sc[:, 1:2])
    nc.vector.tensor_scalar_mul(sc[:, 1:2], sc[:, 1:2], nm1inv)
    # rsqrt(var_s) via recip seed + NR
    nc.vector.reciprocal(sc[:, 2:3], sc[:, 1:2])
    for _ in range(4):
        nc.vector.tensor_mul(sc[:, 3:4], sc[:, 2:3], sc[:, 2:3])
        nc.vector.tensor_mul(sc[:, 3:4], sc[:, 3:4], sc[:, 1:2])
        nc.vector.tensor_scalar(sc[:, 3:4], sc[:, 3:4], -0.5, 1.5, AT.mult, AT.add)
        nc.vector.tensor_mul(sc[:, 2:3], sc[:, 2:3], sc[:, 3:4])
    nc.vector.tensor_mul(sc[:, 4:5], sc[:, 0:1], sc[:, 2:3])
    nc.vector.tensor_scalar_mul(sc[:, 4:5], sc[:, 4:5], 8.0 / 30.0)
    nsr_r = spool.tile([1, 1], F32, tag="nsr_r")
    nc.vector.tensor_copy(nsr_r[:], sc[:, 4:5])
    ps_b = ppool.tile([128, 8], F32, tag="ps_small")
    nc.tensor.matmul(ps_b[:, 0:1], CT["onesr"][:], nsr_r[:], start=True, stop=True)
    nsr128 = spool.tile([128, 1], F32, tag="nsr128")
    nc.vector.tensor_copy(nsr128[:], ps_b[:, 0:1])
    return nsr128


def _make_exec(nc, devs):
    """Build a sharded jit executable over the given device subset."""
    import jax
    from jax.sharding import Mesh, PartitionSpec, NamedSharding
    from jax.experimental.shard_map import shard_map
    from concourse import bass2jax

    bass2jax.install_neuronx_cc_hook()
    partition_name = nc.partition_id_tensor.name if nc.partition_id_tensor else None
    in_names, out_names, out_avals = [], [], []
    for alloc in nc.m.functions[0].allocations:
        if not isinstance(alloc, mybir.MemoryLocationSet):
            continue
        name = alloc.memorylocations[0].name
        if alloc.kind == "ExternalInput":
            if name != partition_name:
                in_names.append(name)
        elif alloc.kind == "ExternalOutput":
            out_names.append(name)
            out_avals.append(jax.core.ShapedArray(
                tuple(alloc.tensor_shape), mybir.dt.np(alloc.dtype)))
    all_in_names = in_names + out_names + ([partition_name] if partition_name else [])

    def _body(*args):
        ops = list(args)
        if partition_name is not None:
            ops.append(bass2jax.partition_id_tensor())
        outs = bass2jax._bass_exec_p.bind(
            *ops, out_avals=tuple(out_avals), in_names=tuple(all_in_names),
            out_names=tuple(out_names), lowering_input_output_aliases=(),
            sim_require_finite=True, sim_require_nnan=True, nc=nc)
        return tuple(outs)

    mesh = Mesh(np.asarray(devs), ("core",))
    n_args = len(in_names) + len(out_names)
    sharded = jax.jit(
        shard_map(_body, mesh=mesh,
                  in_specs=(PartitionSpec("core"),) * n_args,
                  out_specs=(PartitionSpec("core"),) * len(out_names),
                  check_rep=False),
        keep_unused=True)
    sh = NamedSharding(mesh, PartitionSpec("core"))
    return sharded, sh, in_names, out_names, out_avals


def _setup():
    """First-call setup: build program, four 2-core jits (batch b on
    cores 2b, 2b+1), push resident buffers."""
    import jax
    nc = _build()
    consts = _host_constants()
    devices = jax.devices()[:8]
    groups = []
    for g in range(4):
        dd = devices[2 * g:2 * g + 2]
        sharded, sh, in_names, out_names, out_avals = _make_exec(nc, dd)
        resident = {}
        for nm, arr in consts.items():
            resident[nm] = jax.device_put(
                np.concatenate([np.asarray(arr)] * 2, axis=0), sh)
        zeros = [jax.device_put(
            np.zeros((2 * a.shape[0], *a.shape[1:]), a.dtype), sh)
            for a in out_avals]
        groups.append(dict(sharded=sharded, sh=sh, resident=resident,
                           zeros=zeros, devs=list(dd)))
    _CACHE.update(nc=nc, groups=groups, in_names=in_names,
                  devices=list(devices))


def kernel(x, kernel, w_reduce, b_reduce, w_expand, b_expand):
    import jax
    from concurrent.futures import ThreadPoolExecutor
    if "groups" not in _CACHE:
        _setup()
    devices = _CACHE["devices"]
    in_names = _CACHE["in_names"]

    x = np.asarray(x, np.float32).reshape(B, NF, NPIX)
    kern = np.asarray(kernel, np.float32)
    w_reduce = np.asarray(w_reduce, np.float32)
    b_reduce = np.asarray(b_reduce, np.float32)
    w_expand = np.asarray(w_expand, np.float32)
    b_expand = np.asarray(b_expand, np.float32)

    pool = _CACHE.setdefault("pool", ThreadPoolExecutor(max_workers=8))
    if "wf16_buf" not in _CACHE:
        _CACHE["wf16_buf"] = np.empty((8, CH, HB, W), np.float16)
        _CACHE["y_buf"] = np.empty((C, NPIX), np.float32)
        claugs = np.empty((B, C + 1, NPIX), np.float32)
        claugs[:, C] = 1.0
        _CACHE["claug_bufs"] = claugs
    wf16 = _CACHE["wf16_buf"]
    ybuf = _CACHE["y_buf"]
    claugs = _CACHE["claug_bufs"]
    Waug = np.concatenate([w_expand, b_expand[:, None]], axis=1)  # [256, 65]
    out = np.empty((B, NF, NPIX), np.float32)
    locks = [threading.Lock() for _ in range(B)]
    pending = [2] * B
    done_futs = []

    # host reduce conv -> bordered fp16 wf per core, device_put from
    # worker threads as packed (upload of batch b overlaps sgemm b+1).
    # Four dispatches on disjoint 2-core meshes (batch b on cores 2b,
    # 2b+1): batch 0's download starts while batch 2's upload is still
    # streaming, so the serialized ~75 ms per-dispatch execs and the
    # expand sgemms all hide inside the wire windows.
    def _pack_put(core, rows, put_list):
        blk = wf16[core]
        blk[:, 1:H + 1, :] = rows.reshape(CH, H, W)
        blk[:, 0, :] = blk[:, 1, :]
        blk[:, H + 1, :] = blk[:, H, :]
        put_list.append(pool.submit(
            jax.device_put, blk.reshape(CH, HB * W), devices[core]))

    def _land(shard, b, h):
        # fp16 shard -> f32 claug slice; the worker that lands a batch's
        # SECOND shard runs that batch's expand sgemm (completion order).
        claugs[b, CH * h:CH * h + CH] = np.asarray(shard.data).reshape(CH, NPIX)
        with locks[b]:
            pending[b] -= 1
            ready = pending[b] == 0
        if ready:
            np.dot(Waug, claugs[b], out=out[b])

    # hoist the four tiny kern puts out of the streaming loop: they cost
    # main-thread RPC serialization but negligible wire, so issue at t=0
    d_kerns = []
    kern_r = np.empty((B, KS, KS), np.float32)
    for b in range(B):
        kern_r[b] = _round_f32r(kern[b, 0])
        d_kerns.append(jax.device_put(
            np.concatenate([kern_r[b]] * 2, axis=0), _CACHE["groups"][b]["sh"]))

    for b in range(B):
        G = _CACHE["groups"][b]
        put_list = []
        if b == 0:
            # split batch 0 per core so first bytes hit the wire sooner
            for h in range(2):
                yh = np.dot(w_reduce[CH * h:CH * h + CH], x[0],
                            out=ybuf[CH * h:CH * h + CH])
                yh += b_reduce[CH * h:CH * h + CH, None]
                _pack_put(h, yh, put_list)
        else:
            y = np.dot(w_reduce, x[b], out=ybuf)    # [64, NPIX]
            y += b_reduce[:, None]
            for h in range(2):
                _pack_put(2 * b + h, y[CH * h:CH * h + CH], put_list)
        d_kern = d_kerns[b]
        # dispatch immediately: the dispatch RPC is what triggers the
        # server-side exec, so deferring it (to overlap the put wait with
        # the next sgemm) measurably delays every download (A/B: +90 ms)
        d_wf = jax.make_array_from_single_device_arrays(
            (2 * CH, HB * W), G["sh"], [f.result() for f in put_list])
        per_call = {"wf16": d_wf, "kern": d_kern}
        args = [per_call[nm] if nm in per_call else G["resident"][nm]
                for nm in in_names]
        outs = G["sharded"](*args, *G["zeros"])
        osh = sorted(outs[0].addressable_shards, key=lambda s: s.index[0].start)
        for i, s in enumerate(osh):
            done_futs.append(pool.submit(_land, s, b, i))

    for f in done_futs:
        f.result()
    return out.reshape(B, NF, H, W)


if __name__ == "__main__":
    import reference
    inputs = reference.setup_inputs()
    out = kernel(**{k: np.asarray(v) for k, v in inputs.items()})
    print("out", out.shape, out.dtype)



# revision 25
# speedup vs baseline: 1.4128x; 1.4128x over previous
"""DWDN forward (Wiener deconvolution) Trainium2 Bass kernel.

Sharding: 8 cores = 4 batches x 2 channel-halves. Core k handles batch
b=k//2, reduced-channel half h=k%2 (32 of the 64 reduced channels).

The axon link to the cores moves ~70-110 MB/s half-duplex and every
program dispatch costs a fixed ~75 ms that serializes server-side (a
trivial program costs the same), so the design minimizes per-call
host<->device bytes (wf/clear are the rank-64 cut of the dataflow;
fp16 is the smallest dtype that holds the 2e-2 error gate) and uses
FOUR dispatches on disjoint 2-core meshes (batch b on cores 2b, 2b+1):
each batch's download starts while later batches' uploads still
stream, so the serialized fixed dispatch costs and the expand sgemms
hide inside the wire windows:
  host:   1x1 reduce conv (BLAS sgemm) -> wf [4,64,256,256], + bias,
          per-channel absmax int8 quantization (the Wiener filter is
          linear and both signal and quantization noise are white, so
          the relative error survives the deconvolution unchanged,
          ~1e-2 per int8 leg vs the 2e-2 gate) with replicated border
          rows -> 17 MB upload, streamed per-core from worker threads
          while later batches' sgemms run
  device: P0 OTF via small DFT matmuls (otf = E @ kern @ E^T), D=|otf|^2
          P2 per channel image: 3x3-median NSR stats (fp16 min/max
          network with border-band decomposition, nsr rescaled by the
          upload scale) + Wiener deconvolution as pad-folded DFT
          matmuls with Hermitian symmetry (freq rows k=0..147 of 294);
          emits clear as uint8 with per-row absmax scales (+128.5
          offset so the cast is exact under truncation or RNE)
          -> 17 MB download + tiny [CH,128] scale sidecar
  host:   per-row dequant, then 1x1 expand conv + bias (sgemm with
          ones-row augmentation), run per batch inside the fetch worker
          that lands the batch's second shard, so expands stream in
          completion order
The sharded jit executable, DFT constants, and zero output buffers stay
device-resident across calls; only wf8 + scales + kern move per call.

All matmuls in float32r (13-bit mantissa, full PE rate). No ACT LUT
activations and no custom-DVE ops (both crash on this deployment);
division via nc.vector.reciprocal only.
"""

import os
import sys
import threading
from contextlib import ExitStack

import numpy as np

for _p in ("/opt/trn_rl_repo", "/root/.axon_site/_ro/trn_rl_repo"):
    if os.path.isdir(_p) and _p not in sys.path:
        sys.path.insert(0, _p)

import concourse.tile as tile  # noqa: E402
from concourse import bacc, mybir  # noqa: E402

F32 = mybir.dt.float32
F32R = mybir.dt.float32r
F16 = mybir.dt.float16
I8 = mybir.dt.int8
U8 = mybir.dt.uint8
ACT = mybir.ActivationFunctionType
AT = mybir.AluOpType
AX = mybir.AxisListType

NF, KS, B, H, W = 256, 19, 4, 256, 256
N = H + 2 * KS          # 294
C = 64                  # reduced channels
CH = 32                 # channels per core
KH = N // 2 + 1         # 148 (Hermitian half)
NPIX = H * W
NTOT = N * N
HB = H + 2              # bordered image rows for the median path
LT = [128, 128, 38]     # partition-tile sizes for 294
MT = [128, 20]          # partition-tile sizes for 148


def _round_f32r(x):
    b = np.ascontiguousarray(x, dtype=np.float32).view(np.uint32).astype(np.uint64)
    b = (b + 0x200) & np.uint64(0xFFFFFC00)
    b = np.minimum(b, 0xFFFFFFFF).astype(np.uint32)
    return b.view(np.float32)


def _host_constants():
    k = np.arange(N)
    Fm = np.exp(-2j * np.pi * np.outer(k, k) / N)
    Sx = np.zeros((N, H))
    for i in range(N):
        Sx[i, min(max(i - KS, 0), H - 1)] = 1.0
    A = Fm @ Sx                                   # [294, 256]
    At = A.T                                      # [256, 294]
    B1 = np.conj(Fm)[KS:KS + H, :] / N            # [256, 294]
    wk = np.ones(KH)
    wk[1:KH - 1] = 2.0
    B4 = B1[:, :KH] * wk[None, :]                 # [256, 148]
    E = np.exp(-2j * np.pi * np.outer(k, (np.arange(KS) - KS // 2)) / N)  # [294,19]
    f = lambda v: _round_f32r(np.ascontiguousarray(v, np.float32))
    cst = {
        "AtR": f(At.real), "AtI": f(At.imag), "AtIn": f(-At.imag),
        "B1tR": f(B1.T.real), "B1tI": f(B1.T.imag), "B1tIn": f(-B1.T.imag),
        "B4R": f(B4.T.real), "B4In": f(-B4.T.imag),
        "ER": f(E.T.real), "EI": f(E.T.imag), "EIn": f(-E.T.imag),
        "onesr": f(np.ones((1, 128), np.float32)),
        "onesc": f(np.ones((128, 1), np.float32)),
    }
    u = np.ones((H, 1), np.float32)
    u[0, 0] = 20.0
    u[-1, 0] = 20.0
    cst["u16"] = u.astype(np.float16)
    cst["u32"] = u
    return cst


_CACHE = {}


def _build():
    nc = bacc.Bacc(None, target_bir_lowering=False)
    P = {}
    P["wf8"] = nc.declare_dram_parameter("wf8", [CH, HB * W], I8, isOutput=False)
    P["scl"] = nc.declare_dram_parameter("scl", [1, CH], F32, isOutput=False)
    P["kern"] = nc.declare_dram_parameter("kern", [KS, KS], F32R, isOutput=False)
    for nm, shp in [("AtR", [NF, N]), ("AtI", [NF, N]), ("AtIn", [NF, N]),
                    ("B1tR", [N, H]), ("B1tI", [N, H]), ("B1tIn", [N, H]),
                    ("B4R", [KH, H]), ("B4In", [KH, H]),
                    ("ER", [KS, N]), ("EI", [KS, N]), ("EIn", [KS, N]),
                    ("onesc", [128, 1])]:
        P[nm] = nc.declare_dram_parameter(nm, shp, F32R, isOutput=False)
    P["onesr"] = nc.declare_dram_parameter("onesr", [1, 128], F32, isOutput=False)
    P["u16"] = nc.declare_dram_parameter("u16", [H, 1], F16, isOutput=False)
    P["u32"] = nc.declare_dram_parameter("u32", [H, 1], F32, isOutput=False)
    clear8_d = nc.declare_dram_parameter("clear8", [CH, NPIX], U8, isOutput=True)
    sout_d = nc.declare_dram_parameter("sout", [CH, 128], F32, isOutput=True)

    with tile.TileContext(nc) as tc:
        _emit(nc, tc, P, clear8_d, sout_d)
    nc.finalize()
    return nc


def _load_consts(nc, tc, ctx, P):
    cpool = ctx.enter_context(tc.tile_pool(name="consts", bufs=1))
    CT = {}
    for nm in ("AtR", "AtI", "AtIn"):
        t = cpool.tile([128, 2 * N], F32R, tag=nm)
        for i in range(2):
            nc.sync.dma_start(t[:, i * N:(i + 1) * N], P[nm][i * 128:(i + 1) * 128, :])
        CT[nm] = t
    for nm in ("B1tR", "B1tI", "B1tIn"):
        t = cpool.tile([128, 3 * H], F32R, tag=nm)
        for i in range(3):
            rs = LT[i]
            nc.sync.dma_start(t[0:rs, i * H:i * H + H], P[nm][i * 128:i * 128 + rs, :])
        CT[nm] = t
    for nm in ("B4R", "B4In"):
        t = cpool.tile([128, 2 * H], F32R, tag=nm)
        for i in range(2):
            rs = MT[i]
            nc.sync.dma_start(t[0:rs, i * H:i * H + H], P[nm][i * 128:i * 128 + rs, :])
        CT[nm] = t
    for nm in ("ER", "EI", "EIn"):
        t = cpool.tile([KS, N], F32R, tag=nm)
        nc.sync.dma_start(t[:], P[nm][:, :])
        CT[nm] = t
    for nm, shp, dt in [("onesr", [1, 128], F32), ("onesc", [128, 1], F32R)]:
        t = cpool.tile(shp, dt, tag=nm)
        nc.sync.dma_start(t[:], P[nm][:, :])
        CT[nm] = t
    u16 = cpool.tile([128, 2], F16, tag="u16")
    for i in range(2):
        nc.sync.dma_start(u16[:, i:i + 1], P["u16"][i * 128:(i + 1) * 128, :])
    CT["u16"] = u16
    u32 = cpool.tile([128, 2], F32, tag="u32")
    for i in range(2):
        nc.sync.dma_start(u32[:, i:i + 1], P["u32"][i * 128:(i + 1) * 128, :])
    CT["u32"] = u32
    scl_t = cpool.tile([1, CH], F32, tag="scl")
    nc.sync.dma_start(scl_t[:], P["scl"][:, :])
    CT["scl"] = scl_t
    kern_sb = cpool.tile([KS, KS], F32R, tag="kern")
    nc.sync.dma_start(kern_sb[:], P["kern"][:, :])
    CT["kern"] = kern_sb
    # persistent P0 outputs
    CT["otfR"] = cpool.tile([128, 3 * KH], F32, name="otfR", tag="otfR")
    CT["otfI"] = cpool.tile([128, 3 * KH], F32, name="otfI", tag="otfI")
    CT["Dt"] = cpool.tile([128, 3 * KH], F32, name="Dtt", tag="Dtt")
    return CT


def _emit(nc, tc, P, clear8_d, sout_d):
    with ExitStack() as ctx:
        CT = _load_consts(nc, tc, ctx, P)

        # ---- P0 (scoped pools) ----
        with tc.tile_pool(name="p01", bufs=3) as wp, \
             tc.tile_pool(name="p01ps", bufs=1, space="PSUM") as pp:
            _p0_otf(nc, wp, pp, CT)

        tc.strict_bb_all_engine_barrier()

        # ---- P2 ----
        with tc.tile_pool(name="sw", bufs=2) as spool, \
             tc.tile_pool(name="med", bufs=2) as mpool, \
             tc.tile_pool(name="p2ps", bufs=1, space="PSUM") as ppool:
            for ci in range(CH):
                _image(nc, spool, mpool, ppool, CT, P["wf8"], clear8_d, sout_d, ci)


def _p0_otf(nc, wp, pp, CT):
    # Tt[j,k] = sum_i kern[i,j] E[k,i]: lhsT=kern [19,19], rhs=ER/EI [19,294]
    ps_a = pp.tile([KS, N], F32, tag="ps_otfa")
    ps_b = pp.tile([KS, N], F32, tag="ps_otfb")
    nc.tensor.matmul(ps_a[:], CT["kern"][:], CT["ER"][:], start=True, stop=True)
    nc.tensor.matmul(ps_b[:], CT["kern"][:], CT["EI"][:], start=True, stop=True)
    TtR = wp.tile([KS, N], F32R, tag="TtR")
    TtI = wp.tile([KS, N], F32R, tag="TtI")
    nc.vector.tensor_copy(TtR[:], ps_a[:])
    nc.vector.tensor_copy(TtI[:], ps_b[:])
    # otf_t[l,k] = sum_j E[l,j] Tt[j,k]: lhsT = E^T-as-[j,l] = ER/EI slices
    for lt in range(3):
        rs = LT[lt]
        lsl = slice(lt * 128, lt * 128 + rs)
        osl = slice(lt * KH, lt * KH + KH)
        ps_r = pp.tile([128, KH], F32, tag="ps_otfr")
        ps_i = pp.tile([128, KH], F32, tag="ps_otfi")
        nc.tensor.matmul(ps_r[0:rs, :], CT["ER"][:, lsl], TtR[:, 0:KH], start=True, stop=False)
        nc.tensor.matmul(ps_r[0:rs, :], CT["EIn"][:, lsl], TtI[:, 0:KH], start=False, stop=True)
        nc.tensor.matmul(ps_i[0:rs, :], CT["ER"][:, lsl], TtI[:, 0:KH], start=True, stop=False)
        nc.tensor.matmul(ps_i[0:rs, :], CT["EI"][:, lsl], TtR[:, 0:KH], start=False, stop=True)
        nc.vector.tensor_copy(CT["otfR"][0:rs, osl], ps_r[0:rs, :])
        nc.vector.tensor_copy(CT["otfI"][0:rs, osl], ps_i[0:rs, :])
        t2 = wp.tile([128, KH], F32, tag="d_tmp")
        nc.vector.tensor_mul(CT["Dt"][0:rs, osl], CT["otfR"][0:rs, osl], CT["otfR"][0:rs, osl])
        nc.vector.tensor_mul(t2[0:rs, :], CT["otfI"][0:rs, osl], CT["otfI"][0:rs, osl])
        nc.vector.tensor_add(CT["Dt"][0:rs, osl], CT["Dt"][0:rs, osl], t2[0:rs, :])


def _image(nc, spool, mpool, ppool, CT, wf8_p, clear8_d, sout_d, ci):
    # --- loads: int8 bordered rows; m1/cc/p1 are row-shifted views.
    # bordered row r: 0 = replicate of image row 0, r = image row r-1,
    # 257 = replicate of image row 255. int8 -> fp16 converts are exact.
    m8 = mpool.tile([128, 2 * W], I8, tag="m8")
    c8 = mpool.tile([128, 2 * W], I8, tag="c8")
    p8 = mpool.tile([128, 2 * W], I8, tag="p8")
    for t in range(2):
        base = t * 128
        for tl, off in ((m8, 0), (c8, 1), (p8, 2)):
            nc.sync.dma_start(
                tl[:, t * W:(t + 1) * W],
                wf8_p[ci, (base + off) * W:(base + off + 128) * W]
                .rearrange("(p w) -> p w", w=W))
    r8 = mpool.tile([1, W], I8, tag="r8")
    nc.sync.dma_start(r8[:], wf8_p[ci, 256 * W:257 * W].rearrange("(a w) -> a w", a=1))
    m1 = mpool.tile([128, 2 * W], F16, tag="m1")
    cc = mpool.tile([128, 2 * W], F16, tag="cc")
    p1 = mpool.tile([128, 2 * W], F16, tag="p1s")
    r255 = mpool.tile([1, W], F16, tag="r255")
    nc.scalar.copy(m1[:], m8[:])
    nc.scalar.copy(cc[:], c8[:])
    nc.scalar.copy(p1[:], p8[:])
    nc.scalar.copy(r255[:], r8[:])
    # f32r image rows for the DFT path (== cc, the center rows)
    wfr = spool.tile([128, 2 * W], F32R, tag="wfimg")
    nc.scalar.copy(wfr[:], cc[:])

    # --- vertical clamped sort3 -> lo/me/hi ---
    lo = mpool.tile([128, 2 * W], F16, tag="lo")
    me = mpool.tile([128, 2 * W], F16, tag="me")
    hi = mpool.tile([128, 2 * W], F16, tag="hi")
    tv = mpool.tile([128, 2 * W], F16, tag="tv")
    mn = mpool.tile([128, 2 * W], F16, tag="mnv")
    mx = mpool.tile([128, 2 * W], F16, tag="mxv")
    nc.vector.tensor_tensor(mn[:], cc[:], p1[:], AT.min)
    nc.vector.tensor_tensor(mx[:], cc[:], p1[:], AT.max)
    nc.vector.tensor_tensor(lo[:], m1[:], mn[:], AT.min)
    nc.vector.tensor_tensor(hi[:], m1[:], mx[:], AT.max)
    nc.vector.tensor_tensor(tv[:], m1[:], mx[:], AT.min)
    nc.vector.tensor_tensor(me[:], tv[:], mn[:], AT.max)

    # --- horizontal stage -> Mx ---
    Ax = mpool.tile([128, 2 * W], F16, tag="Ax")
    Cx = mpool.tile([128, 2 * W], F16, tag="Cx")
    Bx = mpool.tile([128, 2 * W], F16, tag="Bx")
    Mx = mpool.tile([128, 2 * W], F16, tag="Mx")
    h1 = mpool.tile([128, 2 * W], F16, tag="h1")
    h2 = mpool.tile([128, 2 * W], F16, tag="h2")
    for t in range(2):
        o = t * W
        nc.vector.tensor_tensor(h1[:, o:o + W - 1], lo[:, o:o + W - 1], lo[:, o + 1:o + W], AT.max)
        nc.vector.tensor_tensor(Ax[:, o + 1:o + W - 1], h1[:, o:o + W - 2], lo[:, o + 2:o + W], AT.max)
        nc.vector.tensor_copy(Ax[:, o:o + 1], h1[:, o:o + 1])
        nc.vector.tensor_copy(Ax[:, o + W - 1:o + W], h1[:, o + W - 2:o + W - 1])
        nc.vector.tensor_tensor(h2[:, o:o + W - 1], hi[:, o:o + W - 1], hi[:, o + 1:o + W], AT.min)
        nc.vector.tensor_tensor(Cx[:, o + 1:o + W - 1], h2[:, o:o + W - 2], hi[:, o + 2:o + W], AT.min)
        nc.vector.tensor_copy(Cx[:, o:o + 1], h2[:, o:o + 1])
        nc.vector.tensor_copy(Cx[:, o + W - 1:o + W], h2[:, o + W - 2:o + W - 1])
        nc.vector.tensor_tensor(h1[:, o:o + W - 1], me[:, o:o + W - 1], me[:, o + 1:o + W], AT.min)
        nc.vector.tensor_tensor(h2[:, o:o + W - 1], me[:, o:o + W - 1], me[:, o + 1:o + W], AT.max)
        nc.vector.tensor_tensor(Bx[:, o + 1:o + W - 1], me[:, o:o + W - 2], h2[:, o + 1:o + W - 1], AT.min)
        nc.vector.tensor_tensor(Bx[:, o + 1:o + W - 1], Bx[:, o + 1:o + W - 1], h1[:, o + 1:o + W - 1], AT.max)
        nc.vector.tensor_copy(Bx[:, o:o + 1], me[:, o:o + 1])
        nc.vector.tensor_copy(Bx[:, o + W - 1:o + W], me[:, o + W - 1:o + W])
        nc.vector.tensor_tensor(h1[:, o:o + W], Ax[:, o:o + W], Bx[:, o:o + W], AT.min)
        nc.vector.tensor_tensor(h2[:, o:o + W], Ax[:, o:o + W], Bx[:, o:o + W], AT.max)
        nc.vector.tensor_tensor(h2[:, o:o + W], h2[:, o:o + W], Cx[:, o:o + W], AT.min)
        nc.vector.tensor_tensor(Mx[:, o:o + W], h2[:, o:o + W], h1[:, o:o + W], AT.max)

    # --- D stats ---
    Dc = mpool.tile([128, 2 * W], F16, tag="Dc")
    D2 = mpool.tile([128, 2 * W], F32, tag="D2")
    nc.vector.tensor_sub(Dc[:], Mx[:], cc[:])
    nc.vector.tensor_mul(D2[:], Dc[:], Dc[:])
    ST = spool.tile([128, 8], F32R, tag="ST")
    red = spool.tile([128, 8], F32, tag="red")
    nc.vector.reduce_sum(red[:, 0:2], Dc.rearrange("p (t w) -> p t w", t=2), axis=AX.X)
    nc.vector.reduce_sum(red[:, 2:4], D2.rearrange("p (t w) -> p t w", t=2), axis=AX.X)
    nc.vector.tensor_add(ST[:, 0:1], red[:, 0:1], red[:, 1:2])
    nc.vector.tensor_add(ST[:, 1:2], red[:, 2:3], red[:, 3:4])
    # left/right vertical-median strips (weight 19)
    sl_ = spool.tile([128, 12], F32, tag="strips")
    nc.vector.tensor_sub(sl_[:, 0:1], me[:, 0:1], cc[:, 0:1])
    nc.vector.tensor_sub(sl_[:, 1:2], me[:, W - 1:W], cc[:, W - 1:W])
    nc.vector.tensor_sub(sl_[:, 2:3], me[:, W:W + 1], cc[:, W:W + 1])
    nc.vector.tensor_sub(sl_[:, 3:4], me[:, 2 * W - 1:2 * W], cc[:, 2 * W - 1:2 * W])
    for q in range(4):
        nc.vector.tensor_mul(sl_[:, 4 + q:5 + q], sl_[:, q:q + 1], sl_[:, q:q + 1])
    nc.vector.tensor_add(sl_[:, 8:9], sl_[:, 0:1], sl_[:, 1:2])
    nc.vector.tensor_add(sl_[:, 9:10], sl_[:, 2:3], sl_[:, 3:4])
    nc.vector.tensor_add(sl_[:, 8:9], sl_[:, 8:9], sl_[:, 9:10])
    nc.vector.tensor_add(sl_[:, 10:11], sl_[:, 4:5], sl_[:, 5:6])
    nc.vector.tensor_add(sl_[:, 11:12], sl_[:, 6:7], sl_[:, 7:8])
    nc.vector.tensor_add(sl_[:, 10:11], sl_[:, 10:11], sl_[:, 11:12])
    nc.vector.tensor_scalar_mul(ST[:, 2:3], sl_[:, 8:9], 19.0)
    nc.vector.tensor_scalar_mul(ST[:, 3:4], sl_[:, 10:11], 19.0)
    # zero ST[0:1, 4:6] then accumulate the two 1-D strips (x19 inside)
    nc.vector.tensor_scalar_mul(ST[0:1, 4:6], CT["onesr"][0:1, 0:2], 0.0)
    _strip1d(nc, spool, ST, cc[0:1, 0:W])
    _strip1d(nc, spool, ST, r255[0:1, 0:W])

    nsr128 = _nsr(nc, spool, mpool, ppool, CT, ST, cc, ci)

    # --- S1: Ut[c,k] = sum_r wf[r,c] At[r,k] ---
    ps_ut = ppool.tile([128, 2 * KH], F32, tag="ps_ut")
    ps_uti = ppool.tile([128, 2 * KH], F32, tag="ps_uti")
    for mt in range(2):
        for kt in range(2):
            lhs = wfr[:, kt * W + mt * 128: kt * W + mt * 128 + 128]
            nc.tensor.matmul(ps_ut[:, mt * KH:(mt + 1) * KH], lhs,
                             CT["AtR"][:, kt * N: kt * N + KH],
                             start=(kt == 0), stop=(kt == 1))
            nc.tensor.matmul(ps_uti[:, mt * KH:(mt + 1) * KH], lhs,
                             CT["AtI"][:, kt * N: kt * N + KH],
                             start=(kt == 0), stop=(kt == 1))
    UtR = spool.tile([128, 2 * KH], F32R, tag="UtR")
    UtI = spool.tile([128, 2 * KH], F32R, tag="UtI")
    nc.vector.tensor_copy(UtR[:], ps_ut[:])
    nc.scalar.copy(UtI[:], ps_uti[:])

    # --- S2: Gt[l,k] = sum_c At[c,l] Ut[c,k] (complex) ---
    ps_gr = ppool.tile([128, 3 * KH], F32, tag="ps_gr")
    ps_gi = ppool.tile([128, 3 * KH], F32, tag="ps_gi")
    for lt in range(3):
        rs = LT[lt]
        o = slice(lt * KH, lt * KH + KH)
        for kt in range(2):
            lA = slice(kt * N + lt * 128, kt * N + lt * 128 + rs)
            uR = UtR[:, kt * KH:(kt + 1) * KH]
            uI = UtI[:, kt * KH:(kt + 1) * KH]
            nc.tensor.matmul(ps_gr[0:rs, o], CT["AtR"][:, lA], uR, start=(kt == 0), stop=False)
            nc.tensor.matmul(ps_gi[0:rs, o], CT["AtR"][:, lA], uI, start=(kt == 0), stop=False)
        for kt in range(2):
            lA = slice(kt * N + lt * 128, kt * N + lt * 128 + rs)
            uR = UtR[:, kt * KH:(kt + 1) * KH]
            uI = UtI[:, kt * KH:(kt + 1) * KH]
            nc.tensor.matmul(ps_gr[0:rs, o], CT["AtIn"][:, lA], uI, start=False, stop=(kt == 1))
            nc.tensor.matmul(ps_gi[0:rs, o], CT["AtI"][:, lA], uR, start=False, stop=(kt == 1))

    # --- Wiener elementwise -> Zt (f32r) ---
    ZtR = spool.tile([128, 3 * KH], F32R, tag="ZtR")
    ZtI = spool.tile([128, 3 * KH], F32R, tag="ZtI")
    Rv = spool.tile([128, 3 * KH], F32, tag="Rv")
    t1 = spool.tile([128, KH], F32, tag="wt1")
    t2 = spool.tile([128, KH], F32, tag="wt2")
    for lt in range(3):
        rs = LT[lt]
        o = slice(lt * KH, lt * KH + KH)
        nc.vector.tensor_scalar_add(Rv[0:rs, o], CT["Dt"][0:rs, o], nsr128[0:rs, 0:1])
        nc.vector.reciprocal(Rv[0:rs, o], Rv[0:rs, o])
        nc.vector.tensor_mul(t1[0:rs, :], ps_gr[0:rs, o], CT["otfR"][0:rs, o])
        nc.vector.tensor_mul(t2[0:rs, :], ps_gi[0:rs, o], CT["otfI"][0:rs, o])
        nc.vector.tensor_add(t1[0:rs, :], t1[0:rs, :], t2[0:rs, :])
        nc.vector.tensor_mul(ZtR[0:rs, o], t1[0:rs, :], Rv[0:rs, o])
        nc.vector.tensor_mul(t1[0:rs, :], ps_gi[0:rs, o], CT["otfR"][0:rs, o])
        nc.vector.tensor_mul(t2[0:rs, :], ps_gr[0:rs, o], CT["otfI"][0:rs, o])
        nc.vector.tensor_sub(t1[0:rs, :], t1[0:rs, :], t2[0:rs, :])
        nc.vector.tensor_mul(ZtI[0:rs, o], t1[0:rs, :], Rv[0:rs, o])

    # --- S3: V[k,j] = sum_l Zt[l,k] B1t[l,j] (complex) ---
    ps_vr = ppool.tile([128, 2 * W], F32, tag="ps_vr")
    ps_vi = ppool.tile([128, 2 * W], F32, tag="ps_vi")
    for mt in range(2):
        ms = MT[mt]
        o = slice(mt * W, mt * W + W)
        for lt in range(3):
            rs = LT[lt]
            zR = ZtR[0:rs, lt * KH + mt * 128: lt * KH + mt * 128 + ms]
            zI = ZtI[0:rs, lt * KH + mt * 128: lt * KH + mt * 128 + ms]
            bR = CT["B1tR"][0:rs, lt * H:(lt + 1) * H]
            bI = CT["B1tI"][0:rs, lt * H:(lt + 1) * H]
            bIn = CT["B1tIn"][0:rs, lt * H:(lt + 1) * H]
            nc.tensor.matmul(ps_vr[0:ms, o], zR, bR, start=(lt == 0), stop=False)
            nc.tensor.matmul(ps_vi[0:ms, o], zR, bI, start=(lt == 0), stop=False)
            nc.tensor.matmul(ps_vr[0:ms, o], zI, bIn, start=False, stop=(lt == 2))
            nc.tensor.matmul(ps_vi[0:ms, o], zI, bR, start=False, stop=(lt == 2))
    VR = spool.tile([128, 2 * W], F32R, tag="VR")
    VI = spool.tile([128, 2 * W], F32R, tag="VI")
    nc.vector.tensor_copy(VR[:, 0:W], ps_vr[:, 0:W])
    nc.scalar.copy(VI[:, 0:W], ps_vi[:, 0:W])
    nc.vector.tensor_copy(VR[0:MT[1], W:2 * W], ps_vr[0:MT[1], W:2 * W])
    nc.scalar.copy(VI[0:MT[1], W:2 * W], ps_vi[0:MT[1], W:2 * W])

    # --- S4: img[i,j] = sum_k B4[i,k] VR[k,j] - B4i[i,k] VI[k,j] ---
    ps_o = ppool.tile([128, 2 * W], F32, tag="ps_img")
    ps_oa = ps_o[:, 0:W]
    ps_ob = ps_o[:, W:2 * W]
    for mt, ps in ((0, ps_oa), (1, ps_ob)):
        for kt in range(2):
            ks = MT[kt]
            b4r = CT["B4R"][0:ks, kt * H + mt * 128: kt * H + mt * 128 + 128]
            b4i = CT["B4In"][0:ks, kt * H + mt * 128: kt * H + mt * 128 + 128]
            nc.tensor.matmul(ps[:], b4r, VR[0:ks, kt * W:(kt + 1) * W],
                             start=(kt == 0), stop=False)
            nc.tensor.matmul(ps[:], b4i, VI[0:ks, kt * W:(kt + 1) * W],
                             start=False, stop=(kt == 1))
    # --- uint8 encode with per-partition (= per image-row-pair) scales:
    # q = rne(clear_q * 127/rowmax + 128); the f32->uint8 cast is RNE
    # and saturating (verified on HW), so the +128 offset is exact.
    pm2 = spool.tile([128, 2], F32, tag="pm2")
    nc.vector.reduce_max(pm2[:], ps_o.rearrange("p (t w) -> p t w", t=2),
                         axis=AX.X, apply_absolute_value=True)
    pm = spool.tile([128, 1], F32, tag="pm")
    nc.vector.tensor_tensor(pm[:], pm2[:, 0:1], pm2[:, 1:2], AT.max)
    e128 = spool.tile([128, 1], F32, tag="e128")
    nc.vector.reciprocal(e128[:], pm[:])
    nc.vector.tensor_scalar_mul(e128[:], e128[:], 127.0)
    img8 = spool.tile([128, 2 * W], U8, tag="img8")
    nc.scalar.activation(img8[:, 0:W], ps_oa, ACT.Copy,
                         bias=128.0, scale=e128[:, 0:1])
    nc.scalar.activation(img8[:, W:2 * W], ps_ob, ACT.Copy,
                         bias=128.0, scale=e128[:, 0:1])
    nc.sync.dma_start(sout_d[ci, 0:128].rearrange("(p w) -> p w", w=1), pm[:])
    for t in range(2):
        nc.sync.dma_start(
            clear8_d[ci, t * 128 * W:(t + 1) * 128 * W].rearrange("(p w) -> p w", w=W),
            img8[:, t * W:(t + 1) * W])


def _strip1d(nc, spool, ST, row):
    """Accumulate 19*(sum, sumsq) of (clamped-1D-med3(row) - row) into
    ST[0:1, 4:6]. row: [1, W] fp16."""
    h = spool.tile([1, 4 * W], F32, tag="strip1d")
    mnp = h[:, 0:W - 1]
    mxp = h[:, W:2 * W - 1]
    med = h[:, 2 * W:3 * W]
    nc.vector.tensor_tensor(mnp[:], row[:, 0:W - 1], row[:, 1:W], AT.min)
    nc.vector.tensor_tensor(mxp[:], row[:, 0:W - 1], row[:, 1:W], AT.max)
    nc.vector.tensor_tensor(h[:, 3 * W + 1:4 * W - 1], row[:, 0:W - 2],
                            mxp[:, 1:W - 1], AT.min)
    nc.vector.tensor_tensor(med[:, 1:W - 1], h[:, 3 * W + 1:4 * W - 1],
                            mnp[:, 1:W - 1], AT.max)
    nc.vector.tensor_copy(med[:, 0:1], row[:, 0:1])
    nc.vector.tensor_copy(med[:, W - 1:W], row[:, W - 1:W])
    d = h[:, 3 * W:4 * W]
    nc.vector.tensor_sub(d[:], med[:], row[:])
    s = spool.tile([1, 2], F32, tag="strip1d_s")
    nc.vector.reduce_sum(s[:, 0:1], d[:], axis=AX.X)
    d2 = h[:, 0:W]
    nc.vector.tensor_mul(d2[:], d[:], d[:])
    nc.vector.reduce_sum(s[:, 1:2], d2[:], axis=AX.X)
    nc.vector.tensor_scalar_mul(s[:], s[:], 19.0)
    nc.vector.tensor_add(ST[0:1, 4:6], ST[0:1, 4:6], s[:, 0:2])


def _nsr(nc, spool, mpool, ppool, CT, ST, cc, ci):
    u16 = CT["u16"]
    wsq = mpool.tile([128, 2 * W], F16, tag="wsq")
    nc.vector.tensor_mul(wsq[:], cc[:], cc[:])
    # u-weighted rowsums: rowsum[c] for c = m*128+p; cols (2m, 2m+1) = (W, W2)
    ps_u = ppool.tile([128, 8], F32, tag="ps_small")
    for m in range(2):
        for t in range(2):
            nc.tensor.matmul(ps_u[:, 2 * m:2 * m + 1],
                             cc[:, t * W + m * 128: t * W + m * 128 + 128],
                             u16[:, t:t + 1], start=(t == 0), stop=(t == 1))
            nc.tensor.matmul(ps_u[:, 2 * m + 1:2 * m + 2],
                             wsq[:, t * W + m * 128: t * W + m * 128 + 128],
                             u16[:, t:t + 1], start=(t == 0), stop=(t == 1))
    # rsum in f32: with int8-scaled inputs the wsq rowsums reach ~5e6,
    # which overflows fp16.
    rsum = spool.tile([128, 4], F32, tag="rsum32")
    nc.vector.tensor_copy(rsum[:], ps_u[:, 0:4])
    # column weights: m=0 cols weighted by u[:,0] (20 at c=0), m=1 by u[:,1]
    u32 = CT["u32"]
    ps_t = ppool.tile([4, 8], F32, tag="ps_small")
    nc.tensor.matmul(ps_t[0:1, 0:2], u32[:, 0:1], rsum[:, 0:2], start=True, stop=True)
    nc.tensor.matmul(ps_t[0:1, 2:4], u32[:, 1:2], rsum[:, 2:4], start=True, stop=True)
    # D-stat cross-partition sums
    nc.tensor.matmul(ps_t[0:1, 4:8], CT["onesc"][:], ST[:, 0:4], start=True, stop=True)
    sW = spool.tile([1, 16], F32, tag="scal")
    nc.vector.tensor_copy(sW[:, 0:8], ps_t[0:1, 0:8])
    # sums: u-wf = c0+c2 ; u-wf2 = c1+c3 ; sD = c4+c6+ST[0,4] ; sD2 = c5+c7+ST[0,5]
    nc.vector.tensor_add(sW[:, 8:9], sW[:, 0:1], sW[:, 2:3])
    nc.vector.tensor_add(sW[:, 9:10], sW[:, 1:2], sW[:, 3:4])
    nc.vector.tensor_add(sW[:, 10:11], sW[:, 4:5], sW[:, 6:7])
    nc.vector.tensor_add(sW[:, 10:11], sW[:, 10:11], ST[0:1, 4:5])
    nc.vector.tensor_add(sW[:, 11:12], sW[:, 5:6], sW[:, 7:8])
    nc.vector.tensor_add(sW[:, 11:12], sW[:, 11:12], ST[0:1, 5:6])
    ninv, nm1inv = 1.0 / NTOT, 1.0 / (NTOT - 1)
    sc = spool.tile([1, 8], F32, tag="scal2")
    # var_n = (sD2 - sD^2/n)/(n-1)
    nc.vector.tensor_mul(sc[:, 0:1], sW[:, 10:11], sW[:, 10:11])
    nc.vector.tensor_scalar_mul(sc[:, 0:1], sc[:, 0:1], ninv)
    nc.vector.tensor_sub(sc[:, 0:1], sW[:, 11:12], sc[:, 0:1])
    nc.vector.tensor_scalar_mul(sc[:, 0:1], sc[:, 0:1], nm1inv)
    # var_s
    nc.vector.tensor_mul(sc[:, 1:2], sW[:, 8:9], sW[:, 8:9])
    nc.vector.tensor_scalar_mul(sc[:, 1:2], sc[:, 1:2], ninv)
    nc.vector.tensor_sub(sc[:, 1:2], sW[:, 9:10], sc[:, 1:2])
    nc.vector.tensor_scalar_mul(sc[:, 1:2], sc[:, 1:2], nm1inv)
    # rsqrt(var_s) via recip seed + NR. var_s of the int8-scaled image is
    # ~(127/4.7)^2 ~ 730, so seed 27/v ~ 1/sqrt(v) within +-7% (a bare
    # 1/v seed is 27x off and 4 NR steps cannot recover it).
    nc.vector.reciprocal(sc[:, 2:3], sc[:, 1:2])
    nc.vector.tensor_scalar_mul(sc[:, 2:3], sc[:, 2:3], 27.0)
    for _ in range(4):
        nc.vector.tensor_mul(sc[:, 3:4], sc[:, 2:3], sc[:, 2:3])
        nc.vector.tensor_mul(sc[:, 3:4], sc[:, 3:4], sc[:, 1:2])
        nc.vector.tensor_scalar(sc[:, 3:4], sc[:, 3:4], -0.5, 1.5, AT.mult, AT.add)
        nc.vector.tensor_mul(sc[:, 2:3], sc[:, 2:3], sc[:, 3:4])
    nc.vector.tensor_mul(sc[:, 4:5], sc[:, 0:1], sc[:, 2:3])
    nc.vector.tensor_scalar_mul(sc[:, 4:5], sc[:, 4:5], 8.0 / 30.0)
    # nsr was computed from wf_q = wf/s_c; true nsr scales linearly in
    # amplitude, so multiply by the upload scale s_c.
    nsr_r = spool.tile([1, 1], F32, tag="nsr_r")
    nc.vector.tensor_mul(nsr_r[:], sc[:, 4:5], CT["scl"][0:1, ci:ci + 1])
    ps_b = ppool.tile([128, 8], F32, tag="ps_small")
    nc.tensor.matmul(ps_b[:, 0:1], CT["onesr"][:], nsr_r[:], start=True, stop=True)
    nsr128 = spool.tile([128, 1], F32, tag="nsr128")
    nc.vector.tensor_copy(nsr128[:], ps_b[:, 0:1])
    return nsr128


def _make_exec(nc, devs):
    """Build a sharded jit executable over the given device subset."""
    import jax
    from jax.sharding import Mesh, PartitionSpec, NamedSharding
    from jax.experimental.shard_map import shard_map
    from concourse import bass2jax

    bass2jax.install_neuronx_cc_hook()
    partition_name = nc.partition_id_tensor.name if nc.partition_id_tensor else None
    in_names, out_names, out_avals = [], [], []
    for alloc in nc.m.functions[0].allocations:
        if not isinstance(alloc, mybir.MemoryLocationSet):
            continue
        name = alloc.memorylocations[0].name
        if alloc.kind == "ExternalInput":
            if name != partition_name:
                in_names.append(name)
        elif alloc.kind == "ExternalOutput":
            out_names.append(name)
            out_avals.append(jax.core.ShapedArray(
                tuple(alloc.tensor_shape), mybir.dt.np(alloc.dtype)))
    all_in_names = in_names + out_names + ([partition_name] if partition_name else [])

    def _body(*args):
        ops = list(args)
        if partition_name is not None:
            ops.append(bass2jax.partition_id_tensor())
        outs = bass2jax._bass_exec_p.bind(
            *ops, out_avals=tuple(out_avals), in_names=tuple(all_in_names),
            out_names=tuple(out_names), lowering_input_output_aliases=(),
            sim_require_finite=True, sim_require_nnan=True, nc=nc)
        return tuple(outs)

    mesh = Mesh(np.asarray(devs), ("core",))
    n_args = len(in_names) + len(out_names)
    sharded = jax.jit(
        shard_map(_body, mesh=mesh,
                  in_specs=(PartitionSpec("core"),) * n_args,
                  out_specs=(PartitionSpec("core"),) * len(out_names),
                  check_rep=False),
        keep_unused=True)
    sh = NamedSharding(mesh, PartitionSpec("core"))
    return sharded, sh, in_names, out_names, out_avals


def _setup():
    """First-call setup: build program, four 2-core jits (batch b on
    cores 2b, 2b+1), push resident buffers."""
    import jax
    nc = _build()
    consts = _host_constants()
    devices = jax.devices()[:8]
    groups = []
    for g in range(4):
        dd = devices[2 * g:2 * g + 2]
        sharded, sh, in_names, out_names, out_avals = _make_exec(nc, dd)
        resident = {}
        for nm, arr in consts.items():
            resident[nm] = jax.device_put(
                np.concatenate([np.asarray(arr)] * 2, axis=0), sh)
        zeros = [jax.device_put(
            np.zeros((2 * a.shape[0], *a.shape[1:]), a.dtype), sh)
            for a in out_avals]
        groups.append(dict(sharded=sharded, sh=sh, resident=resident,
                           zeros=zeros, devs=list(dd)))
    _CACHE.update(nc=nc, groups=groups, in_names=in_names,
                  devices=list(devices))


def kernel(x, kernel, w_reduce, b_reduce, w_expand, b_expand):
    import jax
    import time
    from concurrent.futures import ThreadPoolExecutor
    if "groups" not in _CACHE:
        _setup()
    dbg = os.environ.get("DWDN_T")
    t00 = time.monotonic()

    def _tr(msg):
        if dbg:
            print(f"[{time.monotonic() - t00:7.3f}] {msg}", flush=True)
    devices = _CACHE["devices"]
    in_names = _CACHE["in_names"]

    x = np.asarray(x, np.float32).reshape(B, NF, NPIX)
    kern = np.asarray(kernel, np.float32)
    w_reduce = np.asarray(w_reduce, np.float32)
    b_reduce = np.asarray(b_reduce, np.float32)
    w_expand = np.asarray(w_expand, np.float32)
    b_expand = np.asarray(b_expand, np.float32)

    pool = _CACHE.setdefault("pool", ThreadPoolExecutor(max_workers=8))
    if "wf8_buf" not in _CACHE:
        _CACHE["wf8_buf"] = np.empty((8, CH, HB, W), np.int8)
        _CACHE["y_buf"] = np.empty((C, NPIX), np.float32)
        claugs = np.empty((B, C + 1, NPIX), np.float32)
        claugs[:, C] = 1.0
        _CACHE["claug_bufs"] = claugs
    wf8 = _CACHE["wf8_buf"]
    ybuf = _CACHE["y_buf"]
    claugs = _CACHE["claug_bufs"]
    Waug = np.concatenate([w_expand, b_expand[:, None]], axis=1)  # [256, 65]
    out = np.empty((B, NF, NPIX), np.float32)
    locks = [threading.Lock() for _ in range(B)]
    pending = [2] * B
    s_up = np.empty((B, 2, CH), np.float32)
    done_futs = []

    # host reduce conv -> per-channel int8 quantized bordered wf per
    # core (exact per-channel absmax scale, so no clipping is needed),
    # device_put from worker threads (upload of batch b overlaps sgemm
    # b+1). Four dispatches on disjoint 2-core meshes (batch b on cores
    # 2b, 2b+1): batch 0's download starts while batch 2's upload is
    # still streaming, so the serialized ~75 ms per-dispatch execs and
    # the expand sgemms all hide inside the wire windows.
    def _pack_put(core, rows, put_list, scl_list, b, h):
        amax = np.abs(rows).max(axis=1)
        np.maximum(amax, 1e-20, out=amax)
        s_up[b, h] = amax / 127.0
        q = np.rint(rows * (127.0 / amax)[:, None])
        blk = wf8[core]
        blk[:, 1:H + 1, :] = q.reshape(CH, H, W)
        blk[:, 0, :] = blk[:, 1, :]
        blk[:, H + 1, :] = blk[:, H, :]
        _tr(f"pack done core {core}")

        def _put(a, d, c=core):
            r = jax.device_put(a, d)
            jax.block_until_ready(r)
            _tr(f"put done core {c}")
            return r
        put_list.append(pool.submit(
            _put, blk.reshape(CH, HB * W), devices[core]))
        scl_list.append(pool.submit(
            jax.device_put, s_up[b, h].reshape(1, CH), devices[core]))

    def _land(shard, sshard, b, h):
        # uint8 shard -> f32 claug slice, undoing the per-row download
        # scales and the per-channel upload scales; the worker that
        # lands a batch's SECOND shard runs that batch's expand sgemm.
        pm = np.asarray(sshard.data)                  # [CH, 128] row maxes
        srow = np.concatenate([pm, pm], axis=1)       # row r and r+128
        srow *= (s_up[b, h] / 127.0)[:, None]
        qa = claugs[b, CH * h:CH * h + CH].reshape(CH, H, W)
        qa[:] = np.asarray(shard.data).reshape(CH, H, W)
        _tr(f"fetch done b {b} h {h}")
        qa -= 128.0
        qa *= srow[:, :, None]
        with locks[b]:
            pending[b] -= 1
            ready = pending[b] == 0
        if ready:
            np.dot(Waug, claugs[b], out=out[b])
            _tr(f"expand done b {b}")

    # hoist the four tiny kern puts out of the streaming loop: they cost
    # main-thread RPC serialization but negligible wire, so issue at t=0
    d_kerns = []
    kern_r = np.empty((B, KS, KS), np.float32)
    for b in range(B):
        kern_r[b] = _round_f32r(kern[b, 0])
        d_kerns.append(jax.device_put(
            np.concatenate([kern_r[b]] * 2, axis=0), _CACHE["groups"][b]["sh"]))

    for b in range(B):
        G = _CACHE["groups"][b]
        put_list, scl_list = [], []
        if b == 0:
            # split batch 0 per core so first bytes hit the wire sooner
            for h in range(2):
                yh = np.dot(w_reduce[CH * h:CH * h + CH], x[0],
                            out=ybuf[CH * h:CH * h + CH])
                yh += b_reduce[CH * h:CH * h + CH, None]
                _pack_put(h, yh, put_list, scl_list, 0, h)
        else:
            y = np.dot(w_reduce, x[b], out=ybuf)    # [64, NPIX]
            y += b_reduce[:, None]
            for h in range(2):
                _pack_put(2 * b + h, y[CH * h:CH * h + CH], put_list,
                          scl_list, b, h)
        d_kern = d_kerns[b]
        # dispatch immediately: the dispatch RPC is what triggers the
        # server-side exec, so deferring it (to overlap the put wait with
        # the next sgemm) measurably delays every download (A/B: +90 ms)
        d_wf = jax.make_array_from_single_device_arrays(
            (2 * CH, HB * W), G["sh"], [f.result() for f in put_list])
        d_scl = jax.make_array_from_single_device_arrays(
            (2, CH), G["sh"], [f.result() for f in scl_list])
        per_call = {"wf8": d_wf, "kern": d_kern, "scl": d_scl}
        args = [per_call[nm] if nm in per_call else G["resident"][nm]
                for nm in in_names]
        outs = G["sharded"](*args, *G["zeros"])
        _tr(f"dispatched b {b}")
        osh = sorted(outs[0].addressable_shards, key=lambda s: s.index[0].start)
        ssh = sorted(outs[1].addressable_shards, key=lambda s: s.index[0].start)
        for i, (s, ss) in enumerate(zip(osh, ssh)):
            done_futs.append(pool.submit(_land, s, ss, b, i))

    for f in done_futs:
        f.result()
    return out.reshape(B, NF, H, W)


if __name__ == "__main__":
    import reference
    inputs = reference.setup_inputs()
    out = kernel(**{k: np.asarray(v) for k, v in inputs.items()})
    print("out", out.shape, out.dtype)



# revision 28
# speedup vs baseline: 1.4865x; 1.0521x over previous
"""DWDN forward (Wiener deconvolution) Trainium2 Bass kernel.

Sharding: 8 cores = 4 batches x 2 channel-halves. Core k handles batch
b=k//2, reduced-channel half h=k%2 (32 of the 64 reduced channels).

The axon link to the cores has a ~140-150 ms fixed cost PER RPC
(device_put / fetch / dispatch, any size) plus ~40-60 MB/s of
bandwidth, and the host has ONE cpu. The design therefore (a) moves
the minimal rank-64 cut of the dataflow (wf/clear), (b) quantizes both
directions to 8 bits -- the Wiener filter is linear and both signal
and quantization noise are white, so the relative error survives the
deconvolution unchanged, ~1e-2 per leg against the 2e-2 gate --, and
(c) packs each direction into ONE rpc per core:
  host:   1x1 reduce conv (BLAS sgemm) -> wf [64,256,256] per batch,
          per-channel absmax uint8 quantization (+128 offset; the cast
          is a fused saturating numpy add) packed WITH the f32 scale
          sideband and the f32r-rounded 19x19 PSF into a single flat
          int8 buffer per core, device_put from worker threads while
          later batches' sgemms run on the main thread
  device: P0 OTF via small DFT matmuls (otf = E @ kern @ E^T), D=|otf|^2
          P2 per channel image: 3x3-median NSR stats (fp16 min/max
          network with border-band decomposition; the int8 decode
          applies bias -128 inside the convert activations; nsr is
          rescaled by the uploaded per-channel scale) + Wiener
          deconvolution as pad-folded DFT matmuls with Hermitian
          symmetry (freq rows k=0..147 of 294); emits clear as uint8
          with one global absmax scale per channel (cross-partition
          max via PE-transpose against an identity), the f32 maxes
          bitcast into the tail of the same uint8 output buffer
  host:   per-channel dequant FOLDED into the expand weights (host
          decode is a single uint8->f32 cast), then 1x1 expand conv +
          bias as sgemm with ones-row augmentation, run inside the
          fetch worker that lands the batch's second shard
Four dispatches on disjoint 2-core meshes (batch b on cores 2b, 2b+1)
so each batch's download starts while later batches' uploads stream.
The sharded jit executables, DFT constants, and zero output buffers
stay device-resident across calls; only wf8 moves per call.

All matmuls in float32r (13-bit mantissa, full PE rate). No ACT LUT
activations and no custom-DVE ops (both crash on this deployment);
division via nc.vector.reciprocal only (seeded x27 for the int8 value
range, since NR from a bare reciprocal seed cannot recover a 27x-off
rsqrt seed in 4 steps).
"""

import os
import sys
import threading
from contextlib import ExitStack

import numpy as np

for _p in ("/opt/trn_rl_repo", "/root/.axon_site/_ro/trn_rl_repo"):
    if os.path.isdir(_p) and _p not in sys.path:
        sys.path.insert(0, _p)

import concourse.tile as tile  # noqa: E402
from concourse import bacc, mybir  # noqa: E402

F32 = mybir.dt.float32
F32R = mybir.dt.float32r
F16 = mybir.dt.float16
I8 = mybir.dt.int8
U8 = mybir.dt.uint8
ACT = mybir.ActivationFunctionType
AT = mybir.AluOpType
AX = mybir.AxisListType

NF, KS, B, H, W = 256, 19, 4, 256, 256
N = H + 2 * KS          # 294
C = 64                  # reduced channels
CH = 32                 # channels per core
KH = N // 2 + 1         # 148 (Hermitian half)
NPIX = H * W
NTOT = N * N
HB = H + 2              # bordered image rows for the median path
LT = [128, 128, 38]     # partition-tile sizes for 294
MT = [128, 20]          # partition-tile sizes for 148
SB = CH * HB * W        # sideband offset in the flat upload buffer
UPN = SB + 2048         # flat upload bytes per core
DWN = CH * NPIX + 128   # flat download bytes per core (128 = CH f32 maxes)


def _round_f32r(x):
    b = np.ascontiguousarray(x, dtype=np.float32).view(np.uint32).astype(np.uint64)
    b = (b + 0x200) & np.uint64(0xFFFFFC00)
    b = np.minimum(b, 0xFFFFFFFF).astype(np.uint32)
    return b.view(np.float32)


def _host_constants():
    k = np.arange(N)
    Fm = np.exp(-2j * np.pi * np.outer(k, k) / N)
    Sx = np.zeros((N, H))
    for i in range(N):
        Sx[i, min(max(i - KS, 0), H - 1)] = 1.0
    A = Fm @ Sx                                   # [294, 256]
    At = A.T                                      # [256, 294]
    B1 = np.conj(Fm)[KS:KS + H, :] / N            # [256, 294]
    wk = np.ones(KH)
    wk[1:KH - 1] = 2.0
    B4 = B1[:, :KH] * wk[None, :]                 # [256, 148]
    E = np.exp(-2j * np.pi * np.outer(k, (np.arange(KS) - KS // 2)) / N)  # [294,19]
    f = lambda v: _round_f32r(np.ascontiguousarray(v, np.float32))
    cst = {
        "AtR": f(At.real), "AtI": f(At.imag), "AtIn": f(-At.imag),
        "B1tR": f(B1.T.real), "B1tI": f(B1.T.imag), "B1tIn": f(-B1.T.imag),
        "B4R": f(B4.T.real), "B4In": f(-B4.T.imag),
        "ER": f(E.T.real), "EI": f(E.T.imag), "EIn": f(-E.T.imag),
        "onesr": np.ones((1, 128), np.float32),
        "onesc": f(np.ones((128, 1), np.float32)),
        "ident": np.eye(128, dtype=np.float32),
    }
    u = np.ones((H, 1), np.float32)
    u[0, 0] = 20.0
    u[-1, 0] = 20.0
    cst["u16"] = u.astype(np.float16)
    cst["u32"] = u
    return cst


_CACHE = {}


def _build():
    nc = bacc.Bacc(None, target_bir_lowering=False)
    P = {}
    P["wf8"] = nc.declare_dram_parameter("wf8", [1, UPN], I8, isOutput=False)
    for nm, shp in [("AtR", [NF, N]), ("AtI", [NF, N]), ("AtIn", [NF, N]),
                    ("B1tR", [N, H]), ("B1tI", [N, H]), ("B1tIn", [N, H]),
                    ("B4R", [KH, H]), ("B4In", [KH, H]),
                    ("ER", [KS, N]), ("EI", [KS, N]), ("EIn", [KS, N]),
                    ("onesc", [128, 1])]:
        P[nm] = nc.declare_dram_parameter(nm, shp, F32R, isOutput=False)
    P["onesr"] = nc.declare_dram_parameter("onesr", [1, 128], F32, isOutput=False)
    P["ident"] = nc.declare_dram_parameter("ident", [128, 128], F32, isOutput=False)
    P["u16"] = nc.declare_dram_parameter("u16", [H, 1], F16, isOutput=False)
    P["u32"] = nc.declare_dram_parameter("u32", [H, 1], F32, isOutput=False)
    clear8_d = nc.declare_dram_parameter("clear8", [1, DWN], U8, isOutput=True)

    with tile.TileContext(nc) as tc:
        _emit(nc, tc, P, clear8_d)
    nc.finalize()
    return nc


def _load_consts(nc, tc, ctx, P):
    cpool = ctx.enter_context(tc.tile_pool(name="consts", bufs=1))
    CT = {}
    for nm in ("AtR", "AtI", "AtIn"):
        t = cpool.tile([128, 2 * N], F32R, tag=nm)
        for i in range(2):
            nc.sync.dma_start(t[:, i * N:(i + 1) * N], P[nm][i * 128:(i + 1) * 128, :])
        CT[nm] = t
    for nm in ("B1tR", "B1tI", "B1tIn"):
        t = cpool.tile([128, 3 * H], F32R, tag=nm)
        for i in range(3):
            rs = LT[i]
            nc.sync.dma_start(t[0:rs, i * H:i * H + H], P[nm][i * 128:i * 128 + rs, :])
        CT[nm] = t
    for nm in ("B4R", "B4In"):
        t = cpool.tile([128, 2 * H], F32R, tag=nm)
        for i in range(2):
            rs = MT[i]
            nc.sync.dma_start(t[0:rs, i * H:i * H + H], P[nm][i * 128:i * 128 + rs, :])
        CT[nm] = t
    for nm in ("ER", "EI", "EIn"):
        t = cpool.tile([KS, N], F32R, tag=nm)
        nc.sync.dma_start(t[:], P[nm][:, :])
        CT[nm] = t
    for nm, shp, dt in [("onesr", [1, 128], F32), ("onesc", [128, 1], F32R),
                        ("ident", [128, 128], F32)]:
        t = cpool.tile(shp, dt, tag=nm)
        nc.sync.dma_start(t[:], P[nm][:, :])
        CT[nm] = t
    u16 = cpool.tile([128, 2], F16, tag="u16")
    for i in range(2):
        nc.sync.dma_start(u16[:, i:i + 1], P["u16"][i * 128:(i + 1) * 128, :])
    CT["u16"] = u16
    u32 = cpool.tile([128, 2], F32, tag="u32")
    for i in range(2):
        nc.sync.dma_start(u32[:, i:i + 1], P["u32"][i * 128:(i + 1) * 128, :])
    CT["u32"] = u32
    # sideband: per-channel upload scales + f32r PSF, bitcast out of the
    # flat int8 upload buffer
    scl_t = cpool.tile([1, CH], F32, tag="scl")
    nc.sync.dma_start(
        scl_t[:], P["wf8"][0, SB:SB + 4 * CH].bitcast(F32)
        .rearrange("(a w) -> a w", a=1))
    CT["scl"] = scl_t
    kern_sb = cpool.tile([KS, KS], F32R, tag="kern")
    nc.sync.dma_start(
        kern_sb[:], P["wf8"][0, SB + 4 * CH:SB + 4 * CH + 4 * KS * KS]
        .bitcast(F32R).rearrange("(p w) -> p w", w=KS))
    CT["kern"] = kern_sb
    # persistent P0 outputs
    CT["otfR"] = cpool.tile([128, 3 * KH], F32, name="otfR", tag="otfR")
    CT["otfI"] = cpool.tile([128, 3 * KH], F32, name="otfI", tag="otfI")
    CT["Dt"] = cpool.tile([128, 3 * KH], F32, name="Dtt", tag="Dtt")
    return CT


def _emit(nc, tc, P, clear8_d):
    with ExitStack() as ctx:
        CT = _load_consts(nc, tc, ctx, P)

        # ---- P0 (scoped pools) ----
        with tc.tile_pool(name="p01", bufs=3) as wp, \
             tc.tile_pool(name="p01ps", bufs=1, space="PSUM") as pp:
            _p0_otf(nc, wp, pp, CT)

        tc.strict_bb_all_engine_barrier()

        # ---- P2 ----
        with tc.tile_pool(name="sw", bufs=2) as spool, \
             tc.tile_pool(name="med", bufs=2) as mpool, \
             tc.tile_pool(name="p2ps", bufs=1, space="PSUM") as ppool:
            for ci in range(CH):
                _image(nc, spool, mpool, ppool, CT, P["wf8"], clear8_d, ci)


def _p0_otf(nc, wp, pp, CT):
    # Tt[j,k] = sum_i kern[i,j] E[k,i]: lhsT=kern [19,19], rhs=ER/EI [19,294]
    ps_a = pp.tile([KS, N], F32, tag="ps_otfa")
    ps_b = pp.tile([KS, N], F32, tag="ps_otfb")
    nc.tensor.matmul(ps_a[:], CT["kern"][:], CT["ER"][:], start=True, stop=True)
    nc.tensor.matmul(ps_b[:], CT["kern"][:], CT["EI"][:], start=True, stop=True)
    TtR = wp.tile([KS, N], F32R, tag="TtR")
    TtI = wp.tile([KS, N], F32R, tag="TtI")
    nc.vector.tensor_copy(TtR[:], ps_a[:])
    nc.vector.tensor_copy(TtI[:], ps_b[:])
    # otf_t[l,k] = sum_j E[l,j] Tt[j,k]: lhsT = E^T-as-[j,l] = ER/EI slices
    for lt in range(3):
        rs = LT[lt]
        lsl = slice(lt * 128, lt * 128 + rs)
        osl = slice(lt * KH, lt * KH + KH)
        ps_r = pp.tile([128, KH], F32, tag="ps_otfr")
        ps_i = pp.tile([128, KH], F32, tag="ps_otfi")
        nc.tensor.matmul(ps_r[0:rs, :], CT["ER"][:, lsl], TtR[:, 0:KH], start=True, stop=False)
        nc.tensor.matmul(ps_r[0:rs, :], CT["EIn"][:, lsl], TtI[:, 0:KH], start=False, stop=True)
        nc.tensor.matmul(ps_i[0:rs, :], CT["ER"][:, lsl], TtI[:, 0:KH], start=True, stop=False)
        nc.tensor.matmul(ps_i[0:rs, :], CT["EI"][:, lsl], TtR[:, 0:KH], start=False, stop=True)
        nc.vector.tensor_copy(CT["otfR"][0:rs, osl], ps_r[0:rs, :])
        nc.vector.tensor_copy(CT["otfI"][0:rs, osl], ps_i[0:rs, :])
        t2 = wp.tile([128, KH], F32, tag="d_tmp")
        nc.vector.tensor_mul(CT["Dt"][0:rs, osl], CT["otfR"][0:rs, osl], CT["otfR"][0:rs, osl])
        nc.vector.tensor_mul(t2[0:rs, :], CT["otfI"][0:rs, osl], CT["otfI"][0:rs, osl])
        nc.vector.tensor_add(CT["Dt"][0:rs, osl], CT["Dt"][0:rs, osl], t2[0:rs, :])


def _image(nc, spool, mpool, ppool, CT, wf8_p, clear8_d, ci):
    # --- loads: uint8(+128) bordered rows; m1/cc/p1 are row-shifted
    # views. bordered row r: 0 = replicate of image row 0, r = image row
    # r-1, 257 = replicate of image row 255. The convert activations
    # apply bias -128, so cc/m1/p1 hold the centered int values exactly.
    cb = ci * HB * W
    m8 = mpool.tile([128, 2 * W], U8, tag="m8")
    c8 = mpool.tile([128, 2 * W], U8, tag="c8")
    p8 = mpool.tile([128, 2 * W], U8, tag="p8")
    for t in range(2):
        base = t * 128
        for tl, off in ((m8, 0), (c8, 1), (p8, 2)):
            nc.sync.dma_start(
                tl[:, t * W:(t + 1) * W],
                wf8_p[0, cb + (base + off) * W:cb + (base + off + 128) * W]
                .bitcast(U8).rearrange("(p w) -> p w", w=W))
    r8 = mpool.tile([1, W], U8, tag="r8")
    nc.sync.dma_start(r8[:], wf8_p[0, cb + 256 * W:cb + 257 * W]
                      .bitcast(U8).rearrange("(a w) -> a w", a=1))
    m1 = mpool.tile([128, 2 * W], F16, tag="m1")
    cc = mpool.tile([128, 2 * W], F16, tag="cc")
    p1 = mpool.tile([128, 2 * W], F16, tag="p1s")
    r255 = mpool.tile([1, W], F16, tag="r255")
    nc.scalar.activation(m1[:], m8[:], ACT.Copy, bias=-128.0)
    nc.scalar.activation(cc[:], c8[:], ACT.Copy, bias=-128.0)
    nc.scalar.activation(p1[:], p8[:], ACT.Copy, bias=-128.0)
    nc.scalar.activation(r255[:], r8[:], ACT.Copy, bias=-128.0)
    # f32r image rows for the DFT path (== cc, the center rows)
    wfr = spool.tile([128, 2 * W], F32R, tag="wfimg")
    nc.scalar.copy(wfr[:], cc[:])

    # --- vertical clamped sort3 -> lo/me/hi ---
    lo = mpool.tile([128, 2 * W], F16, tag="lo")
    me = mpool.tile([128, 2 * W], F16, tag="me")
    hi = mpool.tile([128, 2 * W], F16, tag="hi")
    tv = mpool.tile([128, 2 * W], F16, tag="tv")
    mn = mpool.tile([128, 2 * W], F16, tag="mnv")
    mx = mpool.tile([128, 2 * W], F16, tag="mxv")
    nc.vector.tensor_tensor(mn[:], cc[:], p1[:], AT.min)
    nc.vector.tensor_tensor(mx[:], cc[:], p1[:], AT.max)
    nc.vector.tensor_tensor(lo[:], m1[:], mn[:], AT.min)
    nc.vector.tensor_tensor(hi[:], m1[:], mx[:], AT.max)
    nc.vector.tensor_tensor(tv[:], m1[:], mx[:], AT.min)
    nc.vector.tensor_tensor(me[:], tv[:], mn[:], AT.max)

    # --- horizontal stage -> Mx ---
    Ax = mpool.tile([128, 2 * W], F16, tag="Ax")
    Cx = mpool.tile([128, 2 * W], F16, tag="Cx")
    Bx = mpool.tile([128, 2 * W], F16, tag="Bx")
    Mx = mpool.tile([128, 2 * W], F16, tag="Mx")
    h1 = mpool.tile([128, 2 * W], F16, tag="h1")
    h2 = mpool.tile([128, 2 * W], F16, tag="h2")
    for t in range(2):
        o = t * W
        nc.vector.tensor_tensor(h1[:, o:o + W - 1], lo[:, o:o + W - 1], lo[:, o + 1:o + W], AT.max)
        nc.vector.tensor_tensor(Ax[:, o + 1:o + W - 1], h1[:, o:o + W - 2], lo[:, o + 2:o + W], AT.max)
        nc.vector.tensor_copy(Ax[:, o:o + 1], h1[:, o:o + 1])
        nc.vector.tensor_copy(Ax[:, o + W - 1:o + W], h1[:, o + W - 2:o + W - 1])
        nc.vector.tensor_tensor(h2[:, o:o + W - 1], hi[:, o:o + W - 1], hi[:, o + 1:o + W], AT.min)
        nc.vector.tensor_tensor(Cx[:, o + 1:o + W - 1], h2[:, o:o + W - 2], hi[:, o + 2:o + W], AT.min)
        nc.vector.tensor_copy(Cx[:, o:o + 1], h2[:, o:o + 1])
        nc.vector.tensor_copy(Cx[:, o + W - 1:o + W], h2[:, o + W - 2:o + W - 1])
        nc.vector.tensor_tensor(h1[:, o:o + W - 1], me[:, o:o + W - 1], me[:, o + 1:o + W], AT.min)
        nc.vector.tensor_tensor(h2[:, o:o + W - 1], me[:, o:o + W - 1], me[:, o + 1:o + W], AT.max)
        nc.vector.tensor_tensor(Bx[:, o + 1:o + W - 1], me[:, o:o + W - 2], h2[:, o + 1:o + W - 1], AT.min)
        nc.vector.tensor_tensor(Bx[:, o + 1:o + W - 1], Bx[:, o + 1:o + W - 1], h1[:, o + 1:o + W - 1], AT.max)
        nc.vector.tensor_copy(Bx[:, o:o + 1], me[:, o:o + 1])
        nc.vector.tensor_copy(Bx[:, o + W - 1:o + W], me[:, o + W - 1:o + W])
        nc.vector.tensor_tensor(h1[:, o:o + W], Ax[:, o:o + W], Bx[:, o:o + W], AT.min)
        nc.vector.tensor_tensor(h2[:, o:o + W], Ax[:, o:o + W], Bx[:, o:o + W], AT.max)
        nc.vector.tensor_tensor(h2[:, o:o + W], h2[:, o:o + W], Cx[:, o:o + W], AT.min)
        nc.vector.tensor_tensor(Mx[:, o:o + W], h2[:, o:o + W], h1[:, o:o + W], AT.max)

    # --- D stats ---
    Dc = mpool.tile([128, 2 * W], F16, tag="Dc")
    D2 = mpool.tile([128, 2 * W], F32, tag="D2")
    nc.vector.tensor_sub(Dc[:], Mx[:], cc[:])
    nc.vector.tensor_mul(D2[:], Dc[:], Dc[:])
    ST = spool.tile([128, 8], F32R, tag="ST")
    red = spool.tile([128, 8], F32, tag="red")
    nc.vector.reduce_sum(red[:, 0:2], Dc.rearrange("p (t w) -> p t w", t=2), axis=AX.X)
    nc.vector.reduce_sum(red[:, 2:4], D2.rearrange("p (t w) -> p t w", t=2), axis=AX.X)
    nc.vector.tensor_add(ST[:, 0:1], red[:, 0:1], red[:, 1:2])
    nc.vector.tensor_add(ST[:, 1:2], red[:, 2:3], red[:, 3:4])
    # left/right vertical-median strips (weight 19)
    sl_ = spool.tile([128, 12], F32, tag="strips")
    nc.vector.tensor_sub(sl_[:, 0:1], me[:, 0:1], cc[:, 0:1])
    nc.vector.tensor_sub(sl_[:, 1:2], me[:, W - 1:W], cc[:, W - 1:W])
    nc.vector.tensor_sub(sl_[:, 2:3], me[:, W:W + 1], cc[:, W:W + 1])
    nc.vector.tensor_sub(sl_[:, 3:4], me[:, 2 * W - 1:2 * W], cc[:, 2 * W - 1:2 * W])
    for q in range(4):
        nc.vector.tensor_mul(sl_[:, 4 + q:5 + q], sl_[:, q:q + 1], sl_[:, q:q + 1])
    nc.vector.tensor_add(sl_[:, 8:9], sl_[:, 0:1], sl_[:, 1:2])
    nc.vector.tensor_add(sl_[:, 9:10], sl_[:, 2:3], sl_[:, 3:4])
    nc.vector.tensor_add(sl_[:, 8:9], sl_[:, 8:9], sl_[:, 9:10])
    nc.vector.tensor_add(sl_[:, 10:11], sl_[:, 4:5], sl_[:, 5:6])
    nc.vector.tensor_add(sl_[:, 11:12], sl_[:, 6:7], sl_[:, 7:8])
    nc.vector.tensor_add(sl_[:, 10:11], sl_[:, 10:11], sl_[:, 11:12])
    nc.vector.tensor_scalar_mul(ST[:, 2:3], sl_[:, 8:9], 19.0)
    nc.vector.tensor_scalar_mul(ST[:, 3:4], sl_[:, 10:11], 19.0)
    # zero ST[0:1, 4:6] then accumulate the two 1-D strips (x19 inside)
    nc.vector.tensor_scalar_mul(ST[0:1, 4:6], CT["onesr"][0:1, 0:2], 0.0)
    _strip1d(nc, spool, ST, cc[0:1, 0:W])
    _strip1d(nc, spool, ST, r255[0:1, 0:W])

    nsr128 = _nsr(nc, spool, mpool, ppool, CT, ST, cc, ci)

    # --- S1: Ut[c,k] = sum_r wf[r,c] At[r,k] ---
    ps_ut = ppool.tile([128, 2 * KH], F32, tag="ps_ut")
    ps_uti = ppool.tile([128, 2 * KH], F32, tag="ps_uti")
    for mt in range(2):
        for kt in range(2):
            lhs = wfr[:, kt * W + mt * 128: kt * W + mt * 128 + 128]
            nc.tensor.matmul(ps_ut[:, mt * KH:(mt + 1) * KH], lhs,
                             CT["AtR"][:, kt * N: kt * N + KH],
                             start=(kt == 0), stop=(kt == 1))
            nc.tensor.matmul(ps_uti[:, mt * KH:(mt + 1) * KH], lhs,
                             CT["AtI"][:, kt * N: kt * N + KH],
                             start=(kt == 0), stop=(kt == 1))
    UtR = spool.tile([128, 2 * KH], F32R, tag="UtR")
    UtI = spool.tile([128, 2 * KH], F32R, tag="UtI")
    nc.vector.tensor_copy(UtR[:], ps_ut[:])
    nc.scalar.copy(UtI[:], ps_uti[:])

    # --- S2: Gt[l,k] = sum_c At[c,l] Ut[c,k] (complex) ---
    ps_gr = ppool.tile([128, 3 * KH], F32, tag="ps_gr")
    ps_gi = ppool.tile([128, 3 * KH], F32, tag="ps_gi")
    for lt in range(3):
        rs = LT[lt]
        o = slice(lt * KH, lt * KH + KH)
        for kt in range(2):
            lA = slice(kt * N + lt * 128, kt * N + lt * 128 + rs)
            uR = UtR[:, kt * KH:(kt + 1) * KH]
            uI = UtI[:, kt * KH:(kt + 1) * KH]
            nc.tensor.matmul(ps_gr[0:rs, o], CT["AtR"][:, lA], uR, start=(kt == 0), stop=False)
            nc.tensor.matmul(ps_gi[0:rs, o], CT["AtR"][:, lA], uI, start=(kt == 0), stop=False)
        for kt in range(2):
            lA = slice(kt * N + lt * 128, kt * N + lt * 128 + rs)
            uR = UtR[:, kt * KH:(kt + 1) * KH]
            uI = UtI[:, kt * KH:(kt + 1) * KH]
            nc.tensor.matmul(ps_gr[0:rs, o], CT["AtIn"][:, lA], uI, start=False, stop=(kt == 1))
            nc.tensor.matmul(ps_gi[0:rs, o], CT["AtI"][:, lA], uR, start=False, stop=(kt == 1))

    # --- Wiener elementwise -> Zt (f32r) ---
    ZtR = spool.tile([128, 3 * KH], F32R, tag="ZtR")
    ZtI = spool.tile([128, 3 * KH], F32R, tag="ZtI")
    Rv = spool.tile([128, 3 * KH], F32, tag="Rv")
    t1 = spool.tile([128, KH], F32, tag="wt1")
    t2 = spool.tile([128, KH], F32, tag="wt2")
    for lt in range(3):
        rs = LT[lt]
        o = slice(lt * KH, lt * KH + KH)
        nc.vector.tensor_scalar_add(Rv[0:rs, o], CT["Dt"][0:rs, o], nsr128[0:rs, 0:1])
        nc.vector.reciprocal(Rv[0:rs, o], Rv[0:rs, o])
        nc.vector.tensor_mul(t1[0:rs, :], ps_gr[0:rs, o], CT["otfR"][0:rs, o])
        nc.vector.tensor_mul(t2[0:rs, :], ps_gi[0:rs, o], CT["otfI"][0:rs, o])
        nc.vector.tensor_add(t1[0:rs, :], t1[0:rs, :], t2[0:rs, :])
        nc.vector.tensor_mul(ZtR[0:rs, o], t1[0:rs, :], Rv[0:rs, o])
        nc.vector.tensor_mul(t1[0:rs, :], ps_gi[0:rs, o], CT["otfR"][0:rs, o])
        nc.vector.tensor_mul(t2[0:rs, :], ps_gr[0:rs, o], CT["otfI"][0:rs, o])
        nc.vector.tensor_sub(t1[0:rs, :], t1[0:rs, :], t2[0:rs, :])
        nc.vector.tensor_mul(ZtI[0:rs, o], t1[0:rs, :], Rv[0:rs, o])

    # --- S3: V[k,j] = sum_l Zt[l,k] B1t[l,j] (complex) ---
    ps_vr = ppool.tile([128, 2 * W], F32, tag="ps_vr")
    ps_vi = ppool.tile([128, 2 * W], F32, tag="ps_vi")
    for mt in range(2):
        ms = MT[mt]
        o = slice(mt * W, mt * W + W)
        for lt in range(3):
            rs = LT[lt]
            zR = ZtR[0:rs, lt * KH + mt * 128: lt * KH + mt * 128 + ms]
            zI = ZtI[0:rs, lt * KH + mt * 128: lt * KH + mt * 128 + ms]
            bR = CT["B1tR"][0:rs, lt * H:(lt + 1) * H]
            bI = CT["B1tI"][0:rs, lt * H:(lt + 1) * H]
            bIn = CT["B1tIn"][0:rs, lt * H:(lt + 1) * H]
            nc.tensor.matmul(ps_vr[0:ms, o], zR, bR, start=(lt == 0), stop=False)
            nc.tensor.matmul(ps_vi[0:ms, o], zR, bI, start=(lt == 0), stop=False)
            nc.tensor.matmul(ps_vr[0:ms, o], zI, bIn, start=False, stop=(lt == 2))
            nc.tensor.matmul(ps_vi[0:ms, o], zI, bR, start=False, stop=(lt == 2))
    VR = spool.tile([128, 2 * W], F32R, tag="VR")
    VI = spool.tile([128, 2 * W], F32R, tag="VI")
    nc.vector.tensor_copy(VR[:, 0:W], ps_vr[:, 0:W])
    nc.scalar.copy(VI[:, 0:W], ps_vi[:, 0:W])
    nc.vector.tensor_copy(VR[0:MT[1], W:2 * W], ps_vr[0:MT[1], W:2 * W])
    nc.scalar.copy(VI[0:MT[1], W:2 * W], ps_vi[0:MT[1], W:2 * W])

    # --- S4: img[i,j] = sum_k B4[i,k] VR[k,j] - B4i[i,k] VI[k,j] ---
    ps_o = ppool.tile([128, 2 * W], F32, tag="ps_img")
    ps_oa = ps_o[:, 0:W]
    ps_ob = ps_o[:, W:2 * W]
    for mt, ps in ((0, ps_oa), (1, ps_ob)):
        for kt in range(2):
            ks = MT[kt]
            b4r = CT["B4R"][0:ks, kt * H + mt * 128: kt * H + mt * 128 + 128]
            b4i = CT["B4In"][0:ks, kt * H + mt * 128: kt * H + mt * 128 + 128]
            nc.tensor.matmul(ps[:], b4r, VR[0:ks, kt * W:(kt + 1) * W],
                             start=(kt == 0), stop=False)
            nc.tensor.matmul(ps[:], b4i, VI[0:ks, kt * W:(kt + 1) * W],
                             start=False, stop=(kt == 1))

    # --- uint8 encode with ONE absmax scale per channel (so the host can
    # fold dequant into the expand weights): per-partition |max| via
    # reduce, cross-partition max via PE transpose against identity,
    # then q = rne(clear_q * 127/max + 128) (the f32->uint8 cast is RNE
    # and saturating, verified on HW).
    pm2 = spool.tile([128, 2], F32, tag="pm2")
    nc.vector.reduce_max(pm2[:], ps_o.rearrange("p (t w) -> p t w", t=2),
                         axis=AX.X, apply_absolute_value=True)
    pm = spool.tile([128, 1], F32, tag="pm")
    nc.vector.tensor_tensor(pm[:], pm2[:, 0:1], pm2[:, 1:2], AT.max)
    ps_t = ppool.tile([1, 128], F32, tag="ps_small")
    nc.tensor.matmul(ps_t[:], pm[:], CT["ident"][:], start=True, stop=True)
    m1t = spool.tile([1, 2], F32, tag="m1t")
    nc.vector.reduce_max(m1t[:, 0:1], ps_t[:], axis=AX.X)
    nc.vector.reciprocal(m1t[:, 1:2], m1t[:, 0:1])
    nc.vector.tensor_scalar_mul(m1t[:, 1:2], m1t[:, 1:2], 127.0)
    ps_e = ppool.tile([128, 1], F32, tag="ps_small")
    nc.tensor.matmul(ps_e[:], CT["onesr"][:], m1t[:, 1:2], start=True, stop=True)
    e128 = spool.tile([128, 1], F32, tag="e128")
    nc.vector.tensor_copy(e128[:], ps_e[:])
    img8 = spool.tile([128, 2 * W], U8, tag="img8")
    nc.scalar.activation(img8[:, 0:W], ps_oa, ACT.Copy,
                         bias=128.0, scale=e128[:, 0:1])
    nc.scalar.activation(img8[:, W:2 * W], ps_ob, ACT.Copy,
                         bias=128.0, scale=e128[:, 0:1])
    nc.sync.dma_start(
        clear8_d[0, CH * NPIX + 4 * ci:CH * NPIX + 4 * ci + 4].bitcast(F32)
        .rearrange("(a w) -> a w", a=1), m1t[:, 0:1])
    for t in range(2):
        nc.sync.dma_start(
            clear8_d[0, ci * NPIX + t * 128 * W:ci * NPIX + (t + 1) * 128 * W]
            .bitcast(U8).rearrange("(p w) -> p w", w=W),
            img8[:, t * W:(t + 1) * W])


def _strip1d(nc, spool, ST, row):
    """Accumulate 19*(sum, sumsq) of (clamped-1D-med3(row) - row) into
    ST[0:1, 4:6]. row: [1, W] fp16."""
    h = spool.tile([1, 4 * W], F32, tag="strip1d")
    mnp = h[:, 0:W - 1]
    mxp = h[:, W:2 * W - 1]
    med = h[:, 2 * W:3 * W]
    nc.vector.tensor_tensor(mnp[:], row[:, 0:W - 1], row[:, 1:W], AT.min)
    nc.vector.tensor_tensor(mxp[:], row[:, 0:W - 1], row[:, 1:W], AT.max)
    nc.vector.tensor_tensor(h[:, 3 * W + 1:4 * W - 1], row[:, 0:W - 2],
                            mxp[:, 1:W - 1], AT.min)
    nc.vector.tensor_tensor(med[:, 1:W - 1], h[:, 3 * W + 1:4 * W - 1],
                            mnp[:, 1:W - 1], AT.max)
    nc.vector.tensor_copy(med[:, 0:1], row[:, 0:1])
    nc.vector.tensor_copy(med[:, W - 1:W], row[:, W - 1:W])
    d = h[:, 3 * W:4 * W]
    nc.vector.tensor_sub(d[:], med[:], row[:])
    s = spool.tile([1, 2], F32, tag="strip1d_s")
    nc.vector.reduce_sum(s[:, 0:1], d[:], axis=AX.X)
    d2 = h[:, 0:W]
    nc.vector.tensor_mul(d2[:], d[:], d[:])
    nc.vector.reduce_sum(s[:, 1:2], d2[:], axis=AX.X)
    nc.vector.tensor_scalar_mul(s[:], s[:], 19.0)
    nc.vector.tensor_add(ST[0:1, 4:6], ST[0:1, 4:6], s[:, 0:2])


def _nsr(nc, spool, mpool, ppool, CT, ST, cc, ci):
    u16 = CT["u16"]
    wsq = mpool.tile([128, 2 * W], F16, tag="wsq")
    nc.vector.tensor_mul(wsq[:], cc[:], cc[:])
    # u-weighted rowsums: rowsum[c] for c = m*128+p; cols (2m, 2m+1) = (W, W2)
    ps_u = ppool.tile([128, 8], F32, tag="ps_small")
    for m in range(2):
        for t in range(2):
            nc.tensor.matmul(ps_u[:, 2 * m:2 * m + 1],
                             cc[:, t * W + m * 128: t * W + m * 128 + 128],
                             u16[:, t:t + 1], start=(t == 0), stop=(t == 1))
            nc.tensor.matmul(ps_u[:, 2 * m + 1:2 * m + 2],
                             wsq[:, t * W + m * 128: t * W + m * 128 + 128],
                             u16[:, t:t + 1], start=(t == 0), stop=(t == 1))
    # rsum in f32: with int8-scaled inputs the wsq rowsums reach ~5e6,
    # which overflows fp16.
    rsum = spool.tile([128, 4], F32, tag="rsum32")
    nc.vector.tensor_copy(rsum[:], ps_u[:, 0:4])
    # column weights: m=0 cols weighted by u[:,0] (20 at c=0), m=1 by u[:,1]
    u32 = CT["u32"]
    ps_t = ppool.tile([4, 8], F32, tag="ps_small")
    nc.tensor.matmul(ps_t[0:1, 0:2], u32[:, 0:1], rsum[:, 0:2], start=True, stop=True)
    nc.tensor.matmul(ps_t[0:1, 2:4], u32[:, 1:2], rsum[:, 2:4], start=True, stop=True)
    # D-stat cross-partition sums
    nc.tensor.matmul(ps_t[0:1, 4:8], CT["onesc"][:], ST[:, 0:4], start=True, stop=True)
    sW = spool.tile([1, 16], F32, tag="scal")
    nc.vector.tensor_copy(sW[:, 0:8], ps_t[0:1, 0:8])
    # sums: u-wf = c0+c2 ; u-wf2 = c1+c3 ; sD = c4+c6+ST[0,4] ; sD2 = c5+c7+ST[0,5]
    nc.vector.tensor_add(sW[:, 8:9], sW[:, 0:1], sW[:, 2:3])
    nc.vector.tensor_add(sW[:, 9:10], sW[:, 1:2], sW[:, 3:4])
    nc.vector.tensor_add(sW[:, 10:11], sW[:, 4:5], sW[:, 6:7])
    nc.vector.tensor_add(sW[:, 10:11], sW[:, 10:11], ST[0:1, 4:5])
    nc.vector.tensor_add(sW[:, 11:12], sW[:, 5:6], sW[:, 7:8])
    nc.vector.tensor_add(sW[:, 11:12], sW[:, 11:12], ST[0:1, 5:6])
    ninv, nm1inv = 1.0 / NTOT, 1.0 / (NTOT - 1)
    sc = spool.tile([1, 8], F32, tag="scal2")
    # var_n = (sD2 - sD^2/n)/(n-1)
    nc.vector.tensor_mul(sc[:, 0:1], sW[:, 10:11], sW[:, 10:11])
    nc.vector.tensor_scalar_mul(sc[:, 0:1], sc[:, 0:1], ninv)
    nc.vector.tensor_sub(sc[:, 0:1], sW[:, 11:12], sc[:, 0:1])
    nc.vector.tensor_scalar_mul(sc[:, 0:1], sc[:, 0:1], nm1inv)
    # var_s
    nc.vector.tensor_mul(sc[:, 1:2], sW[:, 8:9], sW[:, 8:9])
    nc.vector.tensor_scalar_mul(sc[:, 1:2], sc[:, 1:2], ninv)
    nc.vector.tensor_sub(sc[:, 1:2], sW[:, 9:10], sc[:, 1:2])
    nc.vector.tensor_scalar_mul(sc[:, 1:2], sc[:, 1:2], nm1inv)
    # rsqrt(var_s) via recip seed + NR. var_s of the int8-scaled image is
    # ~(127/4.7)^2 ~ 730, so seed 27/v ~ 1/sqrt(v) within +-7% (a bare
    # 1/v seed is 27x off and 4 NR steps cannot recover it).
    nc.vector.reciprocal(sc[:, 2:3], sc[:, 1:2])
    nc.vector.tensor_scalar_mul(sc[:, 2:3], sc[:, 2:3], 27.0)
    for _ in range(4):
        nc.vector.tensor_mul(sc[:, 3:4], sc[:, 2:3], sc[:, 2:3])
        nc.vector.tensor_mul(sc[:, 3:4], sc[:, 3:4], sc[:, 1:2])
        nc.vector.tensor_scalar(sc[:, 3:4], sc[:, 3:4], -0.5, 1.5, AT.mult, AT.add)
        nc.vector.tensor_mul(sc[:, 2:3], sc[:, 2:3], sc[:, 3:4])
    nc.vector.tensor_mul(sc[:, 4:5], sc[:, 0:1], sc[:, 2:3])
    nc.vector.tensor_scalar_mul(sc[:, 4:5], sc[:, 4:5], 8.0 / 30.0)
    # nsr was computed from wf_q = wf/s_c; true nsr scales linearly in
    # amplitude, so multiply by the upload scale s_c.
    nsr_r = spool.tile([1, 1], F32, tag="nsr_r")
    nc.vector.tensor_mul(nsr_r[:], sc[:, 4:5], CT["scl"][0:1, ci:ci + 1])
    ps_b = ppool.tile([128, 8], F32, tag="ps_small")
    nc.tensor.matmul(ps_b[:, 0:1], CT["onesr"][:], nsr_r[:], start=True, stop=True)
    nsr128 = spool.tile([128, 1], F32, tag="nsr128")
    nc.vector.tensor_copy(nsr128[:], ps_b[:, 0:1])
    return nsr128


def _make_exec(nc, devs):
    """Build a sharded jit executable over the given device subset."""
    import jax
    from jax.sharding import Mesh, PartitionSpec, NamedSharding
    from jax.experimental.shard_map import shard_map
    from concourse import bass2jax

    bass2jax.install_neuronx_cc_hook()
    partition_name = nc.partition_id_tensor.name if nc.partition_id_tensor else None
    in_names, out_names, out_avals = [], [], []
    for alloc in nc.m.functions[0].allocations:
        if not isinstance(alloc, mybir.MemoryLocationSet):
            continue
        name = alloc.memorylocations[0].name
        if alloc.kind == "ExternalInput":
            if name != partition_name:
                in_names.append(name)
        elif alloc.kind == "ExternalOutput":
            out_names.append(name)
            out_avals.append(jax.core.ShapedArray(
                tuple(alloc.tensor_shape), mybir.dt.np(alloc.dtype)))
    all_in_names = in_names + out_names + ([partition_name] if partition_name else [])

    def _body(*args):
        ops = list(args)
        if partition_name is not None:
            ops.append(bass2jax.partition_id_tensor())
        outs = bass2jax._bass_exec_p.bind(
            *ops, out_avals=tuple(out_avals), in_names=tuple(all_in_names),
            out_names=tuple(out_names), lowering_input_output_aliases=(),
            sim_require_finite=True, sim_require_nnan=True, nc=nc)
        return tuple(outs)

    mesh = Mesh(np.asarray(devs), ("core",))
    n_args = len(in_names) + len(out_names)
    sharded = jax.jit(
        shard_map(_body, mesh=mesh,
                  in_specs=(PartitionSpec("core"),) * n_args,
                  out_specs=(PartitionSpec("core"),) * len(out_names),
                  check_rep=False),
        keep_unused=True)
    sh = NamedSharding(mesh, PartitionSpec("core"))
    return sharded, sh, in_names, out_names, out_avals


def _setup():
    """First-call setup: build program, four 2-core jits (batch b on
    cores 2b, 2b+1), push resident buffers."""
    import jax
    nc = _build()
    consts = _host_constants()
    devices = jax.devices()[:8]
    groups = []
    for g in range(4):
        dd = devices[2 * g:2 * g + 2]
        sharded, sh, in_names, out_names, out_avals = _make_exec(nc, dd)
        resident = {}
        for nm, arr in consts.items():
            resident[nm] = jax.device_put(
                np.concatenate([np.asarray(arr)] * 2, axis=0), sh)
        zeros = [jax.device_put(
            np.zeros((2 * a.shape[0], *a.shape[1:]), a.dtype), sh)
            for a in out_avals]
        groups.append(dict(sharded=sharded, sh=sh, resident=resident,
                           zeros=zeros, devs=list(dd)))
    _CACHE.update(nc=nc, groups=groups, in_names=in_names,
                  devices=list(devices))


def kernel(x, kernel, w_reduce, b_reduce, w_expand, b_expand):
    import jax
    import time
    from concurrent.futures import ThreadPoolExecutor
    if "groups" not in _CACHE:
        _setup()
    dbg = os.environ.get("DWDN_T")
    t00 = time.monotonic()

    def _tr(msg):
        if dbg:
            print(f"[{time.monotonic() - t00:7.3f}] {msg}", flush=True)

    devices = _CACHE["devices"]
    in_names = _CACHE["in_names"]

    x = np.asarray(x, np.float32).reshape(B, NF, NPIX)
    kern = np.asarray(kernel, np.float32)
    w_reduce = np.asarray(w_reduce, np.float32)
    b_reduce = np.asarray(b_reduce, np.float32)
    w_expand = np.asarray(w_expand, np.float32)
    b_expand = np.asarray(b_expand, np.float32)

    pool = _CACHE.setdefault("pool", ThreadPoolExecutor(max_workers=10))
    if "wf8_buf" not in _CACHE:
        _CACHE["wf8_buf"] = np.zeros((8, UPN), np.int8)
        _CACHE["y_buf"] = np.empty((C, NPIX), np.float32)
        _CACHE["t_buf"] = np.empty((CH, NPIX), np.float32)
        claugs = np.empty((B, C + 1, NPIX), np.float32)
        claugs[:, C] = 1.0
        _CACHE["claug_bufs"] = claugs
    wf8 = _CACHE["wf8_buf"]
    ybuf = _CACHE["y_buf"]
    tbuf = _CACHE["t_buf"]
    claugs = _CACHE["claug_bufs"]
    out = np.empty((B, NF, NPIX), np.float32)
    locks = [threading.Lock() for _ in range(B)]
    pending = [2] * B
    s_up = np.empty((B, 2, CH), np.float32)
    done_futs = []

    # quantize half-batch rows into core's flat buffer + sideband; the
    # uint8 cast happens inside a fused saturating numpy add.
    def _pack(core, rows, b, h):
        amax = np.abs(rows).max(axis=1)
        np.maximum(amax, 1e-20, out=amax)
        s_up[b, h] = amax / 127.0
        np.multiply(rows, (127.0 / amax)[:, None], out=tbuf)
        blk = wf8[core, :SB].view(np.uint8).reshape(CH, HB, W)
        np.add(tbuf.reshape(CH, H, W), 128.5, out=blk[:, 1:H + 1, :],
               casting='unsafe')
        blk[:, 0, :] = blk[:, 1, :]
        blk[:, H + 1, :] = blk[:, H, :]
        side = wf8[core, SB:SB + 4 * CH + 4 * KS * KS]
        side[0:4 * CH] = s_up[b, h].view(np.int8)
        side[4 * CH:] = _round_f32r(kern[b, 0]).ravel().view(np.int8)
        _tr(f"pack done core {core}")

    def _put(core):
        r = jax.device_put(wf8[core].reshape(1, UPN), devices[core])
        jax.block_until_ready(r)
        _tr(f"put done core {core}")
        return r

    def _land(shard, b, h):
        # uint8 shard -> f32 claug rows (plain cast; dequant scales are
        # folded into the expand weights); the worker that lands a
        # batch's SECOND shard runs that batch's expand sgemm.
        raw = np.asarray(shard.data).ravel()
        m = raw[CH * NPIX:CH * NPIX + 4 * CH].view(np.float32).copy()
        claugs[b, CH * h:CH * h + CH] = raw[:CH * NPIX].reshape(CH, NPIX)
        _tr(f"fetch done b {b} h {h}")
        cs[b, CH * h:CH * h + CH] = s_up[b, h] * (m / 127.0)
        with locks[b]:
            pending[b] -= 1
            ready = pending[b] == 0
        if ready:
            # fold dequant into the expand weights: clear = (q-128)*cs
            Wb = w_expand * cs[b][None, :]
            Waug = np.concatenate(
                [Wb, (b_expand - 128.0 * Wb.sum(axis=1))[:, None]], axis=1)
            np.dot(Waug, claugs[b], out=out[b])
            _tr(f"expand done b {b}")

    cs = np.empty((B, C), np.float32)

    # single producer thread owns ALL upload-side cpu (the host has one
    # cpu, so parallel sgemms only thrash); puts stream from the pool;
    # the main thread just waits for each batch's puts and dispatches.
    import queue
    ready_q = queue.Queue()

    def _producer():
        for b in range(B):
            y = np.dot(w_reduce, x[b], out=ybuf)    # [64, NPIX]
            y += b_reduce[:, None]
            futs = []
            for h in range(2):
                _pack(2 * b + h, y[CH * h:CH * h + CH], b, h)
                futs.append(pool.submit(_put, 2 * b + h))
            ready_q.put(futs)

    prod = pool.submit(_producer)

    for b in range(B):
        G = _CACHE["groups"][b]
        put_futs = ready_q.get()
        d_wf = jax.make_array_from_single_device_arrays(
            (2, UPN), G["sh"], [f.result() for f in put_futs])
        per_call = {"wf8": d_wf}
        args = [per_call[nm] if nm in per_call else G["resident"][nm]
                for nm in in_names]
        outs = G["sharded"](*args, *G["zeros"])
        _tr(f"dispatched b {b}")
        osh = sorted(outs[0].addressable_shards, key=lambda s: s.index[0].start)
        for i, s in enumerate(osh):
            done_futs.append(pool.submit(_land, s, b, i))

    prod.result()
    for f in done_futs:
        f.result()
    return out.reshape(B, NF, H, W)


if __name__ == "__main__":
    import reference
    inputs = reference.setup_inputs()
    out = kernel(**{k: np.asarray(v) for k, v in inputs.items()})
    print("out", out.shape, out.dtype)


# revision 34
# speedup vs baseline: 1.5306x; 1.0297x over previous
"""DWDN forward (Wiener deconvolution) Trainium2 Bass kernel.

Sharding: 8 cores = 4 batches x 2 channel-halves. Core k handles batch
b=k//2, reduced-channel half h=k%2 (32 of the 64 reduced channels).

The axon link to the cores has a ~140-150 ms fixed cost PER RPC
(device_put / fetch / dispatch, any size) plus ~40-60 MB/s of
bandwidth, and the host has ONE cpu. The design therefore (a) moves
the minimal rank-64 cut of the dataflow (wf/clear), (b) quantizes both
directions to 8 bits -- the Wiener filter is linear and both signal
and quantization noise are white, so the relative error survives the
deconvolution unchanged, ~1e-2 per leg against the 2e-2 gate --, and
(c) packs each direction into ONE rpc per core:
  host:   1x1 reduce conv (BLAS sgemm) -> wf [64,256,256] per batch,
          per-channel absmax uint8 quantization (+128 offset; the cast
          is a fused saturating numpy add) packed WITH the f32 scale
          sideband and the f32r-rounded 19x19 PSF into a single flat
          int8 buffer per core, device_put from worker threads while
          later batches' sgemms run on the main thread
  device: P0 OTF via small DFT matmuls (otf = E @ kern @ E^T), D=|otf|^2
          P2 per channel image: 3x3-median NSR stats (fp16 min/max
          network with border-band decomposition; the int8 decode
          applies bias -128 inside the convert activations; nsr is
          rescaled by the uploaded per-channel scale) + Wiener
          deconvolution as pad-folded DFT matmuls with Hermitian
          symmetry (freq rows k=0..147 of 294); emits clear as uint8
          with one global absmax scale per channel (cross-partition
          max via PE-transpose against an identity), the f32 maxes
          bitcast into the tail of the same uint8 output buffer
  host:   per-channel dequant FOLDED into the expand weights (host
          decode is a single uint8->f32 cast), then 1x1 expand conv +
          bias as sgemm with ones-row augmentation, run inside the
          fetch worker that lands the batch's second shard
Four dispatches on disjoint 2-core meshes (batch b on cores 2b, 2b+1)
so each batch's download starts while later batches' uploads stream.
The sharded jit executables, DFT constants, and zero output buffers
stay device-resident across calls; only wf8 moves per call.

All matmuls in float32r (13-bit mantissa, full PE rate). No ACT LUT
activations and no custom-DVE ops (both crash on this deployment);
division via nc.vector.reciprocal only (seeded x27 for the int8 value
range, since NR from a bare reciprocal seed cannot recover a 27x-off
rsqrt seed in 4 steps).
"""

import os
import sys
import threading
from contextlib import ExitStack

import numpy as np

for _p in ("/opt/trn_rl_repo", "/root/.axon_site/_ro/trn_rl_repo"):
    if os.path.isdir(_p) and _p not in sys.path:
        sys.path.insert(0, _p)

import concourse.tile as tile  # noqa: E402
from concourse import bacc, mybir  # noqa: E402

F32 = mybir.dt.float32
F32R = mybir.dt.float32r
F16 = mybir.dt.float16
I8 = mybir.dt.int8
U8 = mybir.dt.uint8
ACT = mybir.ActivationFunctionType
AT = mybir.AluOpType
AX = mybir.AxisListType

NF, KS, B, H, W = 256, 19, 4, 256, 256
N = H + 2 * KS          # 294
C = 64                  # reduced channels
CH = 32                 # channels per core
KH = N // 2 + 1         # 148 (Hermitian half)
NPIX = H * W
NTOT = N * N
HB = H + 2              # bordered image rows for the median path
LT = [128, 128, 38]     # partition-tile sizes for 294
MT = [128, 20]          # partition-tile sizes for 148
SB = CH * HB * W        # sideband offset in the flat upload buffer
UPN = SB + 2048         # flat upload bytes per core
DWN = CH * NPIX + 128   # flat download bytes per core (128 = CH f32 maxes)


def _round_f32r(x):
    b = np.ascontiguousarray(x, dtype=np.float32).view(np.uint32).astype(np.uint64)
    b = (b + 0x200) & np.uint64(0xFFFFFC00)
    b = np.minimum(b, 0xFFFFFFFF).astype(np.uint32)
    return b.view(np.float32)


def _host_constants():
    k = np.arange(N)
    Fm = np.exp(-2j * np.pi * np.outer(k, k) / N)
    Sx = np.zeros((N, H))
    for i in range(N):
        Sx[i, min(max(i - KS, 0), H - 1)] = 1.0
    A = Fm @ Sx                                   # [294, 256]
    At = A.T                                      # [256, 294]
    B1 = np.conj(Fm)[KS:KS + H, :] / N            # [256, 294]
    wk = np.ones(KH)
    wk[1:KH - 1] = 2.0
    B4 = B1[:, :KH] * wk[None, :]                 # [256, 148]
    E = np.exp(-2j * np.pi * np.outer(k, (np.arange(KS) - KS // 2)) / N)  # [294,19]
    f = lambda v: _round_f32r(np.ascontiguousarray(v, np.float32))
    cst = {
        "AtR": f(At.real), "AtI": f(At.imag), "AtIn": f(-At.imag),
        "B1tR": f(B1.T.real), "B1tI": f(B1.T.imag), "B1tIn": f(-B1.T.imag),
        "B4R": f(B4.T.real), "B4In": f(-B4.T.imag),
        "ER": f(E.T.real), "EI": f(E.T.imag), "EIn": f(-E.T.imag),
        "onesr": np.ones((1, 128), np.float32),
        "onesc": f(np.ones((128, 1), np.float32)),
        "ident": np.eye(128, dtype=np.float32),
    }
    u = np.ones((H, 1), np.float32)
    u[0, 0] = 20.0
    u[-1, 0] = 20.0
    cst["u16"] = u.astype(np.float16)
    cst["u32"] = u
    return cst


_CACHE = {}


def _build():
    nc = bacc.Bacc(None, target_bir_lowering=False)
    P = {}
    P["wf8"] = nc.declare_dram_parameter("wf8", [1, UPN], I8, isOutput=False)
    for nm, shp in [("AtR", [NF, N]), ("AtI", [NF, N]), ("AtIn", [NF, N]),
                    ("B1tR", [N, H]), ("B1tI", [N, H]), ("B1tIn", [N, H]),
                    ("B4R", [KH, H]), ("B4In", [KH, H]),
                    ("ER", [KS, N]), ("EI", [KS, N]), ("EIn", [KS, N]),
                    ("onesc", [128, 1])]:
        P[nm] = nc.declare_dram_parameter(nm, shp, F32R, isOutput=False)
    P["onesr"] = nc.declare_dram_parameter("onesr", [1, 128], F32, isOutput=False)
    P["ident"] = nc.declare_dram_parameter("ident", [128, 128], F32, isOutput=False)
    P["u16"] = nc.declare_dram_parameter("u16", [H, 1], F16, isOutput=False)
    P["u32"] = nc.declare_dram_parameter("u32", [H, 1], F32, isOutput=False)
    clear8_d = nc.declare_dram_parameter("clear8", [1, DWN], U8, isOutput=True)

    with tile.TileContext(nc) as tc:
        _emit(nc, tc, P, clear8_d)
    nc.finalize()
    return nc


def _load_consts(nc, tc, ctx, P):
    cpool = ctx.enter_context(tc.tile_pool(name="consts", bufs=1))
    CT = {}
    for nm in ("AtR", "AtI", "AtIn"):
        t = cpool.tile([128, 2 * N], F32R, tag=nm)
        for i in range(2):
            nc.sync.dma_start(t[:, i * N:(i + 1) * N], P[nm][i * 128:(i + 1) * 128, :])
        CT[nm] = t
    for nm in ("B1tR", "B1tI", "B1tIn"):
        t = cpool.tile([128, 3 * H], F32R, tag=nm)
        for i in range(3):
            rs = LT[i]
            nc.sync.dma_start(t[0:rs, i * H:i * H + H], P[nm][i * 128:i * 128 + rs, :])
        CT[nm] = t
    for nm in ("B4R", "B4In"):
        t = cpool.tile([128, 2 * H], F32R, tag=nm)
        for i in range(2):
            rs = MT[i]
            nc.sync.dma_start(t[0:rs, i * H:i * H + H], P[nm][i * 128:i * 128 + rs, :])
        CT[nm] = t
    for nm in ("ER", "EI", "EIn"):
        t = cpool.tile([KS, N], F32R, tag=nm)
        nc.sync.dma_start(t[:], P[nm][:, :])
        CT[nm] = t
    for nm, shp, dt in [("onesr", [1, 128], F32), ("onesc", [128, 1], F32R),
                        ("ident", [128, 128], F32)]:
        t = cpool.tile(shp, dt, tag=nm)
        nc.sync.dma_start(t[:], P[nm][:, :])
        CT[nm] = t
    u16 = cpool.tile([128, 2], F16, tag="u16")
    for i in range(2):
        nc.sync.dma_start(u16[:, i:i + 1], P["u16"][i * 128:(i + 1) * 128, :])
    CT["u16"] = u16
    u32 = cpool.tile([128, 2], F32, tag="u32")
    for i in range(2):
        nc.sync.dma_start(u32[:, i:i + 1], P["u32"][i * 128:(i + 1) * 128, :])
    CT["u32"] = u32
    # sideband: per-channel upload scales + f32r PSF, bitcast out of the
    # flat int8 upload buffer
    scl_t = cpool.tile([1, CH], F32, tag="scl")
    nc.sync.dma_start(
        scl_t[:], P["wf8"][0, SB:SB + 4 * CH].bitcast(F32)
        .rearrange("(a w) -> a w", a=1))
    CT["scl"] = scl_t
    kern_sb = cpool.tile([KS, KS], F32R, tag="kern")
    nc.sync.dma_start(
        kern_sb[:], P["wf8"][0, SB + 4 * CH:SB + 4 * CH + 4 * KS * KS]
        .bitcast(F32R).rearrange("(p w) -> p w", w=KS))
    CT["kern"] = kern_sb
    # persistent P0 outputs
    CT["otfR"] = cpool.tile([128, 3 * KH], F32, name="otfR", tag="otfR")
    CT["otfI"] = cpool.tile([128, 3 * KH], F32, name="otfI", tag="otfI")
    CT["Dt"] = cpool.tile([128, 3 * KH], F32, name="Dtt", tag="Dtt")
    return CT


def _emit(nc, tc, P, clear8_d):
    with ExitStack() as ctx:
        CT = _load_consts(nc, tc, ctx, P)

        # ---- P0 (scoped pools) ----
        with tc.tile_pool(name="p01", bufs=3) as wp, \
             tc.tile_pool(name="p01ps", bufs=1, space="PSUM") as pp:
            _p0_otf(nc, wp, pp, CT)

        tc.strict_bb_all_engine_barrier()

        # ---- P2 ----
        with tc.tile_pool(name="sw", bufs=2) as spool, \
             tc.tile_pool(name="med", bufs=2) as mpool, \
             tc.tile_pool(name="p2ps", bufs=1, space="PSUM") as ppool:
            for ci in range(CH):
                _image(nc, spool, mpool, ppool, CT, P["wf8"], clear8_d, ci)


def _p0_otf(nc, wp, pp, CT):
    # Tt[j,k] = sum_i kern[i,j] E[k,i]: lhsT=kern [19,19], rhs=ER/EI [19,294]
    ps_a = pp.tile([KS, N], F32, tag="ps_otfa")
    ps_b = pp.tile([KS, N], F32, tag="ps_otfb")
    nc.tensor.matmul(ps_a[:], CT["kern"][:], CT["ER"][:], start=True, stop=True)
    nc.tensor.matmul(ps_b[:], CT["kern"][:], CT["EI"][:], start=True, stop=True)
    TtR = wp.tile([KS, N], F32R, tag="TtR")
    TtI = wp.tile([KS, N], F32R, tag="TtI")
    nc.vector.tensor_copy(TtR[:], ps_a[:])
    nc.vector.tensor_copy(TtI[:], ps_b[:])
    # otf_t[l,k] = sum_j E[l,j] Tt[j,k]: lhsT = E^T-as-[j,l] = ER/EI slices
    for lt in range(3):
        rs = LT[lt]
        lsl = slice(lt * 128, lt * 128 + rs)
        osl = slice(lt * KH, lt * KH + KH)
        ps_r = pp.tile([128, KH], F32, tag="ps_otfr")
        ps_i = pp.tile([128, KH], F32, tag="ps_otfi")
        nc.tensor.matmul(ps_r[0:rs, :], CT["ER"][:, lsl], TtR[:, 0:KH], start=True, stop=False)
        nc.tensor.matmul(ps_r[0:rs, :], CT["EIn"][:, lsl], TtI[:, 0:KH], start=False, stop=True)
        nc.tensor.matmul(ps_i[0:rs, :], CT["ER"][:, lsl], TtI[:, 0:KH], start=True, stop=False)
        nc.tensor.matmul(ps_i[0:rs, :], CT["EI"][:, lsl], TtR[:, 0:KH], start=False, stop=True)
        nc.vector.tensor_copy(CT["otfR"][0:rs, osl], ps_r[0:rs, :])
        nc.vector.tensor_copy(CT["otfI"][0:rs, osl], ps_i[0:rs, :])
        t2 = wp.tile([128, KH], F32, tag="d_tmp")
        nc.vector.tensor_mul(CT["Dt"][0:rs, osl], CT["otfR"][0:rs, osl], CT["otfR"][0:rs, osl])
        nc.vector.tensor_mul(t2[0:rs, :], CT["otfI"][0:rs, osl], CT["otfI"][0:rs, osl])
        nc.vector.tensor_add(CT["Dt"][0:rs, osl], CT["Dt"][0:rs, osl], t2[0:rs, :])


def _image(nc, spool, mpool, ppool, CT, wf8_p, clear8_d, ci):
    # --- loads: uint8(+128) bordered rows; m1/cc/p1 are row-shifted
    # views. bordered row r: 0 = replicate of image row 0, r = image row
    # r-1, 257 = replicate of image row 255. The convert activations
    # apply bias -128, so cc/m1/p1 hold the centered int values exactly.
    cb = ci * HB * W
    m8 = mpool.tile([128, 2 * W], U8, tag="m8")
    c8 = mpool.tile([128, 2 * W], U8, tag="c8")
    p8 = mpool.tile([128, 2 * W], U8, tag="p8")
    for t in range(2):
        base = t * 128
        for tl, off in ((m8, 0), (c8, 1), (p8, 2)):
            nc.sync.dma_start(
                tl[:, t * W:(t + 1) * W],
                wf8_p[0, cb + (base + off) * W:cb + (base + off + 128) * W]
                .bitcast(U8).rearrange("(p w) -> p w", w=W))
    r8 = mpool.tile([1, W], U8, tag="r8")
    nc.sync.dma_start(r8[:], wf8_p[0, cb + 256 * W:cb + 257 * W]
                      .bitcast(U8).rearrange("(a w) -> a w", a=1))
    m1 = mpool.tile([128, 2 * W], F16, tag="m1")
    cc = mpool.tile([128, 2 * W], F16, tag="cc")
    p1 = mpool.tile([128, 2 * W], F16, tag="p1s")
    r255 = mpool.tile([1, W], F16, tag="r255")
    nc.scalar.activation(m1[:], m8[:], ACT.Copy, bias=-128.0)
    nc.scalar.activation(cc[:], c8[:], ACT.Copy, bias=-128.0)
    nc.scalar.activation(p1[:], p8[:], ACT.Copy, bias=-128.0)
    nc.scalar.activation(r255[:], r8[:], ACT.Copy, bias=-128.0)
    # f32r image rows for the DFT path (== cc, the center rows)
    wfr = spool.tile([128, 2 * W], F32R, tag="wfimg")
    nc.scalar.copy(wfr[:], cc[:])

    # --- vertical clamped sort3 -> lo/me/hi ---
    lo = mpool.tile([128, 2 * W], F16, tag="lo")
    me = mpool.tile([128, 2 * W], F16, tag="me")
    hi = mpool.tile([128, 2 * W], F16, tag="hi")
    tv = mpool.tile([128, 2 * W], F16, tag="tv")
    mn = mpool.tile([128, 2 * W], F16, tag="mnv")
    mx = mpool.tile([128, 2 * W], F16, tag="mxv")
    nc.vector.tensor_tensor(mn[:], cc[:], p1[:], AT.min)
    nc.vector.tensor_tensor(mx[:], cc[:], p1[:], AT.max)
    nc.vector.tensor_tensor(lo[:], m1[:], mn[:], AT.min)
    nc.vector.tensor_tensor(hi[:], m1[:], mx[:], AT.max)
    nc.vector.tensor_tensor(tv[:], m1[:], mx[:], AT.min)
    nc.vector.tensor_tensor(me[:], tv[:], mn[:], AT.max)

    # --- horizontal stage -> Mx ---
    Ax = mpool.tile([128, 2 * W], F16, tag="Ax")
    Cx = mpool.tile([128, 2 * W], F16, tag="Cx")
    Bx = mpool.tile([128, 2 * W], F16, tag="Bx")
    Mx = mpool.tile([128, 2 * W], F16, tag="Mx")
    h1 = mpool.tile([128, 2 * W], F16, tag="h1")
    h2 = mpool.tile([128, 2 * W], F16, tag="h2")
    for t in range(2):
        o = t * W
        nc.vector.tensor_tensor(h1[:, o:o + W - 1], lo[:, o:o + W - 1], lo[:, o + 1:o + W], AT.max)
        nc.vector.tensor_tensor(Ax[:, o + 1:o + W - 1], h1[:, o:o + W - 2], lo[:, o + 2:o + W], AT.max)
        nc.vector.tensor_copy(Ax[:, o:o + 1], h1[:, o:o + 1])
        nc.vector.tensor_copy(Ax[:, o + W - 1:o + W], h1[:, o + W - 2:o + W - 1])
        nc.vector.tensor_tensor(h2[:, o:o + W - 1], hi[:, o:o + W - 1], hi[:, o + 1:o + W], AT.min)
        nc.vector.tensor_tensor(Cx[:, o + 1:o + W - 1], h2[:, o:o + W - 2], hi[:, o + 2:o + W], AT.min)
        nc.vector.tensor_copy(Cx[:, o:o + 1], h2[:, o:o + 1])
        nc.vector.tensor_copy(Cx[:, o + W - 1:o + W], h2[:, o + W - 2:o + W - 1])
        nc.vector.tensor_tensor(h1[:, o:o + W - 1], me[:, o:o + W - 1], me[:, o + 1:o + W], AT.min)
        nc.vector.tensor_tensor(h2[:, o:o + W - 1], me[:, o:o + W - 1], me[:, o + 1:o + W], AT.max)
        nc.vector.tensor_tensor(Bx[:, o + 1:o + W - 1], me[:, o:o + W - 2], h2[:, o + 1:o + W - 1], AT.min)
        nc.vector.tensor_tensor(Bx[:, o + 1:o + W - 1], Bx[:, o + 1:o + W - 1], h1[:, o + 1:o + W - 1], AT.max)
        nc.vector.tensor_copy(Bx[:, o:o + 1], me[:, o:o + 1])
        nc.vector.tensor_copy(Bx[:, o + W - 1:o + W], me[:, o + W - 1:o + W])
        nc.vector.tensor_tensor(h1[:, o:o + W], Ax[:, o:o + W], Bx[:, o:o + W], AT.min)
        nc.vector.tensor_tensor(h2[:, o:o + W], Ax[:, o:o + W], Bx[:, o:o + W], AT.max)
        nc.vector.tensor_tensor(h2[:, o:o + W], h2[:, o:o + W], Cx[:, o:o + W], AT.min)
        nc.vector.tensor_tensor(Mx[:, o:o + W], h2[:, o:o + W], h1[:, o:o + W], AT.max)

    # --- D stats ---
    Dc = mpool.tile([128, 2 * W], F16, tag="Dc")
    D2 = mpool.tile([128, 2 * W], F32, tag="D2")
    nc.vector.tensor_sub(Dc[:], Mx[:], cc[:])
    nc.vector.tensor_mul(D2[:], Dc[:], Dc[:])
    ST = spool.tile([128, 8], F32R, tag="ST")
    red = spool.tile([128, 8], F32, tag="red")
    nc.vector.reduce_sum(red[:, 0:2], Dc.rearrange("p (t w) -> p t w", t=2), axis=AX.X)
    nc.vector.reduce_sum(red[:, 2:4], D2.rearrange("p (t w) -> p t w", t=2), axis=AX.X)
    nc.vector.tensor_add(ST[:, 0:1], red[:, 0:1], red[:, 1:2])
    nc.vector.tensor_add(ST[:, 1:2], red[:, 2:3], red[:, 3:4])
    # left/right vertical-median strips (weight 19)
    sl_ = spool.tile([128, 12], F32, tag="strips")
    nc.vector.tensor_sub(sl_[:, 0:1], me[:, 0:1], cc[:, 0:1])
    nc.vector.tensor_sub(sl_[:, 1:2], me[:, W - 1:W], cc[:, W - 1:W])
    nc.vector.tensor_sub(sl_[:, 2:3], me[:, W:W + 1], cc[:, W:W + 1])
    nc.vector.tensor_sub(sl_[:, 3:4], me[:, 2 * W - 1:2 * W], cc[:, 2 * W - 1:2 * W])
    for q in range(4):
        nc.vector.tensor_mul(sl_[:, 4 + q:5 + q], sl_[:, q:q + 1], sl_[:, q:q + 1])
    nc.vector.tensor_add(sl_[:, 8:9], sl_[:, 0:1], sl_[:, 1:2])
    nc.vector.tensor_add(sl_[:, 9:10], sl_[:, 2:3], sl_[:, 3:4])
    nc.vector.tensor_add(sl_[:, 8:9], sl_[:, 8:9], sl_[:, 9:10])
    nc.vector.tensor_add(sl_[:, 10:11], sl_[:, 4:5], sl_[:, 5:6])
    nc.vector.tensor_add(sl_[:, 11:12], sl_[:, 6:7], sl_[:, 7:8])
    nc.vector.tensor_add(sl_[:, 10:11], sl_[:, 10:11], sl_[:, 11:12])
    nc.vector.tensor_scalar_mul(ST[:, 2:3], sl_[:, 8:9], 19.0)
    nc.vector.tensor_scalar_mul(ST[:, 3:4], sl_[:, 10:11], 19.0)
    # zero ST[0:1, 4:6] then accumulate the two 1-D strips (x19 inside)
    nc.vector.tensor_scalar_mul(ST[0:1, 4:6], CT["onesr"][0:1, 0:2], 0.0)
    _strip1d(nc, spool, ST, cc[0:1, 0:W])
    _strip1d(nc, spool, ST, r255[0:1, 0:W])

    nsr128 = _nsr(nc, spool, mpool, ppool, CT, ST, cc, ci)

    # --- S1: Ut[c,k] = sum_r wf[r,c] At[r,k] ---
    ps_ut = ppool.tile([128, 2 * KH], F32, tag="ps_ut")
    ps_uti = ppool.tile([128, 2 * KH], F32, tag="ps_uti")
    for mt in range(2):
        for kt in range(2):
            lhs = wfr[:, kt * W + mt * 128: kt * W + mt * 128 + 128]
            nc.tensor.matmul(ps_ut[:, mt * KH:(mt + 1) * KH], lhs,
                             CT["AtR"][:, kt * N: kt * N + KH],
                             start=(kt == 0), stop=(kt == 1))
            nc.tensor.matmul(ps_uti[:, mt * KH:(mt + 1) * KH], lhs,
                             CT["AtI"][:, kt * N: kt * N + KH],
                             start=(kt == 0), stop=(kt == 1))
    UtR = spool.tile([128, 2 * KH], F32R, tag="UtR")
    UtI = spool.tile([128, 2 * KH], F32R, tag="UtI")
    nc.vector.tensor_copy(UtR[:], ps_ut[:])
    nc.scalar.copy(UtI[:], ps_uti[:])

    # --- S2: Gt[l,k] = sum_c At[c,l] Ut[c,k] (complex) ---
    ps_gr = ppool.tile([128, 3 * KH], F32, tag="ps_gr")
    ps_gi = ppool.tile([128, 3 * KH], F32, tag="ps_gi")
    for lt in range(3):
        rs = LT[lt]
        o = slice(lt * KH, lt * KH + KH)
        for kt in range(2):
            lA = slice(kt * N + lt * 128, kt * N + lt * 128 + rs)
            uR = UtR[:, kt * KH:(kt + 1) * KH]
            uI = UtI[:, kt * KH:(kt + 1) * KH]
            nc.tensor.matmul(ps_gr[0:rs, o], CT["AtR"][:, lA], uR, start=(kt == 0), stop=False)
            nc.tensor.matmul(ps_gi[0:rs, o], CT["AtR"][:, lA], uI, start=(kt == 0), stop=False)
        for kt in range(2):
            lA = slice(kt * N + lt * 128, kt * N + lt * 128 + rs)
            uR = UtR[:, kt * KH:(kt + 1) * KH]
            uI = UtI[:, kt * KH:(kt + 1) * KH]
            nc.tensor.matmul(ps_gr[0:rs, o], CT["AtIn"][:, lA], uI, start=False, stop=(kt == 1))
            nc.tensor.matmul(ps_gi[0:rs, o], CT["AtI"][:, lA], uR, start=False, stop=(kt == 1))

    # --- Wiener elementwise -> Zt (f32r) ---
    ZtR = spool.tile([128, 3 * KH], F32R, tag="ZtR")
    ZtI = spool.tile([128, 3 * KH], F32R, tag="ZtI")
    Rv = spool.tile([128, 3 * KH], F32, tag="Rv")
    t1 = spool.tile([128, KH], F32, tag="wt1")
    t2 = spool.tile([128, KH], F32, tag="wt2")
    for lt in range(3):
        rs = LT[lt]
        o = slice(lt * KH, lt * KH + KH)
        nc.vector.tensor_scalar_add(Rv[0:rs, o], CT["Dt"][0:rs, o], nsr128[0:rs, 0:1])
        nc.vector.reciprocal(Rv[0:rs, o], Rv[0:rs, o])
        nc.vector.tensor_mul(t1[0:rs, :], ps_gr[0:rs, o], CT["otfR"][0:rs, o])
        nc.vector.tensor_mul(t2[0:rs, :], ps_gi[0:rs, o], CT["otfI"][0:rs, o])
        nc.vector.tensor_add(t1[0:rs, :], t1[0:rs, :], t2[0:rs, :])
        nc.vector.tensor_mul(ZtR[0:rs, o], t1[0:rs, :], Rv[0:rs, o])
        nc.vector.tensor_mul(t1[0:rs, :], ps_gi[0:rs, o], CT["otfR"][0:rs, o])
        nc.vector.tensor_mul(t2[0:rs, :], ps_gr[0:rs, o], CT["otfI"][0:rs, o])
        nc.vector.tensor_sub(t1[0:rs, :], t1[0:rs, :], t2[0:rs, :])
        nc.vector.tensor_mul(ZtI[0:rs, o], t1[0:rs, :], Rv[0:rs, o])

    # --- S3: V[k,j] = sum_l Zt[l,k] B1t[l,j] (complex) ---
    ps_vr = ppool.tile([128, 2 * W], F32, tag="ps_vr")
    ps_vi = ppool.tile([128, 2 * W], F32, tag="ps_vi")
    for mt in range(2):
        ms = MT[mt]
        o = slice(mt * W, mt * W + W)
        for lt in range(3):
            rs = LT[lt]
            zR = ZtR[0:rs, lt * KH + mt * 128: lt * KH + mt * 128 + ms]
            zI = ZtI[0:rs, lt * KH + mt * 128: lt * KH + mt * 128 + ms]
            bR = CT["B1tR"][0:rs, lt * H:(lt + 1) * H]
            bI = CT["B1tI"][0:rs, lt * H:(lt + 1) * H]
            bIn = CT["B1tIn"][0:rs, lt * H:(lt + 1) * H]
            nc.tensor.matmul(ps_vr[0:ms, o], zR, bR, start=(lt == 0), stop=False)
            nc.tensor.matmul(ps_vi[0:ms, o], zR, bI, start=(lt == 0), stop=False)
            nc.tensor.matmul(ps_vr[0:ms, o], zI, bIn, start=False, stop=(lt == 2))
            nc.tensor.matmul(ps_vi[0:ms, o], zI, bR, start=False, stop=(lt == 2))
    VR = spool.tile([128, 2 * W], F32R, tag="VR")
    VI = spool.tile([128, 2 * W], F32R, tag="VI")
    nc.vector.tensor_copy(VR[:, 0:W], ps_vr[:, 0:W])
    nc.scalar.copy(VI[:, 0:W], ps_vi[:, 0:W])
    nc.vector.tensor_copy(VR[0:MT[1], W:2 * W], ps_vr[0:MT[1], W:2 * W])
    nc.scalar.copy(VI[0:MT[1], W:2 * W], ps_vi[0:MT[1], W:2 * W])

    # --- S4: img[i,j] = sum_k B4[i,k] VR[k,j] - B4i[i,k] VI[k,j] ---
    ps_o = ppool.tile([128, 2 * W], F32, tag="ps_img")
    ps_oa = ps_o[:, 0:W]
    ps_ob = ps_o[:, W:2 * W]
    for mt, ps in ((0, ps_oa), (1, ps_ob)):
        for kt in range(2):
            ks = MT[kt]
            b4r = CT["B4R"][0:ks, kt * H + mt * 128: kt * H + mt * 128 + 128]
            b4i = CT["B4In"][0:ks, kt * H + mt * 128: kt * H + mt * 128 + 128]
            nc.tensor.matmul(ps[:], b4r, VR[0:ks, kt * W:(kt + 1) * W],
                             start=(kt == 0), stop=False)
            nc.tensor.matmul(ps[:], b4i, VI[0:ks, kt * W:(kt + 1) * W],
                             start=False, stop=(kt == 1))

    # --- uint8 encode with ONE absmax scale per channel (so the host can
    # fold dequant into the expand weights): per-partition |max| via
    # reduce, cross-partition max via PE transpose against identity,
    # then q = rne(clear_q * 127/max + 128) (the f32->uint8 cast is RNE
    # and saturating, verified on HW).
    pm2 = spool.tile([128, 2], F32, tag="pm2")
    nc.vector.reduce_max(pm2[:], ps_o.rearrange("p (t w) -> p t w", t=2),
                         axis=AX.X, apply_absolute_value=True)
    pm = spool.tile([128, 1], F32, tag="pm")
    nc.vector.tensor_tensor(pm[:], pm2[:, 0:1], pm2[:, 1:2], AT.max)
    ps_t = ppool.tile([1, 128], F32, tag="ps_small")
    nc.tensor.matmul(ps_t[:], pm[:], CT["ident"][:], start=True, stop=True)
    m1t = spool.tile([1, 2], F32, tag="m1t")
    nc.vector.reduce_max(m1t[:, 0:1], ps_t[:], axis=AX.X)
    nc.vector.reciprocal(m1t[:, 1:2], m1t[:, 0:1])
    nc.vector.tensor_scalar_mul(m1t[:, 1:2], m1t[:, 1:2], 127.0)
    ps_e = ppool.tile([128, 1], F32, tag="ps_small")
    nc.tensor.matmul(ps_e[:], CT["onesr"][:], m1t[:, 1:2], start=True, stop=True)
    e128 = spool.tile([128, 1], F32, tag="e128")
    nc.vector.tensor_copy(e128[:], ps_e[:])
    img8 = spool.tile([128, 2 * W], U8, tag="img8")
    nc.scalar.activation(img8[:, 0:W], ps_oa, ACT.Copy,
                         bias=128.0, scale=e128[:, 0:1])
    nc.scalar.activation(img8[:, W:2 * W], ps_ob, ACT.Copy,
                         bias=128.0, scale=e128[:, 0:1])
    nc.sync.dma_start(
        clear8_d[0, CH * NPIX + 4 * ci:CH * NPIX + 4 * ci + 4].bitcast(F32)
        .rearrange("(a w) -> a w", a=1), m1t[:, 0:1])
    for t in range(2):
        nc.sync.dma_start(
            clear8_d[0, ci * NPIX + t * 128 * W:ci * NPIX + (t + 1) * 128 * W]
            .bitcast(U8).rearrange("(p w) -> p w", w=W),
            img8[:, t * W:(t + 1) * W])


def _strip1d(nc, spool, ST, row):
    """Accumulate 19*(sum, sumsq) of (clamped-1D-med3(row) - row) into
    ST[0:1, 4:6]. row: [1, W] fp16."""
    h = spool.tile([1, 4 * W], F32, tag="strip1d")
    mnp = h[:, 0:W - 1]
    mxp = h[:, W:2 * W - 1]
    med = h[:, 2 * W:3 * W]
    nc.vector.tensor_tensor(mnp[:], row[:, 0:W - 1], row[:, 1:W], AT.min)
    nc.vector.tensor_tensor(mxp[:], row[:, 0:W - 1], row[:, 1:W], AT.max)
    nc.vector.tensor_tensor(h[:, 3 * W + 1:4 * W - 1], row[:, 0:W - 2],
                            mxp[:, 1:W - 1], AT.min)
    nc.vector.tensor_tensor(med[:, 1:W - 1], h[:, 3 * W + 1:4 * W - 1],
                            mnp[:, 1:W - 1], AT.max)
    nc.vector.tensor_copy(med[:, 0:1], row[:, 0:1])
    nc.vector.tensor_copy(med[:, W - 1:W], row[:, W - 1:W])
    d = h[:, 3 * W:4 * W]
    nc.vector.tensor_sub(d[:], med[:], row[:])
    s = spool.tile([1, 2], F32, tag="strip1d_s")
    nc.vector.reduce_sum(s[:, 0:1], d[:], axis=AX.X)
    d2 = h[:, 0:W]
    nc.vector.tensor_mul(d2[:], d[:], d[:])
    nc.vector.reduce_sum(s[:, 1:2], d2[:], axis=AX.X)
    nc.vector.tensor_scalar_mul(s[:], s[:], 19.0)
    nc.vector.tensor_add(ST[0:1, 4:6], ST[0:1, 4:6], s[:, 0:2])


def _nsr(nc, spool, mpool, ppool, CT, ST, cc, ci):
    u16 = CT["u16"]
    wsq = mpool.tile([128, 2 * W], F16, tag="wsq")
    nc.vector.tensor_mul(wsq[:], cc[:], cc[:])
    # u-weighted rowsums: rowsum[c] for c = m*128+p; cols (2m, 2m+1) = (W, W2)
    ps_u = ppool.tile([128, 8], F32, tag="ps_small")
    for m in range(2):
        for t in range(2):
            nc.tensor.matmul(ps_u[:, 2 * m:2 * m + 1],
                             cc[:, t * W + m * 128: t * W + m * 128 + 128],
                             u16[:, t:t + 1], start=(t == 0), stop=(t == 1))
            nc.tensor.matmul(ps_u[:, 2 * m + 1:2 * m + 2],
                             wsq[:, t * W + m * 128: t * W + m * 128 + 128],
                             u16[:, t:t + 1], start=(t == 0), stop=(t == 1))
    # rsum in f32: with int8-scaled inputs the wsq rowsums reach ~5e6,
    # which overflows fp16.
    rsum = spool.tile([128, 4], F32, tag="rsum32")
    nc.vector.tensor_copy(rsum[:], ps_u[:, 0:4])
    # column weights: m=0 cols weighted by u[:,0] (20 at c=0), m=1 by u[:,1]
    u32 = CT["u32"]
    ps_t = ppool.tile([4, 8], F32, tag="ps_small")
    nc.tensor.matmul(ps_t[0:1, 0:2], u32[:, 0:1], rsum[:, 0:2], start=True, stop=True)
    nc.tensor.matmul(ps_t[0:1, 2:4], u32[:, 1:2], rsum[:, 2:4], start=True, stop=True)
    # D-stat cross-partition sums
    nc.tensor.matmul(ps_t[0:1, 4:8], CT["onesc"][:], ST[:, 0:4], start=True, stop=True)
    sW = spool.tile([1, 16], F32, tag="scal")
    nc.vector.tensor_copy(sW[:, 0:8], ps_t[0:1, 0:8])
    # sums: u-wf = c0+c2 ; u-wf2 = c1+c3 ; sD = c4+c6+ST[0,4] ; sD2 = c5+c7+ST[0,5]
    nc.vector.tensor_add(sW[:, 8:9], sW[:, 0:1], sW[:, 2:3])
    nc.vector.tensor_add(sW[:, 9:10], sW[:, 1:2], sW[:, 3:4])
    nc.vector.tensor_add(sW[:, 10:11], sW[:, 4:5], sW[:, 6:7])
    nc.vector.tensor_add(sW[:, 10:11], sW[:, 10:11], ST[0:1, 4:5])
    nc.vector.tensor_add(sW[:, 11:12], sW[:, 5:6], sW[:, 7:8])
    nc.vector.tensor_add(sW[:, 11:12], sW[:, 11:12], ST[0:1, 5:6])
    ninv, nm1inv = 1.0 / NTOT, 1.0 / (NTOT - 1)
    sc = spool.tile([1, 8], F32, tag="scal2")
    # var_n = (sD2 - sD^2/n)/(n-1)
    nc.vector.tensor_mul(sc[:, 0:1], sW[:, 10:11], sW[:, 10:11])
    nc.vector.tensor_scalar_mul(sc[:, 0:1], sc[:, 0:1], ninv)
    nc.vector.tensor_sub(sc[:, 0:1], sW[:, 11:12], sc[:, 0:1])
    nc.vector.tensor_scalar_mul(sc[:, 0:1], sc[:, 0:1], nm1inv)
    # var_s
    nc.vector.tensor_mul(sc[:, 1:2], sW[:, 8:9], sW[:, 8:9])
    nc.vector.tensor_scalar_mul(sc[:, 1:2], sc[:, 1:2], ninv)
    nc.vector.tensor_sub(sc[:, 1:2], sW[:, 9:10], sc[:, 1:2])
    nc.vector.tensor_scalar_mul(sc[:, 1:2], sc[:, 1:2], nm1inv)
    # rsqrt(var_s) via recip seed + NR. var_s of the int8-scaled image is
    # ~(127/4.7)^2 ~ 730, so seed 27/v ~ 1/sqrt(v) within +-7% (a bare
    # 1/v seed is 27x off and 4 NR steps cannot recover it).
    nc.vector.reciprocal(sc[:, 2:3], sc[:, 1:2])
    nc.vector.tensor_scalar_mul(sc[:, 2:3], sc[:, 2:3], 27.0)
    for _ in range(4):
        nc.vector.tensor_mul(sc[:, 3:4], sc[:, 2:3], sc[:, 2:3])
        nc.vector.tensor_mul(sc[:, 3:4], sc[:, 3:4], sc[:, 1:2])
        nc.vector.tensor_scalar(sc[:, 3:4], sc[:, 3:4], -0.5, 1.5, AT.mult, AT.add)
        nc.vector.tensor_mul(sc[:, 2:3], sc[:, 2:3], sc[:, 3:4])
    nc.vector.tensor_mul(sc[:, 4:5], sc[:, 0:1], sc[:, 2:3])
    nc.vector.tensor_scalar_mul(sc[:, 4:5], sc[:, 4:5], 8.0 / 30.0)
    # nsr was computed from wf_q = wf/s_c; true nsr scales linearly in
    # amplitude, so multiply by the upload scale s_c.
    nsr_r = spool.tile([1, 1], F32, tag="nsr_r")
    nc.vector.tensor_mul(nsr_r[:], sc[:, 4:5], CT["scl"][0:1, ci:ci + 1])
    ps_b = ppool.tile([128, 8], F32, tag="ps_small")
    nc.tensor.matmul(ps_b[:, 0:1], CT["onesr"][:], nsr_r[:], start=True, stop=True)
    nsr128 = spool.tile([128, 1], F32, tag="nsr128")
    nc.vector.tensor_copy(nsr128[:], ps_b[:, 0:1])
    return nsr128


def _make_exec(nc, devs):
    """Build a sharded jit executable over the given device subset."""
    import jax
    from jax.sharding import Mesh, PartitionSpec, NamedSharding
    from jax.experimental.shard_map import shard_map
    from concourse import bass2jax

    bass2jax.install_neuronx_cc_hook()
    partition_name = nc.partition_id_tensor.name if nc.partition_id_tensor else None
    in_names, out_names, out_avals = [], [], []
    for alloc in nc.m.functions[0].allocations:
        if not isinstance(alloc, mybir.MemoryLocationSet):
            continue
        name = alloc.memorylocations[0].name
        if alloc.kind == "ExternalInput":
            if name != partition_name:
                in_names.append(name)
        elif alloc.kind == "ExternalOutput":
            out_names.append(name)
            out_avals.append(jax.core.ShapedArray(
                tuple(alloc.tensor_shape), mybir.dt.np(alloc.dtype)))
    all_in_names = in_names + out_names + ([partition_name] if partition_name else [])

    def _body(*args):
        ops = list(args)
        if partition_name is not None:
            ops.append(bass2jax.partition_id_tensor())
        outs = bass2jax._bass_exec_p.bind(
            *ops, out_avals=tuple(out_avals), in_names=tuple(all_in_names),
            out_names=tuple(out_names), lowering_input_output_aliases=(),
            sim_require_finite=True, sim_require_nnan=True, nc=nc)
        return tuple(outs)

    mesh = Mesh(np.asarray(devs), ("core",))
    n_args = len(in_names) + len(out_names)
    sharded = jax.jit(
        shard_map(_body, mesh=mesh,
                  in_specs=(PartitionSpec("core"),) * n_args,
                  out_specs=(PartitionSpec("core"),) * len(out_names),
                  check_rep=False),
        keep_unused=True)
    sh = NamedSharding(mesh, PartitionSpec("core"))
    return sharded, sh, in_names, out_names, out_avals


def _setup():
    """First-call setup: build program, four 2-core jits (batch b on
    cores 2b, 2b+1), push resident buffers."""
    import jax
    nc = _build()
    consts = _host_constants()
    devices = jax.devices()[:8]
    groups = []
    for g in range(4):
        dd = devices[2 * g:2 * g + 2]
        sharded, sh, in_names, out_names, out_avals = _make_exec(nc, dd)
        resident = {}
        for nm, arr in consts.items():
            resident[nm] = jax.device_put(
                np.concatenate([np.asarray(arr)] * 2, axis=0), sh)
        zeros = [jax.device_put(
            np.zeros((2 * a.shape[0], *a.shape[1:]), a.dtype), sh)
            for a in out_avals]
        # AOT-compile so per-call dispatch skips jit tracing/validation
        d_wf0 = jax.device_put(np.zeros((2, UPN), np.int8), sh)
        full_args = [d_wf0 if nm == "wf8" else resident[nm]
                     for nm in in_names] + zeros
        compiled = sharded.lower(*full_args).compile()
        groups.append(dict(sharded=sharded, exec=compiled, sh=sh,
                           resident=resident, zeros=zeros, devs=list(dd)))
    _CACHE.update(nc=nc, groups=groups, in_names=in_names,
                  devices=list(devices))


def kernel(x, kernel, w_reduce, b_reduce, w_expand, b_expand):
    import jax
    import time
    from concurrent.futures import ThreadPoolExecutor
    if "groups" not in _CACHE:
        _setup()
    dbg = os.environ.get("DWDN_T")
    t00 = time.monotonic()

    def _tr(msg):
        if dbg:
            print(f"[{time.monotonic() - t00:7.3f}] {msg}", flush=True)

    devices = _CACHE["devices"]
    in_names = _CACHE["in_names"]

    x = np.asarray(x, np.float32).reshape(B, NF, NPIX)
    kern = np.asarray(kernel, np.float32)
    w_reduce = np.asarray(w_reduce, np.float32)
    b_reduce = np.asarray(b_reduce, np.float32)
    w_expand = np.asarray(w_expand, np.float32)
    b_expand = np.asarray(b_expand, np.float32)

    pool = _CACHE.setdefault("pool", ThreadPoolExecutor(max_workers=10))
    if "wf8_buf" not in _CACHE:
        _CACHE["wf8_buf"] = np.zeros((8, UPN), np.int8)
        _CACHE["y_buf"] = np.empty((C, NPIX), np.float32)
        _CACHE["t_buf"] = np.empty((CH, NPIX), np.float32)
        claugs = np.empty((B, C + 1, NPIX), np.float32)
        claugs[:, C] = 1.0
        _CACHE["claug_bufs"] = claugs
    wf8 = _CACHE["wf8_buf"]
    ybuf = _CACHE["y_buf"]
    tbuf = _CACHE["t_buf"]
    claugs = _CACHE["claug_bufs"]
    if "out_buf" not in _CACHE:
        _CACHE["out_buf"] = np.empty((B, NF, NPIX), np.float32)
    out = _CACHE["out_buf"]
    locks = [threading.Lock() for _ in range(B)]
    pending = [2] * B
    s_up = np.empty((B, 2, CH), np.float32)
    done_futs = []

    # quantize half-batch rows into core's flat buffer + sideband; the
    # uint8 cast happens inside a fused saturating numpy add. The reduce
    # bias is folded into the quantize offset (scale from the bound
    # max|y|+|b| so no clipping is possible), skipping a full bias pass.
    def _pack(core, rows, b, h):
        bsl = b_reduce[CH * h:CH * h + CH]
        amax = np.abs(rows).max(axis=1)
        amax += np.abs(bsl)
        np.maximum(amax, 1e-20, out=amax)
        s_up[b, h] = amax / 127.0
        sc127 = 127.0 / amax
        np.multiply(rows, sc127[:, None], out=tbuf)
        blk = wf8[core, :SB].view(np.uint8).reshape(CH, HB, W)
        np.add(tbuf.reshape(CH, H, W), (128.5 + bsl * sc127)[:, None, None],
               out=blk[:, 1:H + 1, :], casting='unsafe')
        blk[:, 0, :] = blk[:, 1, :]
        blk[:, H + 1, :] = blk[:, H, :]
        side = wf8[core, SB:SB + 4 * CH + 4 * KS * KS]
        side[0:4 * CH] = s_up[b, h].view(np.int8)
        side[4 * CH:] = _round_f32r(kern[b, 0]).ravel().view(np.int8)
        _tr(f"pack done core {core}")

    def _put(core):
        r = jax.device_put(wf8[core].reshape(1, UPN), devices[core])
        jax.block_until_ready(r)
        _tr(f"put done core {core}")
        return r

    def _land(shard, b, h):
        # uint8 shard -> f32 claug rows (plain cast; dequant scales are
        # folded into the expand weights); the worker that lands a
        # batch's SECOND shard runs that batch's expand sgemm.
        raw = np.asarray(shard.data).ravel()
        m = raw[CH * NPIX:CH * NPIX + 4 * CH].view(np.float32).copy()
        claugs[b, CH * h:CH * h + CH] = raw[:CH * NPIX].reshape(CH, NPIX)
        _tr(f"fetch done b {b} h {h}")
        cs[b, CH * h:CH * h + CH] = s_up[b, h] * (m / 127.0)
        with locks[b]:
            pending[b] -= 1
            ready = pending[b] == 0
        if ready:
            # fold dequant into the expand weights: clear = (q-128)*cs
            _tr(f"expand start b {b}")
            Wb = w_expand * cs[b][None, :]
            Waug = np.concatenate(
                [Wb, (b_expand - 128.0 * Wb.sum(axis=1))[:, None]], axis=1)
            _tr(f"expand gemm start b {b}")
            np.dot(Waug, claugs[b], out=out[b])
            _tr(f"expand done b {b}")

    cs = np.empty((B, C), np.float32)

    # single producer thread owns ALL upload-side cpu (the host has one
    # cpu, so parallel sgemms only thrash); puts stream from the pool;
    # the main thread just waits for each batch's puts and dispatches.
    import queue
    ready_q = queue.Queue()

    def _producer():
        for b in range(B):
            y = np.dot(w_reduce, x[b], out=ybuf)    # [64, NPIX], bias folded
            futs = []
            for h in range(2):
                _pack(2 * b + h, y[CH * h:CH * h + CH], b, h)
                futs.append(pool.submit(_put, 2 * b + h))
            ready_q.put(futs)

    prod = pool.submit(_producer)

    for b in range(B):
        G = _CACHE["groups"][b]
        put_futs = ready_q.get()
        d_wf = jax.make_array_from_single_device_arrays(
            (2, UPN), G["sh"], [f.result() for f in put_futs])
        args = [d_wf if nm == "wf8" else G["resident"][nm]
                for nm in in_names]
        outs = G["exec"](*args, *G["zeros"])
        _tr(f"dispatched b {b}")
        osh = sorted(outs[0].addressable_shards, key=lambda s: s.index[0].start)
        for i, s in enumerate(osh):
            done_futs.append(pool.submit(_land, s, b, i))

    prod.result()
    for f in done_futs:
        f.result()
    return out.reshape(B, NF, H, W)


if __name__ == "__main__":
    import reference
    inputs = reference.setup_inputs()
    out = kernel(**{k: np.asarray(v) for k, v in inputs.items()})
    print("out", out.shape, out.dtype)


# revision 45
# speedup vs baseline: 1.8488x; 1.2079x over previous
"""DWDN forward (Wiener deconvolution) Trainium2 Bass kernel.

Sharding: 8 cores = 4 batches x 2 channel-halves. Core k handles batch
b=k//2, reduced-channel half h=k%2 (32 of the 64 reduced channels).

The axon link to the cores has a ~140-150 ms fixed cost PER RPC
(device_put / fetch / dispatch, any size) plus ~40-60 MB/s of
bandwidth, and the host has ONE cpu. The design therefore (a) moves
the minimal rank-64 cut of the dataflow (wf/clear), (b) quantizes both
directions to 8 bits -- the Wiener filter is linear and both signal
and quantization noise are white, so the relative error survives the
deconvolution unchanged, ~1e-2 per leg against the 2e-2 gate --, and
(c) packs each direction into ONE rpc per core:
  host:   1x1 reduce conv (BLAS sgemm) -> wf [64,256,256] per batch,
          per-channel absmax uint8 quantization (+128 offset; the cast
          is a fused saturating numpy add) packed WITH the f32 scale
          sideband and the f32r-rounded 19x19 PSF into a single flat
          int8 buffer per core, device_put from worker threads while
          later batches' sgemms run on the main thread
  device: P0 OTF via small DFT matmuls (otf = E @ kern @ E^T), D=|otf|^2
          P2 per channel image: 3x3-median NSR stats (fp16 min/max
          network with border-band decomposition; the int8 decode
          applies bias -128 inside the convert activations; nsr is
          rescaled by the uploaded per-channel scale) + Wiener
          deconvolution as pad-folded DFT matmuls with Hermitian
          symmetry (freq rows k=0..147 of 294); emits clear as uint8
          with one global absmax scale per channel (cross-partition
          max via PE-transpose against an identity), the f32 maxes
          bitcast into the tail of the same uint8 output buffer
  host:   per-channel dequant FOLDED into the expand weights (host
          decode is a single uint8->f32 cast), then 1x1 expand conv +
          bias as sgemm with ones-row augmentation, run inside the
          fetch worker that lands the batch's second shard
Four dispatches on disjoint 2-core meshes (batch b on cores 2b, 2b+1)
so each batch's download starts while later batches' uploads stream.
The sharded jit executables, DFT constants, and zero output buffers
stay device-resident across calls; only wf8 moves per call.

All matmuls in float32r (13-bit mantissa, full PE rate). No ACT LUT
activations and no custom-DVE ops (both crash on this deployment);
division via nc.vector.reciprocal only (seeded x27 for the int8 value
range, since NR from a bare reciprocal seed cannot recover a 27x-off
rsqrt seed in 4 steps).
"""

import os
import sys
from contextlib import ExitStack

import numpy as np

for _p in ("/opt/trn_rl_repo", "/root/.axon_site/_ro/trn_rl_repo"):
    if os.path.isdir(_p) and _p not in sys.path:
        sys.path.insert(0, _p)

import concourse.tile as tile  # noqa: E402
from concourse import bacc, mybir  # noqa: E402

F32 = mybir.dt.float32
F32R = mybir.dt.float32r
F16 = mybir.dt.float16
I8 = mybir.dt.int8
U8 = mybir.dt.uint8
ACT = mybir.ActivationFunctionType
AT = mybir.AluOpType
AX = mybir.AxisListType

NF, KS, B, H, W = 256, 19, 4, 256, 256
N = H + 2 * KS          # 294
C = 64                  # reduced channels
CH = 32                 # channels per core
KH = N // 2 + 1         # 148 (Hermitian half)
NPIX = H * W
NTOT = N * N
HB = H + 2              # bordered image rows for the median path
LT = [128, 128, 38]     # partition-tile sizes for 294
MT = [128, 20]          # partition-tile sizes for 148
SB = CH * HB * W        # sideband offset in the flat upload buffer
UPN = SB + 2048         # flat upload bytes per core
DWN = CH * NPIX + 128   # flat download bytes per core (128 = CH f32 maxes)


def _round_f32r(x):
    b = np.ascontiguousarray(x, dtype=np.float32).view(np.uint32).astype(np.uint64)
    b = (b + 0x200) & np.uint64(0xFFFFFC00)
    b = np.minimum(b, 0xFFFFFFFF).astype(np.uint32)
    return b.view(np.float32)


def _host_constants():
    k = np.arange(N)
    Fm = np.exp(-2j * np.pi * np.outer(k, k) / N)
    Sx = np.zeros((N, H))
    for i in range(N):
        Sx[i, min(max(i - KS, 0), H - 1)] = 1.0
    A = Fm @ Sx                                   # [294, 256]
    At = A.T                                      # [256, 294]
    B1 = np.conj(Fm)[KS:KS + H, :] / N            # [256, 294]
    wk = np.ones(KH)
    wk[1:KH - 1] = 2.0
    B4 = B1[:, :KH] * wk[None, :]                 # [256, 148]
    E = np.exp(-2j * np.pi * np.outer(k, (np.arange(KS) - KS // 2)) / N)  # [294,19]
    f = lambda v: _round_f32r(np.ascontiguousarray(v, np.float32))
    cst = {
        "AtR": f(At.real), "AtI": f(At.imag), "AtIn": f(-At.imag),
        "B1tR": f(B1.T.real), "B1tI": f(B1.T.imag), "B1tIn": f(-B1.T.imag),
        "B4R": f(B4.T.real), "B4In": f(-B4.T.imag),
        "ER": f(E.T.real), "EI": f(E.T.imag), "EIn": f(-E.T.imag),
        "onesr": np.ones((1, 128), np.float32),
        "onesc": f(np.ones((128, 1), np.float32)),
        "ident": np.eye(128, dtype=np.float32),
    }
    u = np.ones((H, 1), np.float32)
    u[0, 0] = 20.0
    u[-1, 0] = 20.0
    cst["u16"] = u.astype(np.float16)
    cst["u32"] = u
    return cst


_CACHE = {}


def _build():
    nc = bacc.Bacc(None, target_bir_lowering=False)
    P = {}
    P["wf8"] = nc.declare_dram_parameter("wf8", [1, UPN], I8, isOutput=False)
    for nm, shp in [("AtR", [NF, N]), ("AtI", [NF, N]), ("AtIn", [NF, N]),
                    ("B1tR", [N, H]), ("B1tI", [N, H]), ("B1tIn", [N, H]),
                    ("B4R", [KH, H]), ("B4In", [KH, H]),
                    ("ER", [KS, N]), ("EI", [KS, N]), ("EIn", [KS, N]),
                    ("onesc", [128, 1])]:
        P[nm] = nc.declare_dram_parameter(nm, shp, F32R, isOutput=False)
    P["onesr"] = nc.declare_dram_parameter("onesr", [1, 128], F32, isOutput=False)
    P["ident"] = nc.declare_dram_parameter("ident", [128, 128], F32, isOutput=False)
    P["u16"] = nc.declare_dram_parameter("u16", [H, 1], F16, isOutput=False)
    P["u32"] = nc.declare_dram_parameter("u32", [H, 1], F32, isOutput=False)
    clear8_d = nc.declare_dram_parameter("clear8", [1, DWN], U8, isOutput=True)

    with tile.TileContext(nc) as tc:
        _emit(nc, tc, P, clear8_d)
    nc.finalize()
    return nc


def _load_consts(nc, tc, ctx, P):
    cpool = ctx.enter_context(tc.tile_pool(name="consts", bufs=1))
    CT = {}
    for nm in ("AtR", "AtI", "AtIn"):
        t = cpool.tile([128, 2 * N], F32R, tag=nm)
        for i in range(2):
            nc.sync.dma_start(t[:, i * N:(i + 1) * N], P[nm][i * 128:(i + 1) * 128, :])
        CT[nm] = t
    for nm in ("B1tR", "B1tI", "B1tIn"):
        t = cpool.tile([128, 3 * H], F32R, tag=nm)
        for i in range(3):
            rs = LT[i]
            nc.sync.dma_start(t[0:rs, i * H:i * H + H], P[nm][i * 128:i * 128 + rs, :])
        CT[nm] = t
    for nm in ("B4R", "B4In"):
        t = cpool.tile([128, 2 * H], F32R, tag=nm)
        for i in range(2):
            rs = MT[i]
            nc.sync.dma_start(t[0:rs, i * H:i * H + H], P[nm][i * 128:i * 128 + rs, :])
        CT[nm] = t
    for nm in ("ER", "EI", "EIn"):
        t = cpool.tile([KS, N], F32R, tag=nm)
        nc.sync.dma_start(t[:], P[nm][:, :])
        CT[nm] = t
    for nm, shp, dt in [("onesr", [1, 128], F32), ("onesc", [128, 1], F32R),
                        ("ident", [128, 128], F32)]:
        t = cpool.tile(shp, dt, tag=nm)
        nc.sync.dma_start(t[:], P[nm][:, :])
        CT[nm] = t
    u16 = cpool.tile([128, 2], F16, tag="u16")
    for i in range(2):
        nc.sync.dma_start(u16[:, i:i + 1], P["u16"][i * 128:(i + 1) * 128, :])
    CT["u16"] = u16
    u32 = cpool.tile([128, 2], F32, tag="u32")
    for i in range(2):
        nc.sync.dma_start(u32[:, i:i + 1], P["u32"][i * 128:(i + 1) * 128, :])
    CT["u32"] = u32
    # sideband: per-channel upload scales + f32r PSF, bitcast out of the
    # flat int8 upload buffer
    scl_t = cpool.tile([1, CH], F32, tag="scl")
    nc.sync.dma_start(
        scl_t[:], P["wf8"][0, SB:SB + 4 * CH].bitcast(F32)
        .rearrange("(a w) -> a w", a=1))
    CT["scl"] = scl_t
    kern_sb = cpool.tile([KS, KS], F32R, tag="kern")
    nc.sync.dma_start(
        kern_sb[:], P["wf8"][0, SB + 4 * CH:SB + 4 * CH + 4 * KS * KS]
        .bitcast(F32R).rearrange("(p w) -> p w", w=KS))
    CT["kern"] = kern_sb
    # persistent P0 outputs
    CT["otfR"] = cpool.tile([128, 3 * KH], F32, name="otfR", tag="otfR")
    CT["otfI"] = cpool.tile([128, 3 * KH], F32, name="otfI", tag="otfI")
    CT["Dt"] = cpool.tile([128, 3 * KH], F32, name="Dtt", tag="Dtt")
    return CT


def _emit(nc, tc, P, clear8_d):
    with ExitStack() as ctx:
        CT = _load_consts(nc, tc, ctx, P)

        # ---- P0 (scoped pools) ----
        with tc.tile_pool(name="p01", bufs=3) as wp, \
             tc.tile_pool(name="p01ps", bufs=1, space="PSUM") as pp:
            _p0_otf(nc, wp, pp, CT)

        tc.strict_bb_all_engine_barrier()

        # ---- P2 ----
        with tc.tile_pool(name="sw", bufs=2) as spool, \
             tc.tile_pool(name="med", bufs=2) as mpool, \
             tc.tile_pool(name="p2ps", bufs=1, space="PSUM") as ppool:
            for ci in range(CH):
                _image(nc, spool, mpool, ppool, CT, P["wf8"], clear8_d, ci)


def _p0_otf(nc, wp, pp, CT):
    # Tt[j,k] = sum_i kern[i,j] E[k,i]: lhsT=kern [19,19], rhs=ER/EI [19,294]
    ps_a = pp.tile([KS, N], F32, tag="ps_otfa")
    ps_b = pp.tile([KS, N], F32, tag="ps_otfb")
    nc.tensor.matmul(ps_a[:], CT["kern"][:], CT["ER"][:], start=True, stop=True)
    nc.tensor.matmul(ps_b[:], CT["kern"][:], CT["EI"][:], start=True, stop=True)
    TtR = wp.tile([KS, N], F32R, tag="TtR")
    TtI = wp.tile([KS, N], F32R, tag="TtI")
    nc.vector.tensor_copy(TtR[:], ps_a[:])
    nc.vector.tensor_copy(TtI[:], ps_b[:])
    # otf_t[l,k] = sum_j E[l,j] Tt[j,k]: lhsT = E^T-as-[j,l] = ER/EI slices
    for lt in range(3):
        rs = LT[lt]
        lsl = slice(lt * 128, lt * 128 + rs)
        osl = slice(lt * KH, lt * KH + KH)
        ps_r = pp.tile([128, KH], F32, tag="ps_otfr")
        ps_i = pp.tile([128, KH], F32, tag="ps_otfi")
        nc.tensor.matmul(ps_r[0:rs, :], CT["ER"][:, lsl], TtR[:, 0:KH], start=True, stop=False)
        nc.tensor.matmul(ps_r[0:rs, :], CT["EIn"][:, lsl], TtI[:, 0:KH], start=False, stop=True)
        nc.tensor.matmul(ps_i[0:rs, :], CT["ER"][:, lsl], TtI[:, 0:KH], start=True, stop=False)
        nc.tensor.matmul(ps_i[0:rs, :], CT["EI"][:, lsl], TtR[:, 0:KH], start=False, stop=True)
        nc.vector.tensor_copy(CT["otfR"][0:rs, osl], ps_r[0:rs, :])
        nc.vector.tensor_copy(CT["otfI"][0:rs, osl], ps_i[0:rs, :])
        t2 = wp.tile([128, KH], F32, tag="d_tmp")
        nc.vector.tensor_mul(CT["Dt"][0:rs, osl], CT["otfR"][0:rs, osl], CT["otfR"][0:rs, osl])
        nc.vector.tensor_mul(t2[0:rs, :], CT["otfI"][0:rs, osl], CT["otfI"][0:rs, osl])
        nc.vector.tensor_add(CT["Dt"][0:rs, osl], CT["Dt"][0:rs, osl], t2[0:rs, :])


def _image(nc, spool, mpool, ppool, CT, wf8_p, clear8_d, ci):
    # --- loads: uint8(+128) bordered rows; m1/cc/p1 are row-shifted
    # views. bordered row r: 0 = replicate of image row 0, r = image row
    # r-1, 257 = replicate of image row 255. The convert activations
    # apply bias -128, so cc/m1/p1 hold the centered int values exactly.
    cb = ci * HB * W
    m8 = mpool.tile([128, 2 * W], U8, tag="m8")
    c8 = mpool.tile([128, 2 * W], U8, tag="c8")
    p8 = mpool.tile([128, 2 * W], U8, tag="p8")
    for t in range(2):
        base = t * 128
        for tl, off in ((m8, 0), (c8, 1), (p8, 2)):
            nc.sync.dma_start(
                tl[:, t * W:(t + 1) * W],
                wf8_p[0, cb + (base + off) * W:cb + (base + off + 128) * W]
                .bitcast(U8).rearrange("(p w) -> p w", w=W))
    r8 = mpool.tile([1, W], U8, tag="r8")
    nc.sync.dma_start(r8[:], wf8_p[0, cb + 256 * W:cb + 257 * W]
                      .bitcast(U8).rearrange("(a w) -> a w", a=1))
    m1 = mpool.tile([128, 2 * W], F16, tag="m1")
    cc = mpool.tile([128, 2 * W], F16, tag="cc")
    p1 = mpool.tile([128, 2 * W], F16, tag="p1s")
    r255 = mpool.tile([1, W], F16, tag="r255")
    nc.scalar.activation(m1[:], m8[:], ACT.Copy, bias=-128.0)
    nc.scalar.activation(cc[:], c8[:], ACT.Copy, bias=-128.0)
    nc.scalar.activation(p1[:], p8[:], ACT.Copy, bias=-128.0)
    nc.scalar.activation(r255[:], r8[:], ACT.Copy, bias=-128.0)
    # f32r image rows for the DFT path (== cc, the center rows)
    wfr = spool.tile([128, 2 * W], F32R, tag="wfimg")
    nc.scalar.copy(wfr[:], cc[:])

    # --- vertical clamped sort3 -> lo/me/hi ---
    lo = mpool.tile([128, 2 * W], F16, tag="lo")
    me = mpool.tile([128, 2 * W], F16, tag="me")
    hi = mpool.tile([128, 2 * W], F16, tag="hi")
    tv = mpool.tile([128, 2 * W], F16, tag="tv")
    mn = mpool.tile([128, 2 * W], F16, tag="mnv")
    mx = mpool.tile([128, 2 * W], F16, tag="mxv")
    nc.vector.tensor_tensor(mn[:], cc[:], p1[:], AT.min)
    nc.vector.tensor_tensor(mx[:], cc[:], p1[:], AT.max)
    nc.vector.tensor_tensor(lo[:], m1[:], mn[:], AT.min)
    nc.vector.tensor_tensor(hi[:], m1[:], mx[:], AT.max)
    nc.vector.tensor_tensor(tv[:], m1[:], mx[:], AT.min)
    nc.vector.tensor_tensor(me[:], tv[:], mn[:], AT.max)

    # --- horizontal stage -> Mx ---
    Ax = mpool.tile([128, 2 * W], F16, tag="Ax")
    Cx = mpool.tile([128, 2 * W], F16, tag="Cx")
    Bx = mpool.tile([128, 2 * W], F16, tag="Bx")
    Mx = mpool.tile([128, 2 * W], F16, tag="Mx")
    h1 = mpool.tile([128, 2 * W], F16, tag="h1")
    h2 = mpool.tile([128, 2 * W], F16, tag="h2")
    for t in range(2):
        o = t * W
        nc.vector.tensor_tensor(h1[:, o:o + W - 1], lo[:, o:o + W - 1], lo[:, o + 1:o + W], AT.max)
        nc.vector.tensor_tensor(Ax[:, o + 1:o + W - 1], h1[:, o:o + W - 2], lo[:, o + 2:o + W], AT.max)
        nc.vector.tensor_copy(Ax[:, o:o + 1], h1[:, o:o + 1])
        nc.vector.tensor_copy(Ax[:, o + W - 1:o + W], h1[:, o + W - 2:o + W - 1])
        nc.vector.tensor_tensor(h2[:, o:o + W - 1], hi[:, o:o + W - 1], hi[:, o + 1:o + W], AT.min)
        nc.vector.tensor_tensor(Cx[:, o + 1:o + W - 1], h2[:, o:o + W - 2], hi[:, o + 2:o + W], AT.min)
        nc.vector.tensor_copy(Cx[:, o:o + 1], h2[:, o:o + 1])
        nc.vector.tensor_copy(Cx[:, o + W - 1:o + W], h2[:, o + W - 2:o + W - 1])
        nc.vector.tensor_tensor(h1[:, o:o + W - 1], me[:, o:o + W - 1], me[:, o + 1:o + W], AT.min)
        nc.vector.tensor_tensor(h2[:, o:o + W - 1], me[:, o:o + W - 1], me[:, o + 1:o + W], AT.max)
        nc.vector.tensor_tensor(Bx[:, o + 1:o + W - 1], me[:, o:o + W - 2], h2[:, o + 1:o + W - 1], AT.min)
        nc.vector.tensor_tensor(Bx[:, o + 1:o + W - 1], Bx[:, o + 1:o + W - 1], h1[:, o + 1:o + W - 1], AT.max)
        nc.vector.tensor_copy(Bx[:, o:o + 1], me[:, o:o + 1])
        nc.vector.tensor_copy(Bx[:, o + W - 1:o + W], me[:, o + W - 1:o + W])
        nc.vector.tensor_tensor(h1[:, o:o + W], Ax[:, o:o + W], Bx[:, o:o + W], AT.min)
        nc.vector.tensor_tensor(h2[:, o:o + W], Ax[:, o:o + W], Bx[:, o:o + W], AT.max)
        nc.vector.tensor_tensor(h2[:, o:o + W], h2[:, o:o + W], Cx[:, o:o + W], AT.min)
        nc.vector.tensor_tensor(Mx[:, o:o + W], h2[:, o:o + W], h1[:, o:o + W], AT.max)

    # --- D stats ---
    Dc = mpool.tile([128, 2 * W], F16, tag="Dc")
    D2 = mpool.tile([128, 2 * W], F32, tag="D2")
    nc.vector.tensor_sub(Dc[:], Mx[:], cc[:])
    nc.vector.tensor_mul(D2[:], Dc[:], Dc[:])
    ST = spool.tile([128, 8], F32R, tag="ST")
    red = spool.tile([128, 8], F32, tag="red")
    nc.vector.reduce_sum(red[:, 0:2], Dc.rearrange("p (t w) -> p t w", t=2), axis=AX.X)
    nc.vector.reduce_sum(red[:, 2:4], D2.rearrange("p (t w) -> p t w", t=2), axis=AX.X)
    nc.vector.tensor_add(ST[:, 0:1], red[:, 0:1], red[:, 1:2])
    nc.vector.tensor_add(ST[:, 1:2], red[:, 2:3], red[:, 3:4])
    # left/right vertical-median strips (weight 19)
    sl_ = spool.tile([128, 12], F32, tag="strips")
    nc.vector.tensor_sub(sl_[:, 0:1], me[:, 0:1], cc[:, 0:1])
    nc.vector.tensor_sub(sl_[:, 1:2], me[:, W - 1:W], cc[:, W - 1:W])
    nc.vector.tensor_sub(sl_[:, 2:3], me[:, W:W + 1], cc[:, W:W + 1])
    nc.vector.tensor_sub(sl_[:, 3:4], me[:, 2 * W - 1:2 * W], cc[:, 2 * W - 1:2 * W])
    for q in range(4):
        nc.vector.tensor_mul(sl_[:, 4 + q:5 + q], sl_[:, q:q + 1], sl_[:, q:q + 1])
    nc.vector.tensor_add(sl_[:, 8:9], sl_[:, 0:1], sl_[:, 1:2])
    nc.vector.tensor_add(sl_[:, 9:10], sl_[:, 2:3], sl_[:, 3:4])
    nc.vector.tensor_add(sl_[:, 8:9], sl_[:, 8:9], sl_[:, 9:10])
    nc.vector.tensor_add(sl_[:, 10:11], sl_[:, 4:5], sl_[:, 5:6])
    nc.vector.tensor_add(sl_[:, 11:12], sl_[:, 6:7], sl_[:, 7:8])
    nc.vector.tensor_add(sl_[:, 10:11], sl_[:, 10:11], sl_[:, 11:12])
    nc.vector.tensor_scalar_mul(ST[:, 2:3], sl_[:, 8:9], 19.0)
    nc.vector.tensor_scalar_mul(ST[:, 3:4], sl_[:, 10:11], 19.0)
    # zero ST[0:1, 4:6] then accumulate the two 1-D strips (x19 inside)
    nc.vector.tensor_scalar_mul(ST[0:1, 4:6], CT["onesr"][0:1, 0:2], 0.0)
    _strip1d(nc, spool, ST, cc[0:1, 0:W])
    _strip1d(nc, spool, ST, r255[0:1, 0:W])

    nsr128 = _nsr(nc, spool, mpool, ppool, CT, ST, cc, ci)

    # --- S1: Ut[c,k] = sum_r wf[r,c] At[r,k] ---
    ps_ut = ppool.tile([128, 2 * KH], F32, tag="ps_ut")
    ps_uti = ppool.tile([128, 2 * KH], F32, tag="ps_uti")
    for mt in range(2):
        for kt in range(2):
            lhs = wfr[:, kt * W + mt * 128: kt * W + mt * 128 + 128]
            nc.tensor.matmul(ps_ut[:, mt * KH:(mt + 1) * KH], lhs,
                             CT["AtR"][:, kt * N: kt * N + KH],
                             start=(kt == 0), stop=(kt == 1))
            nc.tensor.matmul(ps_uti[:, mt * KH:(mt + 1) * KH], lhs,
                             CT["AtI"][:, kt * N: kt * N + KH],
                             start=(kt == 0), stop=(kt == 1))
    UtR = spool.tile([128, 2 * KH], F32R, tag="UtR")
    UtI = spool.tile([128, 2 * KH], F32R, tag="UtI")
    nc.vector.tensor_copy(UtR[:], ps_ut[:])
    nc.scalar.copy(UtI[:], ps_uti[:])

    # --- S2: Gt[l,k] = sum_c At[c,l] Ut[c,k] (complex) ---
    ps_gr = ppool.tile([128, 3 * KH], F32, tag="ps_gr")
    ps_gi = ppool.tile([128, 3 * KH], F32, tag="ps_gi")
    for lt in range(3):
        rs = LT[lt]
        o = slice(lt * KH, lt * KH + KH)
        for kt in range(2):
            lA = slice(kt * N + lt * 128, kt * N + lt * 128 + rs)
            uR = UtR[:, kt * KH:(kt + 1) * KH]
            uI = UtI[:, kt * KH:(kt + 1) * KH]
            nc.tensor.matmul(ps_gr[0:rs, o], CT["AtR"][:, lA], uR, start=(kt == 0), stop=False)
            nc.tensor.matmul(ps_gi[0:rs, o], CT["AtR"][:, lA], uI, start=(kt == 0), stop=False)
        for kt in range(2):
            lA = slice(kt * N + lt * 128, kt * N + lt * 128 + rs)
            uR = UtR[:, kt * KH:(kt + 1) * KH]
            uI = UtI[:, kt * KH:(kt + 1) * KH]
            nc.tensor.matmul(ps_gr[0:rs, o], CT["AtIn"][:, lA], uI, start=False, stop=(kt == 1))
            nc.tensor.matmul(ps_gi[0:rs, o], CT["AtI"][:, lA], uR, start=False, stop=(kt == 1))

    # --- Wiener elementwise -> Zt (f32r) ---
    ZtR = spool.tile([128, 3 * KH], F32R, tag="ZtR")
    ZtI = spool.tile([128, 3 * KH], F32R, tag="ZtI")
    Rv = spool.tile([128, 3 * KH], F32, tag="Rv")
    t1 = spool.tile([128, KH], F32, tag="wt1")
    t2 = spool.tile([128, KH], F32, tag="wt2")
    for lt in range(3):
        rs = LT[lt]
        o = slice(lt * KH, lt * KH + KH)
        nc.vector.tensor_scalar_add(Rv[0:rs, o], CT["Dt"][0:rs, o], nsr128[0:rs, 0:1])
        nc.vector.reciprocal(Rv[0:rs, o], Rv[0:rs, o])
        nc.vector.tensor_mul(t1[0:rs, :], ps_gr[0:rs, o], CT["otfR"][0:rs, o])
        nc.vector.tensor_mul(t2[0:rs, :], ps_gi[0:rs, o], CT["otfI"][0:rs, o])
        nc.vector.tensor_add(t1[0:rs, :], t1[0:rs, :], t2[0:rs, :])
        nc.vector.tensor_mul(ZtR[0:rs, o], t1[0:rs, :], Rv[0:rs, o])
        nc.vector.tensor_mul(t1[0:rs, :], ps_gi[0:rs, o], CT["otfR"][0:rs, o])
        nc.vector.tensor_mul(t2[0:rs, :], ps_gr[0:rs, o], CT["otfI"][0:rs, o])
        nc.vector.tensor_sub(t1[0:rs, :], t1[0:rs, :], t2[0:rs, :])
        nc.vector.tensor_mul(ZtI[0:rs, o], t1[0:rs, :], Rv[0:rs, o])

    # --- S3: V[k,j] = sum_l Zt[l,k] B1t[l,j] (complex) ---
    ps_vr = ppool.tile([128, 2 * W], F32, tag="ps_vr")
    ps_vi = ppool.tile([128, 2 * W], F32, tag="ps_vi")
    for mt in range(2):
        ms = MT[mt]
        o = slice(mt * W, mt * W + W)
        for lt in range(3):
            rs = LT[lt]
            zR = ZtR[0:rs, lt * KH + mt * 128: lt * KH + mt * 128 + ms]
            zI = ZtI[0:rs, lt * KH + mt * 128: lt * KH + mt * 128 + ms]
            bR = CT["B1tR"][0:rs, lt * H:(lt + 1) * H]
            bI = CT["B1tI"][0:rs, lt * H:(lt + 1) * H]
            bIn = CT["B1tIn"][0:rs, lt * H:(lt + 1) * H]
            nc.tensor.matmul(ps_vr[0:ms, o], zR, bR, start=(lt == 0), stop=False)
            nc.tensor.matmul(ps_vi[0:ms, o], zR, bI, start=(lt == 0), stop=False)
            nc.tensor.matmul(ps_vr[0:ms, o], zI, bIn, start=False, stop=(lt == 2))
            nc.tensor.matmul(ps_vi[0:ms, o], zI, bR, start=False, stop=(lt == 2))
    VR = spool.tile([128, 2 * W], F32R, tag="VR")
    VI = spool.tile([128, 2 * W], F32R, tag="VI")
    nc.vector.tensor_copy(VR[:, 0:W], ps_vr[:, 0:W])
    nc.scalar.copy(VI[:, 0:W], ps_vi[:, 0:W])
    nc.vector.tensor_copy(VR[0:MT[1], W:2 * W], ps_vr[0:MT[1], W:2 * W])
    nc.scalar.copy(VI[0:MT[1], W:2 * W], ps_vi[0:MT[1], W:2 * W])

    # --- S4: img[i,j] = sum_k B4[i,k] VR[k,j] - B4i[i,k] VI[k,j] ---
    ps_o = ppool.tile([128, 2 * W], F32, tag="ps_img")
    ps_oa = ps_o[:, 0:W]
    ps_ob = ps_o[:, W:2 * W]
    for mt, ps in ((0, ps_oa), (1, ps_ob)):
        for kt in range(2):
            ks = MT[kt]
            b4r = CT["B4R"][0:ks, kt * H + mt * 128: kt * H + mt * 128 + 128]
            b4i = CT["B4In"][0:ks, kt * H + mt * 128: kt * H + mt * 128 + 128]
            nc.tensor.matmul(ps[:], b4r, VR[0:ks, kt * W:(kt + 1) * W],
                             start=(kt == 0), stop=False)
            nc.tensor.matmul(ps[:], b4i, VI[0:ks, kt * W:(kt + 1) * W],
                             start=False, stop=(kt == 1))

    # --- uint8 encode with ONE absmax scale per channel (so the host can
    # fold dequant into the expand weights): per-partition |max| via
    # reduce, cross-partition max via PE transpose against identity,
    # then q = rne(clear_q * 127/max + 128) (the f32->uint8 cast is RNE
    # and saturating, verified on HW).
    pm2 = spool.tile([128, 2], F32, tag="pm2")
    nc.vector.reduce_max(pm2[:], ps_o.rearrange("p (t w) -> p t w", t=2),
                         axis=AX.X, apply_absolute_value=True)
    pm = spool.tile([128, 1], F32, tag="pm")
    nc.vector.tensor_tensor(pm[:], pm2[:, 0:1], pm2[:, 1:2], AT.max)
    ps_t = ppool.tile([1, 128], F32, tag="ps_small")
    nc.tensor.matmul(ps_t[:], pm[:], CT["ident"][:], start=True, stop=True)
    m1t = spool.tile([1, 2], F32, tag="m1t")
    nc.vector.reduce_max(m1t[:, 0:1], ps_t[:], axis=AX.X)
    nc.vector.reciprocal(m1t[:, 1:2], m1t[:, 0:1])
    nc.vector.tensor_scalar_mul(m1t[:, 1:2], m1t[:, 1:2], 127.0)
    ps_e = ppool.tile([128, 1], F32, tag="ps_small")
    nc.tensor.matmul(ps_e[:], CT["onesr"][:], m1t[:, 1:2], start=True, stop=True)
    e128 = spool.tile([128, 1], F32, tag="e128")
    nc.vector.tensor_copy(e128[:], ps_e[:])
    img8 = spool.tile([128, 2 * W], U8, tag="img8")
    nc.scalar.activation(img8[:, 0:W], ps_oa, ACT.Copy,
                         bias=128.0, scale=e128[:, 0:1])
    nc.scalar.activation(img8[:, W:2 * W], ps_ob, ACT.Copy,
                         bias=128.0, scale=e128[:, 0:1])
    nc.sync.dma_start(
        clear8_d[0, CH * NPIX + 4 * ci:CH * NPIX + 4 * ci + 4].bitcast(F32)
        .rearrange("(a w) -> a w", a=1), m1t[:, 0:1])
    for t in range(2):
        nc.sync.dma_start(
            clear8_d[0, ci * NPIX + t * 128 * W:ci * NPIX + (t + 1) * 128 * W]
            .bitcast(U8).rearrange("(p w) -> p w", w=W),
            img8[:, t * W:(t + 1) * W])


def _strip1d(nc, spool, ST, row):
    """Accumulate 19*(sum, sumsq) of (clamped-1D-med3(row) - row) into
    ST[0:1, 4:6]. row: [1, W] fp16."""
    h = spool.tile([1, 4 * W], F32, tag="strip1d")
    mnp = h[:, 0:W - 1]
    mxp = h[:, W:2 * W - 1]
    med = h[:, 2 * W:3 * W]
    nc.vector.tensor_tensor(mnp[:], row[:, 0:W - 1], row[:, 1:W], AT.min)
    nc.vector.tensor_tensor(mxp[:], row[:, 0:W - 1], row[:, 1:W], AT.max)
    nc.vector.tensor_tensor(h[:, 3 * W + 1:4 * W - 1], row[:, 0:W - 2],
                            mxp[:, 1:W - 1], AT.min)
    nc.vector.tensor_tensor(med[:, 1:W - 1], h[:, 3 * W + 1:4 * W - 1],
                            mnp[:, 1:W - 1], AT.max)
    nc.vector.tensor_copy(med[:, 0:1], row[:, 0:1])
    nc.vector.tensor_copy(med[:, W - 1:W], row[:, W - 1:W])
    d = h[:, 3 * W:4 * W]
    nc.vector.tensor_sub(d[:], med[:], row[:])
    s = spool.tile([1, 2], F32, tag="strip1d_s")
    nc.vector.reduce_sum(s[:, 0:1], d[:], axis=AX.X)
    d2 = h[:, 0:W]
    nc.vector.tensor_mul(d2[:], d[:], d[:])
    nc.vector.reduce_sum(s[:, 1:2], d2[:], axis=AX.X)
    nc.vector.tensor_scalar_mul(s[:], s[:], 19.0)
    nc.vector.tensor_add(ST[0:1, 4:6], ST[0:1, 4:6], s[:, 0:2])


def _nsr(nc, spool, mpool, ppool, CT, ST, cc, ci):
    u16 = CT["u16"]
    wsq = mpool.tile([128, 2 * W], F16, tag="wsq")
    nc.vector.tensor_mul(wsq[:], cc[:], cc[:])
    # u-weighted rowsums: rowsum[c] for c = m*128+p; cols (2m, 2m+1) = (W, W2)
    ps_u = ppool.tile([128, 8], F32, tag="ps_small")
    for m in range(2):
        for t in range(2):
            nc.tensor.matmul(ps_u[:, 2 * m:2 * m + 1],
                             cc[:, t * W + m * 128: t * W + m * 128 + 128],
                             u16[:, t:t + 1], start=(t == 0), stop=(t == 1))
            nc.tensor.matmul(ps_u[:, 2 * m + 1:2 * m + 2],
                             wsq[:, t * W + m * 128: t * W + m * 128 + 128],
                             u16[:, t:t + 1], start=(t == 0), stop=(t == 1))
    # rsum in f32: with int8-scaled inputs the wsq rowsums reach ~5e6,
    # which overflows fp16.
    rsum = spool.tile([128, 4], F32, tag="rsum32")
    nc.vector.tensor_copy(rsum[:], ps_u[:, 0:4])
    # column weights: m=0 cols weighted by u[:,0] (20 at c=0), m=1 by u[:,1]
    u32 = CT["u32"]
    ps_t = ppool.tile([4, 8], F32, tag="ps_small")
    nc.tensor.matmul(ps_t[0:1, 0:2], u32[:, 0:1], rsum[:, 0:2], start=True, stop=True)
    nc.tensor.matmul(ps_t[0:1, 2:4], u32[:, 1:2], rsum[:, 2:4], start=True, stop=True)
    # D-stat cross-partition sums
    nc.tensor.matmul(ps_t[0:1, 4:8], CT["onesc"][:], ST[:, 0:4], start=True, stop=True)
    sW = spool.tile([1, 16], F32, tag="scal")
    nc.vector.tensor_copy(sW[:, 0:8], ps_t[0:1, 0:8])
    # sums: u-wf = c0+c2 ; u-wf2 = c1+c3 ; sD = c4+c6+ST[0,4] ; sD2 = c5+c7+ST[0,5]
    nc.vector.tensor_add(sW[:, 8:9], sW[:, 0:1], sW[:, 2:3])
    nc.vector.tensor_add(sW[:, 9:10], sW[:, 1:2], sW[:, 3:4])
    nc.vector.tensor_add(sW[:, 10:11], sW[:, 4:5], sW[:, 6:7])
    nc.vector.tensor_add(sW[:, 10:11], sW[:, 10:11], ST[0:1, 4:5])
    nc.vector.tensor_add(sW[:, 11:12], sW[:, 5:6], sW[:, 7:8])
    nc.vector.tensor_add(sW[:, 11:12], sW[:, 11:12], ST[0:1, 5:6])
    ninv, nm1inv = 1.0 / NTOT, 1.0 / (NTOT - 1)
    sc = spool.tile([1, 8], F32, tag="scal2")
    # var_n = (sD2 - sD^2/n)/(n-1)
    nc.vector.tensor_mul(sc[:, 0:1], sW[:, 10:11], sW[:, 10:11])
    nc.vector.tensor_scalar_mul(sc[:, 0:1], sc[:, 0:1], ninv)
    nc.vector.tensor_sub(sc[:, 0:1], sW[:, 11:12], sc[:, 0:1])
    nc.vector.tensor_scalar_mul(sc[:, 0:1], sc[:, 0:1], nm1inv)
    # var_s
    nc.vector.tensor_mul(sc[:, 1:2], sW[:, 8:9], sW[:, 8:9])
    nc.vector.tensor_scalar_mul(sc[:, 1:2], sc[:, 1:2], ninv)
    nc.vector.tensor_sub(sc[:, 1:2], sW[:, 9:10], sc[:, 1:2])
    nc.vector.tensor_scalar_mul(sc[:, 1:2], sc[:, 1:2], nm1inv)
    # rsqrt(var_s) via recip seed + NR. var_s of the int8-scaled image is
    # ~(127/4.7)^2 ~ 730, so seed 27/v ~ 1/sqrt(v) within +-7% (a bare
    # 1/v seed is 27x off and 4 NR steps cannot recover it).
    nc.vector.reciprocal(sc[:, 2:3], sc[:, 1:2])
    nc.vector.tensor_scalar_mul(sc[:, 2:3], sc[:, 2:3], 27.0)
    for _ in range(4):
        nc.vector.tensor_mul(sc[:, 3:4], sc[:, 2:3], sc[:, 2:3])
        nc.vector.tensor_mul(sc[:, 3:4], sc[:, 3:4], sc[:, 1:2])
        nc.vector.tensor_scalar(sc[:, 3:4], sc[:, 3:4], -0.5, 1.5, AT.mult, AT.add)
        nc.vector.tensor_mul(sc[:, 2:3], sc[:, 2:3], sc[:, 3:4])
    nc.vector.tensor_mul(sc[:, 4:5], sc[:, 0:1], sc[:, 2:3])
    nc.vector.tensor_scalar_mul(sc[:, 4:5], sc[:, 4:5], 8.0 / 30.0)
    # nsr was computed from wf_q = wf/s_c; true nsr scales linearly in
    # amplitude, so multiply by the upload scale s_c.
    nsr_r = spool.tile([1, 1], F32, tag="nsr_r")
    nc.vector.tensor_mul(nsr_r[:], sc[:, 4:5], CT["scl"][0:1, ci:ci + 1])
    ps_b = ppool.tile([128, 8], F32, tag="ps_small")
    nc.tensor.matmul(ps_b[:, 0:1], CT["onesr"][:], nsr_r[:], start=True, stop=True)
    nsr128 = spool.tile([128, 1], F32, tag="nsr128")
    nc.vector.tensor_copy(nsr128[:], ps_b[:, 0:1])
    return nsr128


def _make_exec(nc, devs):
    """Build a sharded jit executable over the given device subset."""
    import jax
    from jax.sharding import Mesh, PartitionSpec, NamedSharding
    from jax.experimental.shard_map import shard_map
    from concourse import bass2jax

    bass2jax.install_neuronx_cc_hook()
    partition_name = nc.partition_id_tensor.name if nc.partition_id_tensor else None
    in_names, out_names, out_avals = [], [], []
    for alloc in nc.m.functions[0].allocations:
        if not isinstance(alloc, mybir.MemoryLocationSet):
            continue
        name = alloc.memorylocations[0].name
        if alloc.kind == "ExternalInput":
            if name != partition_name:
                in_names.append(name)
        elif alloc.kind == "ExternalOutput":
            out_names.append(name)
            out_avals.append(jax.core.ShapedArray(
                tuple(alloc.tensor_shape), mybir.dt.np(alloc.dtype)))
    all_in_names = in_names + out_names + ([partition_name] if partition_name else [])

    def _body(*args):
        ops = list(args)
        if partition_name is not None:
            ops.append(bass2jax.partition_id_tensor())
        outs = bass2jax._bass_exec_p.bind(
            *ops, out_avals=tuple(out_avals), in_names=tuple(all_in_names),
            out_names=tuple(out_names), lowering_input_output_aliases=(),
            sim_require_finite=True, sim_require_nnan=True, nc=nc)
        return tuple(outs)

    mesh = Mesh(np.asarray(devs), ("core",))
    n_args = len(in_names) + len(out_names)
    sharded = jax.jit(
        shard_map(_body, mesh=mesh,
                  in_specs=(PartitionSpec("core"),) * n_args,
                  out_specs=(PartitionSpec("core"),) * len(out_names),
                  check_rep=False),
        keep_unused=True)
    sh = NamedSharding(mesh, PartitionSpec("core"))
    return sharded, sh, in_names, out_names, out_avals


def _setup():
    """First-call setup: build program, four 2-core jits (batch b on
    cores 2b, 2b+1), push resident buffers."""
    import jax
    nc = _build()
    consts = _host_constants()
    devices = jax.devices()[:8]
    groups = []
    for g in range(4):
        dd = devices[2 * g:2 * g + 2]
        sharded, sh, in_names, out_names, out_avals = _make_exec(nc, dd)
        resident = {}
        for nm, arr in consts.items():
            resident[nm] = jax.device_put(
                np.concatenate([np.asarray(arr)] * 2, axis=0), sh)
        zeros = [jax.device_put(
            np.zeros((2 * a.shape[0], *a.shape[1:]), a.dtype), sh)
            for a in out_avals]
        # AOT-compile so per-call dispatch skips jit tracing/validation
        d_wf0 = jax.device_put(np.zeros((2, UPN), np.int8), sh)
        full_args = [d_wf0 if nm == "wf8" else resident[nm]
                     for nm in in_names] + zeros
        compiled = sharded.lower(*full_args).compile()
        groups.append(dict(sharded=sharded, exec=compiled, sh=sh,
                           resident=resident, zeros=zeros, devs=list(dd)))
    _CACHE.update(nc=nc, groups=groups, in_names=in_names,
                  devices=list(devices))


def kernel(x, kernel, w_reduce, b_reduce, w_expand, b_expand):
    import jax
    import time
    if "groups" not in _CACHE:
        _setup()
    dbg = os.environ.get("DWDN_T")
    t00 = time.monotonic()

    def _tr(msg):
        if dbg:
            print(f"[{time.monotonic() - t00:7.3f}] {msg}", flush=True)

    devices = _CACHE["devices"]
    in_names = _CACHE["in_names"]

    x = np.asarray(x, np.float32).reshape(B, NF, NPIX)
    kern = np.asarray(kernel, np.float32)
    w_reduce = np.asarray(w_reduce, np.float32)
    b_reduce = np.asarray(b_reduce, np.float32)
    w_expand = np.asarray(w_expand, np.float32)
    b_expand = np.asarray(b_expand, np.float32)

    if "wf8_buf" not in _CACHE:
        _CACHE["wf8_buf"] = np.zeros((8, UPN), np.int8)
        _CACHE["y_buf"] = np.empty((C, NPIX), np.float32)
        _CACHE["t_buf"] = np.empty((CH, NPIX), np.float32)
        claugs = np.empty((B, C + 1, NPIX), np.float32)
        claugs[:, C] = 1.0
        _CACHE["claug_bufs"] = claugs
    wf8 = _CACHE["wf8_buf"]
    ybuf = _CACHE["y_buf"]
    tbuf = _CACHE["t_buf"]
    claugs = _CACHE["claug_bufs"]
    if "out_buf" not in _CACHE:
        _CACHE["out_buf"] = np.empty((B, NF, NPIX), np.float32)
    out = _CACHE["out_buf"]
    s_up = np.empty((B, 2, CH), np.float32)

    # quantize half-batch rows into core's flat buffer + sideband; the
    # uint8 cast happens inside a fused saturating numpy add. The reduce
    # bias is folded into the quantize offset (scale from the bound
    # max|y|+|b| so no clipping is possible), skipping a full bias pass.
    def _pack(core, rows, b, h):
        bsl = b_reduce[CH * h:CH * h + CH]
        amax = np.maximum(rows.max(axis=1), -rows.min(axis=1))
        amax += np.abs(bsl)
        np.maximum(amax, 1e-20, out=amax)
        s_up[b, h] = amax / 127.0
        sc127 = 127.0 / amax
        np.multiply(rows, sc127[:, None], out=tbuf)
        blk = wf8[core, :SB].view(np.uint8).reshape(CH, HB, W)
        np.add(tbuf.reshape(CH, H, W), (128.5 + bsl * sc127)[:, None, None],
               out=blk[:, 1:H + 1, :], casting='unsafe')
        blk[:, 0, :] = blk[:, 1, :]
        blk[:, H + 1, :] = blk[:, H, :]
        side = wf8[core, SB:SB + 4 * CH + 4 * KS * KS]
        side[0:4 * CH] = s_up[b, h].view(np.int8)
        side[4 * CH:] = _round_f32r(kern[b, 0]).ravel().view(np.int8)
        _tr(f"pack done core {core}")

    cs = np.empty((B, C), np.float32)

    # fully async pipeline (the host has ONE cpu): device_put / exec
    # dispatch / copy_to_host_async all return in ~0.3-10 ms and the
    # transfers+execs stream in the background. The main thread does
    # only cpu work (per-half sgemm + pack, then decode + expand); a
    # single io thread issues the puts/dispatches so their ~10 ms calls
    # (mostly GIL-releasing waits) stay off the producer's critical
    # path.
    import queue
    io_q = queue.Queue()
    done_q = queue.Queue()

    def _io():
        for b in range(B):
            G = _CACHE["groups"][b]
            shards = []
            for h in range(2):
                core = io_q.get()
                shards.append(jax.device_put(
                    wf8[core].reshape(1, UPN), devices[core]))
                _tr(f"put issued core {core}")
            d_wf = jax.make_array_from_single_device_arrays(
                (2, UPN), G["sh"], shards)
            args = [d_wf if nm == "wf8" else G["resident"][nm]
                    for nm in in_names]
            outs = G["exec"](*args, *G["zeros"])
            osh = sorted(outs[0].addressable_shards,
                         key=lambda s: s.index[0].start)
            for s in osh:
                s.data.copy_to_host_async()
            done_q.put(osh)
            _tr(f"dispatched b {b}")

    import threading
    io_t = threading.Thread(target=_io)
    io_t.start()
    for b in range(B):
        for h in range(2):
            yh = np.dot(w_reduce[CH * h:CH * h + CH], x[b],
                        out=ybuf[CH * h:CH * h + CH])
            _pack(2 * b + h, yh, b, h)
            io_q.put(2 * b + h)

    for b in range(B):
        for h, s in enumerate(done_q.get()):
            raw = np.asarray(s.data).ravel()
            _tr(f"fetch done b {b} h {h}")
            m = raw[CH * NPIX:CH * NPIX + 4 * CH].view(np.float32)
            claugs[b, CH * h:CH * h + CH] = raw[:CH * NPIX].reshape(CH, NPIX)
            cs[b, CH * h:CH * h + CH] = s_up[b, h] * (m / 127.0)
        # fold dequant into the expand weights: clear = (q-128)*cs
        Wb = w_expand * cs[b][None, :]
        Waug = np.concatenate(
            [Wb, (b_expand - 128.0 * Wb.sum(axis=1))[:, None]], axis=1)
        np.dot(Waug, claugs[b], out=out[b])
        _tr(f"expand done b {b}")

    return out.reshape(B, NF, H, W)


if __name__ == "__main__":
    import reference
    inputs = reference.setup_inputs()
    out = kernel(**{k: np.asarray(v) for k, v in inputs.items()})
    print("out", out.shape, out.dtype)
